# revision 1
# baseline (speedup 1.0000x reference)
"""Trainium2 Bass kernel for the KGEncoder RGCN (nn_KGEncoder_14027363188782).

Math (per batch element b, L=5 layers):
    x0 = ent_emb                                             (E, D)
    per layer i:
      y_r   = x @ Wb_x[i,r] + 1 * c[i,r]^T    (E, NB)  where c[i,r] = rel_r @ Wb_rel[i,r]
      Z     = sum_r adj_r @ y_r               (E, NB)  == sup @ Wb[i]  (deg term folded via c)
      h     = relu(Z @ Ww[i] + bias[i])
      g     = sigmoid(h @ Wh[i] + bh[i])
      x     = x + g * (h - x)
    out_b = sum_e x[e] * m[e] / max(sum_e m[e], 1)

Sharding: core c handles b = c // 2 (pair-replicated, no collectives).
adj is shipped pre-transposed (j-major) in bf16 (exact for 0/1 values).
Big matmul: out Z.T (NB x E) = sum_{r,k} y'[kchunk]_r.T @ adjT_r[kchunk];
NRES relations stay resident in SBUF, the rest stream from HBM each layer.
"""

import numpy as np
import ml_dtypes

import concourse.bacc as bacc
import concourse.bass as bass
import concourse.mybir as mybir
import concourse.tile as tile
from concourse import bass_utils
from concourse.bass import MemorySpace

B, R, E, D, HID, L, NB = 4, 10, 1500, 100, 100, 5, 3
EP = 1536           # entity (j) dim padded to 12*128
CH = EP // 128      # 12 k-chunks
FP8 = True          # fp8 adj (exact for 0/1) -> all relations SBUF-resident
DR = True           # DoubleRow fp8 matmul: 256-deep contraction, 2 elem/lane/cyc
C2 = 6              # 256-row contraction chunks (DoubleRow)
E2 = 1504           # i dim padded to 16-aligned for DoubleRow strides
YQ = 32             # y_all per-chunk col stride (16-aligned)
NRES = 10 if FP8 else 4   # relations resident in SBUF
SG = 3              # k-chunks per streamed stage tile
NW = 500            # psum free-dim chunk (3 per row of E)
RNB = R * NB        # 30
f32 = mybir.dt.float32
bf16 = mybir.dt.bfloat16
ADT = mybir.dt.float8e4 if FP8 else mybir.dt.bfloat16
ADT_NP = ml_dtypes.float8_e4m3fn if FP8 else ml_dtypes.bfloat16
AF = mybir.ActivationFunctionType
AX = mybir.AxisListType

_NC_CACHE = {}


def _build_nc():
    nc = bacc.Bacc("TRN2", target_bir_lowering=False, debug=False)

    if DR:
        adjT = nc.dram_tensor(
            "adjT", [R, C2, 128, 2, E2], ADT, kind="ExternalInput"
        ).ap()
    else:
        adjT = nc.dram_tensor("adjT", [R, EP, E], ADT, kind="ExternalInput").ap()
    xT0 = nc.dram_tensor("xT0", [D, E], f32, kind="ExternalInput").ap()
    maskrep = nc.dram_tensor("maskrep", [HID, E], f32, kind="ExternalInput").ap()
    relT = nc.dram_tensor("relT", [D, R], f32, kind="ExternalInput").ap()
    wbxD = nc.dram_tensor("wbx", [L, D, RNB], f32, kind="ExternalInput").ap()
    wbrD = nc.dram_tensor("wbr", [L, D, RNB], f32, kind="ExternalInput").ap()
    wwD = nc.dram_tensor("ww", [L, NB, HID], f32, kind="ExternalInput").ap()
    whD = nc.dram_tensor("wh", [L, HID, HID], f32, kind="ExternalInput").ap()
    biasD = nc.dram_tensor("biasL", [L, HID], f32, kind="ExternalInput").ap()
    bhD = nc.dram_tensor("bhL", [L, HID], f32, kind="ExternalInput").ap()
    graphD = nc.dram_tensor("graph", [HID, 1], f32, kind="ExternalOutput").ap()

    with tile.TileContext(nc) as tc:
        with (
            tc.tile_pool(name="singles", bufs=1) as singles,
            tc.tile_pool(name="resp", bufs=1) as resp,
            tc.tile_pool(name="stagep", bufs=4) as stagep,
            tc.tile_pool(name="ypool", bufs=2) as ypool,
            tc.tile_pool(name="workp", bufs=2) as workp,
            tc.tile_pool(name="psY", bufs=1, space=MemorySpace.PSUM) as psY,
            tc.tile_pool(name="psC", bufs=1, space=MemorySpace.PSUM) as psC,
            tc.tile_pool(name="psB", bufs=1, space=MemorySpace.PSUM) as psB,
        ):
            # ---- persistent state ----
            xT = singles.tile([D, EP], f32, tag="xT", name="xT")
            nc.sync.dma_start(out=xT[:, 0:E], in_=xT0)
            nc.vector.memset(xT[:, E:EP], 0.0)

            ones = singles.tile([1, 128], f32, tag="ones", name="ones")
            nc.vector.memset(ones[:, :], 1.0)

            mask_sb = singles.tile([HID, E], f32, tag="mask", name="mask_sb")
            nc.sync.dma_start(out=mask_sb[:, :], in_=maskrep)

            relT_sb = singles.tile([D, R], f32, tag="relT", name="relT_sb")
            nc.sync.dma_start(out=relT_sb[:, :], in_=relT)

            wbx_sb, wbr_sb, ww_sb, wh_sb, bias_sb, bh_sb = [], [], [], [], [], []
            for i in range(L):
                wx = singles.tile([D, RNB], f32, tag=f"wbx{i}", name=f"wbx{i}")
                nc.sync.dma_start(out=wx[:, :], in_=wbxD[i])
                wbx_sb.append(wx)
                wr = singles.tile([D, RNB], f32, tag=f"wbr{i}", name=f"wbr{i}")
                nc.sync.dma_start(out=wr[:, :], in_=wbrD[i])
                wbr_sb.append(wr)
                wwt = singles.tile([NB, HID], f32, tag=f"ww{i}", name=f"ww{i}")
                nc.sync.dma_start(out=wwt[:, :], in_=wwD[i])
                ww_sb.append(wwt)
                wht = singles.tile([HID, HID], f32, tag=f"wh{i}", name=f"wh{i}")
                nc.sync.dma_start(out=wht[:, :], in_=whD[i])
                wh_sb.append(wht)
                bt = singles.tile([HID, 1], f32, tag=f"bias{i}", name=f"bias{i}")
                nc.sync.dma_start(out=bt[:, :], in_=biasD[i].unsqueeze(1))
                bias_sb.append(bt)
                bht = singles.tile([HID, 1], f32, tag=f"bh{i}", name=f"bh{i}")
                nc.sync.dma_start(out=bht[:, :], in_=bhD[i].unsqueeze(1))
                bh_sb.append(bht)

            # resident adjT relations: tile (128, CH*E), chunk k at cols [k*E, (k+1)*E)
            res_tiles = []
            for r in range(NRES):
                if DR:
                    rt = resp.tile([128, C2 * 2 * E2], ADT,
                                   tag=f"res{r}", name=f"res{r}")
                    nc.sync.dma_start(
                        out=rt[:, :].rearrange("p (c t i) -> p c t i", c=C2, t=2),
                        in_=adjT[r].rearrange("c p t i -> p c t i"),
                    )
                else:
                    rt = resp.tile([128, CH * E], ADT, tag=f"res{r}", name=f"res{r}")
                    nc.sync.dma_start(
                        out=rt[:, :].rearrange("p (k i) -> p k i", k=CH),
                        in_=adjT[r].rearrange("(k p) i -> p k i", p=128),
                    )
                res_tiles.append(rt)

            # ---- layers ----
            for i in range(L):
                # c[i, r, :] = rel_r @ Wb_rel[i, r]   -> psum row 0, cols 3r..3r+3
                psc = psC.tile([1, RNB], f32, tag="c", name=f"psc{i}")
                for r in range(R):
                    nc.tensor.matmul(
                        psc[:, 3 * r : 3 * r + 3],
                        relT_sb[:, r : r + 1],
                        wbr_sb[i][:, 3 * r : 3 * r + 3],
                        start=True, stop=True,
                    )
                c_sb = workp.tile([1, RNB], f32, tag="c_sb", name=f"c_sb{i}", bufs=2)
                nc.scalar.copy(out=c_sb[:, :], in_=psc[:, :])

                # y'[kchunk] = x[kchunk] @ Wbx[i]  + 1 (x) c   -> bf16 (128, RNB) per chunk
                YS = YQ if DR else RNB
                y_all = ypool.tile([128, CH * YS], ADT, tag="y_all", name=f"y_all{i}")
                for k in range(CH):
                    psy = psY.tile([128, RNB], f32, tag="y", name=f"psy{i}_{k}")
                    nc.tensor.matmul(
                        psy[:, :], xT[:, k * 128 : (k + 1) * 128], wbx_sb[i][:, :],
                        start=True, stop=False,
                    )
                    nc.tensor.matmul(
                        psy[:, :], ones[:, :], c_sb[:, :],
                        start=False, stop=True,
                    )
                    nc.scalar.copy(out=y_all[:, k * YS : k * YS + RNB], in_=psy[:, :])

                # Z.T (NB, E) = sum_{r, k} y'_r[k].T @ adjT_r[k]
                # per i-chunk n: accumulate Z chunk, then basis/highway tail on
                # ACT/DVE overlaps the next chunk's PE matmuls
                assert DR
                h_sb = workp.tile([HID, E], f32, tag="h", name=f"h_sb{i}", bufs=1)
                y_view = y_all[:, :].rearrange("p (k q) -> p k q", q=YQ)
                res_views = [
                    res_tiles[r][:, :].rearrange("p (c t i) -> p c t i", c=C2, t=2)
                    for r in range(R)
                ]
                for n in range(3):
                    ns = slice(n * NW, (n + 1) * NW)
                    psz = psB.tile([NB, 512], f32, tag="zz", bufs=2,
                                   name=f"psz{i}_{n}")
                    cnt = 0
                    for r in range(R):
                        for c in range(C2):
                            nc.tensor.matmul(
                                psz[:, 0:NW],
                                y_view[:, 2 * c : 2 * c + 2, 3 * r : 3 * r + 3],
                                res_views[r][:, c, :, ns],
                                start=(cnt == 0),
                                stop=(cnt == R * C2 - 1),
                                perf_mode=mybir.MatmulPerfMode.DoubleRow,
                            )
                            cnt += 1
                    z_sb = workp.tile([NB, NW], f32, tag="z_sb", bufs=2,
                                      name=f"z_sb{i}_{n}")
                    nc.scalar.copy(out=z_sb[:, :], in_=psz[:, 0:NW])
                    psh = psB.tile([HID, 512], f32, tag="hh", bufs=1,
                                   name=f"psh{i}_{n}")
                    nc.tensor.matmul(
                        psh[:, 0:NW], ww_sb[i][:, :], z_sb[:, :],
                        start=True, stop=True,
                    )
                    nc.scalar.activation(
                        h_sb[:, ns], psh[:, 0:NW], AF.Relu, bias=bias_sb[i][:, :],
                    )
                    psg = psB.tile([HID, 512], f32, tag="gg", bufs=1,
                                   name=f"psg{i}_{n}")
                    nc.tensor.matmul(
                        psg[:, 0:NW], wh_sb[i][:, :], h_sb[:, ns],
                        start=True, stop=True,
                    )
                    nc.scalar.activation(
                        psg[:, 0:NW], psg[:, 0:NW], AF.Sigmoid, bias=bh_sb[i][:, :],
                    )
                    # x = x + g * (h - x)  (chunk n)
                    nc.vector.tensor_sub(h_sb[:, ns], h_sb[:, ns], xT[:, ns])
                    nc.vector.tensor_mul(h_sb[:, ns], h_sb[:, ns], psg[:, 0:NW])
                    nc.vector.tensor_add(xT[:, ns], xT[:, ns], h_sb[:, ns])

            # ---- masked mean over entities ----
            xm = workp.tile([HID, E], f32, tag="h", name="xm", bufs=1)
            nc.vector.tensor_mul(xm[:, :], xT[:, 0:E], mask_sb[:, :])
            gsum = workp.tile([HID, 1], f32, tag="gsum", name="gsum", bufs=1)
            nc.vector.reduce_sum(gsum[:, :], xm[:, :], axis=AX.X)
            den = workp.tile([HID, 1], f32, tag="den", name="den", bufs=1)
            nc.vector.reduce_sum(den[:, :], mask_sb[:, :], axis=AX.X)
            nc.vector.tensor_scalar_max(den[:, :], den[:, :], 1.0)
            nc.vector.reciprocal(den[:, :], den[:, :])
            nc.vector.tensor_mul(gsum[:, :], gsum[:, :], den[:, :])
            nc.sync.dma_start(out=graphD, in_=gsum[:, :])

    nc.compile()
    return nc


def get_nc():
    if "nc" not in _NC_CACHE:
        _NC_CACHE["nc"] = _build_nc()
    return _NC_CACHE["nc"]


def make_in_maps(adj, mask_ids, ent_emb, rel_emb, Wb, Ww, bias, Wh, bh):
    adj = np.asarray(adj, dtype=np.float32)
    if DR:
        pad = np.zeros((B, R, EP, E2), dtype=ADT_NP)
        pad[:, :, :E, :E] = adj.transpose(0, 1, 3, 2).astype(ADT_NP)
        # [b, r, c, p, t, i] = adj[b, r, i, j = c*256 + t*128 + p]
        adjT = np.ascontiguousarray(
            pad.reshape(B, R, C2, 2, 128, E2).transpose(0, 1, 2, 4, 3, 5)
        )
    else:
        adjT = np.zeros((B, R, EP, E), dtype=ADT_NP)
        adjT[:, :, :E, :] = adj.transpose(0, 1, 3, 2).astype(ADT_NP)
    entT = np.ascontiguousarray(np.asarray(ent_emb, np.float32).T)
    relTh = np.ascontiguousarray(np.asarray(rel_emb, np.float32).T)
    Wb5 = np.asarray(Wb, np.float32).reshape(L, R, 2, D, NB)
    wbx = np.ascontiguousarray(Wb5[:, :, 0].transpose(0, 2, 1, 3).reshape(L, D, RNB))
    wbr = np.ascontiguousarray(Wb5[:, :, 1].transpose(0, 2, 1, 3).reshape(L, D, RNB))
    maskf = np.asarray(mask_ids).astype(np.float32)
    common = dict(
        xT0=entT, relT=relTh, wbx=wbx, wbr=wbr,
        ww=np.ascontiguousarray(np.asarray(Ww, np.float32)),
        wh=np.ascontiguousarray(np.asarray(Wh, np.float32)),
        biasL=np.ascontiguousarray(np.asarray(bias, np.float32)),
        bhL=np.ascontiguousarray(np.asarray(bh, np.float32)),
    )
    in_maps = []
    for c in range(8):
        b = c // 2
        m = dict(common)
        m["adjT"] = np.ascontiguousarray(adjT[b])
        m["maskrep"] = np.ascontiguousarray(
            np.broadcast_to(maskf[b][None, :], (HID, E))
        )
        in_maps.append(m)
    return in_maps


def run(inputs, trace=False):
    nc = get_nc()
    in_maps = make_in_maps(**{k: np.asarray(v) for k, v in inputs.items()})
    res = bass_utils.run_bass_kernel_spmd(
        nc, in_maps, core_ids=list(range(8)), trace=trace
    )
    out = np.stack(
        [np.asarray(res.results[2 * b]["graph"]).reshape(HID) for b in range(B)]
    ).astype(np.float32)
    return out, res


def kernel(**inputs):
    out, _ = run(inputs, trace=False)
    return out



# revision 5
# speedup vs baseline: 1.1850x; 1.1850x over previous
"""Trainium2 Bass kernel for the KGEncoder RGCN (nn_KGEncoder_14027363188782).

Math (per batch element b, L=5 layers):
    x0 = ent_emb                                             (E, D)
    per layer i:
      y_r   = x @ Wb_x[i,r] + 1 * c[i,r]^T    (E, NB)  where c[i,r] = rel_r @ Wb_rel[i,r]
      Z     = sum_r adj_r @ y_r               (E, NB)
      h     = relu(Z @ Ww[i] + bias[i])
      g     = sigmoid(h @ Wh[i] + bh[i])
      x     = x + g * (h - x)
    out_b = sum_e x[e] * m[e] / max(sum_e m[e], 1)

Sharding: core c handles b = c // 2 (pair-replicated, no collectives).
adj shipped pre-transposed, fp8 (exact for 0/1), DoubleRow layout
[r, c, p, t, i] with j = c*256 + t*128 + p.

Schedule: granular (r,c) adj DMAs; layer 0 accumulates in DMA-arrival
order (granule-major over the 3 psum i-chunks) so it finishes right at
DMA end; layers 1-4 run a software-pipelined schedule that keeps PE
busy continuously (bigmm n-chunks back-to-back, highway tail + next
layer's y matmuls interleaved into the bigmm instruction stream).
Tail matmuls use f32r moving operands (1 cyc/row vs 4 for f32).
"""

import numpy as np
import ml_dtypes

import concourse.bacc as bacc
import concourse.bass as bass
import concourse.mybir as mybir
import concourse.tile as tile
from concourse import bass_utils
from concourse.bass import MemorySpace

B, R, E, D, HID, L, NB = 4, 10, 1500, 100, 100, 5, 3
EP = 1536           # j dim padded to 12*128
C2 = 6              # 256-row contraction chunks (DoubleRow)
E2 = 1504           # i dim padded to 16-aligned
YQ = 32             # y_all per-chunk col stride
CH = 12             # y chunks (128 j's each)
RNB = R * NB        # 30
NS = [(0, 512), (512, 1024), (1024, E2)]   # i-dim psum chunks
f32 = mybir.dt.float32
f32r = mybir.dt.float32r
fp8 = mybir.dt.float8e4
FP8_NP = ml_dtypes.float8_e4m3fn
AF = mybir.ActivationFunctionType
AX = mybir.AxisListType
ALU = mybir.AluOpType
DR = mybir.MatmulPerfMode.DoubleRow

ADJ_ORDER = [(r, c) for r in range(R) for c in range(C2)]

_NC_CACHE = {}


def _build_nc():
    nc = bacc.Bacc("TRN2", target_bir_lowering=False, debug=False)

    adjT = nc.dram_tensor("adjT", [R, C2, 128, 2, E2], fp8, kind="ExternalInput").ap()
    xT0 = nc.dram_tensor("xT0", [D, E], f32, kind="ExternalInput").ap()
    maskrep = nc.dram_tensor("maskrep", [HID, E2], f32, kind="ExternalInput").ap()
    relT = nc.dram_tensor("relT", [D, R], f32, kind="ExternalInput").ap()
    wbxD = nc.dram_tensor("wbx", [L, D, RNB], f32, kind="ExternalInput").ap()
    wbrD = nc.dram_tensor("wbr", [L, D, RNB], f32, kind="ExternalInput").ap()
    wwD = nc.dram_tensor("ww", [L, NB, HID], f32r, kind="ExternalInput").ap()
    whD = nc.dram_tensor("wh", [L, HID, HID], f32r, kind="ExternalInput").ap()
    biasD = nc.dram_tensor("biasL", [L, HID], f32, kind="ExternalInput").ap()
    bhD = nc.dram_tensor("bhL", [L, HID], f32, kind="ExternalInput").ap()
    graphD = nc.dram_tensor("graph", [HID, 1], f32, kind="ExternalOutput").ap()

    with tile.TileContext(nc) as tc:
        with (
            tc.tile_pool(name="singles", bufs=1) as singles,
            tc.tile_pool(name="ypool", bufs=2) as ypool,
            tc.tile_pool(name="workp", bufs=2) as workp,
            tc.tile_pool(name="pzp", bufs=1, space=MemorySpace.PSUM) as pzp,
            tc.tile_pool(name="pyp", bufs=2, space=MemorySpace.PSUM) as pyp,
            tc.tile_pool(name="php", bufs=1, space=MemorySpace.PSUM) as php,
            tc.tile_pool(name="pgp", bufs=1, space=MemorySpace.PSUM) as pgp,
        ):
            # ---------------- preamble: small loads ----------------
            relT_sb = singles.tile([D, R], f32, tag="relT", name="relT_sb")
            nc.sync.dma_start(out=relT_sb[:, :], in_=relT)

            wbr_sb, wbxa, ww_sb, wh_sb, bias_sb, bh_sb = [], [], [], [], [], []
            for i in range(L):
                wr = singles.tile([D, RNB], f32, tag=f"wbr{i}", name=f"wbr{i}")
                nc.sync.dma_start(out=wr[:, :], in_=wbrD[i])
                wbr_sb.append(wr)
            for i in range(L):
                # rows 0:100 = Wbx[i]; row 100 = c[i] (written below)
                wx = singles.tile([D + 1, RNB], f32, tag=f"wbxa{i}", name=f"wbxa{i}")
                nc.sync.dma_start(out=wx[0:D, :], in_=wbxD[i])
                wbxa.append(wx)
            # compute engines need 32-aligned partition bases: set the ones
            # row via a 96-based memset, then overwrite rows 96:100 below
            xTa = singles.tile([D + 1, EP], f32, tag="xTa", name="xTa")
            nc.vector.memset(xTa[96 : D + 1, :], 1.0)
            nc.vector.memset(xTa[0:D, E:EP], 0.0)
            nc.sync.dma_start(out=xTa[0:D, 0:E], in_=xT0)
            for i in range(L):
                wwt = singles.tile([NB, HID], f32r, tag=f"ww{i}", name=f"ww{i}")
                nc.sync.dma_start(out=wwt[:, :], in_=wwD[i])
                ww_sb.append(wwt)
                wht = singles.tile([HID, HID], f32r, tag=f"wh{i}", name=f"wh{i}")
                nc.sync.dma_start(out=wht[:, :], in_=whD[i])
                wh_sb.append(wht)
                bt = singles.tile([HID, 1], f32, tag=f"bias{i}", name=f"bias{i}")
                nc.sync.dma_start(out=bt[:, :], in_=biasD[i].unsqueeze(1))
                bias_sb.append(bt)
                bht = singles.tile([HID, 1], f32, tag=f"bh{i}", name=f"bh{i}")
                nc.sync.dma_start(out=bht[:, :], in_=bhD[i].unsqueeze(1))
                bh_sb.append(bht)

            # c[i, (r,q)] = rel_r @ Wbr[i, r] -> row 100 of wbxa[i]
            # (written via SBUF->SBUF DMA: ACT can't write partition base 100)
            for i in range(L):
                psc = pyp.tile([128, 360], f32, tag="py", name=f"psc{i}")
                for r in range(R):
                    nc.tensor.matmul(
                        psc[0:1, 3 * r : 3 * r + 3],
                        relT_sb[:, r : r + 1],
                        wbr_sb[i][:, 3 * r : 3 * r + 3],
                        start=True, stop=True,
                    )
                c_sb = workp.tile([1, RNB], f32, tag="c_sb", name=f"c_sb{i}")
                nc.scalar.copy(out=c_sb[:, :], in_=psc[0:1, 0:RNB])
                nc.sync.dma_start(out=wbxa[i][D : D + 1, :], in_=c_sb[:, :])

            # ---------------- adj granule DMAs ----------------
            res_tiles = []
            for r in range(R):
                rt = singles.tile([128, C2 * 2 * E2], fp8, tag=f"res{r}",
                                  name=f"res{r}")
                res_tiles.append(rt)
            for (r, c) in ADJ_ORDER:
                nc.sync.dma_start(
                    out=res_tiles[r][:, c * 2 * E2 : (c + 1) * 2 * E2],
                    in_=adjT[r, c].rearrange("p t i -> p (t i)"),
                )
            res_views = [
                res_tiles[r][:, :].rearrange("p (c t i) -> p c t i", c=C2, t=2)
                for r in range(R)
            ]
            mask_sb = singles.tile([HID, E2], f32, tag="mask", name="mask_sb")
            nc.sync.dma_start(out=mask_sb[:, :], in_=maskrep)

            h_sb = singles.tile([HID, E2], f32r, tag="h", name="h_sb")

            # ---------------- per-layer emission helpers ----------------
            ydict = {}    # i -> (psy tile, y_all tile, y_view)
            pzt = {}      # i -> psum tile holding the 3 Z chunk regions
            zcnt = {}     # (i, n) -> accumulation counter
            zsb = {}      # (i, n) -> z sbuf tile
            phd = {}      # (i, n) -> psh tile
            pgd = {}      # (i, n) -> psg tile

            def emit_y(i, ks):
                if i not in ydict:
                    psy = pyp.tile([128, 360], f32, tag="py", name=f"py{i}")
                    y_all = ypool.tile([128, CH * YQ], fp8, tag="y_all",
                                       name=f"y_all{i}")
                    yv = y_all[:, :].rearrange("p (k q) -> p k q", q=YQ)
                    ydict[i] = (psy, y_all, yv)
                psy, y_all, _ = ydict[i]
                for k in ks:
                    nc.tensor.matmul(
                        psy[:, 30 * k : 30 * k + 30],
                        xTa[:, 128 * k : 128 * (k + 1)],
                        wbxa[i][:, :],
                        start=True, stop=True,
                    )
                    nc.scalar.copy(
                        out=y_all[:, YQ * k : YQ * k + RNB],
                        in_=psy[:, 30 * k : 30 * k + 30],
                    )

            def bigmm(i, n, cs):
                if (i, n) not in pzt:
                    pzt[(i, n)] = pzp.tile([NB, 512], f32, tag=f"pz{n}",
                                           name=f"pz{i}_{n}")
                lo, hi = NS[n]
                nw = hi - lo
                out = pzt[(i, n)][:, 0:nw]
                yv = ydict[i][2]
                for c in cs:
                    for r in range(R):
                        cnt = zcnt.get((i, n), 0)
                        nc.tensor.matmul(
                            out,
                            yv[:, 2 * c : 2 * c + 2, 3 * r : 3 * r + 3],
                            res_views[r][:, c, :, lo:hi],
                            start=(cnt == 0),
                            stop=(cnt == R * C2 - 1),
                            perf_mode=DR,
                        )
                        zcnt[(i, n)] = cnt + 1

            def emit_zcopy(i, n):
                lo, hi = NS[n]
                nw = hi - lo
                zt = workp.tile([NB, 512], f32r, tag="z", name=f"z{i}_{n}")
                zsb[(i, n)] = zt
                nc.scalar.copy(out=zt[:, 0:nw], in_=pzt[(i, n)][:, 0:nw])

            def emit_psh(i, n):
                lo, hi = NS[n]
                nw = hi - lo
                ph = php.tile([HID, 512], f32, tag="ph", name=f"ph{i}_{n}")
                phd[(i, n)] = ph
                nc.tensor.matmul(
                    ph[:, 0:nw], ww_sb[i][:, :], zsb[(i, n)][:, 0:nw],
                    start=True, stop=True,
                )

            def emit_relu(i, n):
                lo, hi = NS[n]
                nc.scalar.activation(
                    h_sb[:, lo:hi], phd[(i, n)][:, 0 : hi - lo], AF.Relu,
                    bias=bias_sb[i][:, :],
                )

            def emit_psg(i, n):
                lo, hi = NS[n]
                nw = hi - lo
                pg = pgp.tile([HID, 512], f32, tag="pg", name=f"pg{i}_{n}")
                pgd[(i, n)] = pg
                nc.tensor.matmul(
                    pg[:, 0:nw], wh_sb[i][:, :], h_sb[:, lo:hi],
                    start=True, stop=True,
                )

            def emit_sig(i, n):
                lo, hi = NS[n]
                pg = pgd[(i, n)]
                nc.scalar.activation(
                    pg[:, 0 : hi - lo], pg[:, 0 : hi - lo], AF.Sigmoid,
                    bias=bh_sb[i][:, :],
                )

            def emit_xupd(i, n):
                lo, hi = NS[n]
                nw = hi - lo
                pg = pgd[(i, n)]
                nc.vector.tensor_sub(h_sb[:, lo:hi], h_sb[:, lo:hi], xTa[0:D, lo:hi])
                nc.vector.tensor_mul(h_sb[:, lo:hi], h_sb[:, lo:hi], pg[:, 0:nw])
                nc.vector.tensor_add(xTa[0:D, lo:hi], xTa[0:D, lo:hi], h_sb[:, lo:hi])

            # ---------------- layer 0 (DMA-arrival order) ----------------
            emit_y(0, range(CH))
            for n in range(3):
                pzt[(0, n)] = pzp.tile([NB, 512], f32, tag=f"pz{n}",
                                       name=f"pz0_{n}")
            for g, (r, c) in enumerate(ADJ_ORDER):
                yv = ydict[0][2]
                for n in range(3):
                    lo, hi = NS[n]
                    nc.tensor.matmul(
                        pzt[(0, n)][:, 0 : hi - lo],
                        yv[:, 2 * c : 2 * c + 2, 3 * r : 3 * r + 3],
                        res_views[r][:, c, :, lo:hi],
                        start=(g == 0),
                        stop=(g == len(ADJ_ORDER) - 1),
                        perf_mode=DR,
                    )
            for n in range(3):
                emit_zcopy(0, n)
            for n in range(3):
                emit_psh(0, n)
                emit_relu(0, n)
            for n in range(3):
                emit_psg(0, n)
                emit_sig(0, n)
            emit_xupd(0, 0)
            emit_y(1, [0, 1, 2, 3])
            emit_xupd(0, 1)
            emit_xupd(0, 2)

            # ---------------- layers 1..4 (pipelined) ----------------
            for i in range(1, L):
                bigmm(i, 0, [0, 1])
                emit_y(i, [4, 5, 6, 7])
                bigmm(i, 0, [2, 3])
                emit_y(i, [8, 9, 10, 11])
                bigmm(i, 0, [4, 5])
                emit_zcopy(i, 0)
                bigmm(i, 1, [0, 1, 2, 3, 4, 5])
                emit_psh(i, 0)
                emit_relu(i, 0)
                emit_zcopy(i, 1)
                bigmm(i, 2, [0, 1])
                emit_psg(i, 0)
                emit_sig(i, 0)
                emit_xupd(i, 0)
                bigmm(i, 2, [2, 3, 4, 5])
                emit_psh(i, 1)
                emit_relu(i, 1)
                emit_zcopy(i, 2)
                if i < L - 1:
                    emit_y(i + 1, [0, 1, 2, 3])
                emit_psg(i, 1)
                emit_sig(i, 1)
                emit_xupd(i, 1)
                emit_psh(i, 2)
                emit_relu(i, 2)
                emit_psg(i, 2)
                emit_sig(i, 2)
                emit_xupd(i, 2)

            # ---------------- epilogue: masked mean ----------------
            xm = workp.tile([HID, 512], f32, tag="xm", name="xm", bufs=1)
            gacc = []
            for n in range(3):
                lo, hi = NS[n]
                ga = workp.tile([HID, 1], f32, tag=f"ga{n}", name=f"ga{n}", bufs=1)
                gacc.append(ga)
                nc.vector.scalar_tensor_tensor(
                    out=xm[:, 0 : hi - lo],
                    in0=xTa[0:D, lo:hi],
                    scalar=1.0,
                    in1=mask_sb[:, lo:hi],
                    op0=ALU.mult,
                    op1=ALU.mult,
                    accum_out=ga[:, :],
                )
            den = workp.tile([HID, 1], f32, tag="den", name="den", bufs=1)
            nc.vector.reduce_sum(den[:, :], mask_sb[:, :], axis=AX.X)
            nc.vector.tensor_scalar_max(den[:, :], den[:, :], 1.0)
            nc.vector.reciprocal(den[:, :], den[:, :])
            nc.vector.tensor_add(gacc[0][:, :], gacc[0][:, :], gacc[1][:, :])
            nc.vector.tensor_add(gacc[0][:, :], gacc[0][:, :], gacc[2][:, :])
            nc.vector.tensor_mul(gacc[0][:, :], gacc[0][:, :], den[:, :])
            nc.sync.dma_start(out=graphD, in_=gacc[0][:, :])

    nc.compile()
    return nc


def get_nc():
    if "nc" not in _NC_CACHE:
        _NC_CACHE["nc"] = _build_nc()
    return _NC_CACHE["nc"]


def make_in_maps(adj, mask_ids, ent_emb, rel_emb, Wb, Ww, bias, Wh, bh):
    adj = np.asarray(adj, dtype=np.float32)
    pad = np.zeros((B, R, EP, E2), dtype=FP8_NP)
    pad[:, :, :E, :E] = adj.transpose(0, 1, 3, 2).astype(FP8_NP)
    # [b, r, c, p, t, i] = adj[b, r, i, j = c*256 + t*128 + p]
    adjT = np.ascontiguousarray(
        pad.reshape(B, R, C2, 2, 128, E2).transpose(0, 1, 2, 4, 3, 5)
    )
    entT = np.ascontiguousarray(np.asarray(ent_emb, np.float32).T)
    relTh = np.ascontiguousarray(np.asarray(rel_emb, np.float32).T)
    Wb5 = np.asarray(Wb, np.float32).reshape(L, R, 2, D, NB)
    wbx = np.ascontiguousarray(Wb5[:, :, 0].transpose(0, 2, 1, 3).reshape(L, D, RNB))
    wbr = np.ascontiguousarray(Wb5[:, :, 1].transpose(0, 2, 1, 3).reshape(L, D, RNB))
    maskf = np.asarray(mask_ids).astype(np.float32)
    common = dict(
        xT0=entT, relT=relTh, wbx=wbx, wbr=wbr,
        ww=np.ascontiguousarray(np.asarray(Ww, np.float32)),
        wh=np.ascontiguousarray(np.asarray(Wh, np.float32)),
        biasL=np.ascontiguousarray(np.asarray(bias, np.float32)),
        bhL=np.ascontiguousarray(np.asarray(bh, np.float32)),
    )
    in_maps = []
    for c in range(8):
        b = c // 2
        m = dict(common)
        m["adjT"] = np.ascontiguousarray(adjT[b])
        mrep = np.zeros((HID, E2), np.float32)
        mrep[:, :E] = np.broadcast_to(maskf[b][None, :], (HID, E))
        m["maskrep"] = mrep
        in_maps.append(m)
    return in_maps


def run(inputs, trace=False):
    nc = get_nc()
    in_maps = make_in_maps(**{k: np.asarray(v) for k, v in inputs.items()})
    res = bass_utils.run_bass_kernel_spmd(
        nc, in_maps, core_ids=list(range(8)), trace=trace
    )
    out = np.stack(
        [np.asarray(res.results[2 * b]["graph"]).reshape(HID) for b in range(B)]
    ).astype(np.float32)
    return out, res


def kernel(**inputs):
    out, _ = run(inputs, trace=False)
    return out


# revision 8
# speedup vs baseline: 1.3061x; 1.1022x over previous
"""Trainium2 Bass kernel for the KGEncoder RGCN (nn_KGEncoder_14027363188782).

Math (per batch element b, L=5 layers):
    x0 = ent_emb                                             (E, D)
    per layer i:
      y_r   = x @ Wb_x[i,r] + 1 * c[i,r]^T    (E, NB)  where c[i,r] = rel_r @ Wb_rel[i,r]
      Z     = sum_r adj_r @ y_r               (E, NB)
      h     = relu(Z @ Ww[i] + bias[i])
      g     = sigmoid(h @ Wh[i] + bh[i])
      x     = x + g * (h - x)
    out_b = sum_e x[e] * m[e] / max(sum_e m[e], 1)

Sharding: core c handles b = c // 2 (pair-replicated, no collectives).
adj shipped pre-transposed, fp8 (exact for 0/1), DoubleRow layout
[r, c, p, t, i] with j = c*256 + t*128 + p.

Schedule: granular (r,c) adj DMAs; layer 0 accumulates in DMA-arrival
order (granule-major over the 3 psum i-chunks) so it finishes right at
DMA end; layers 1-4 run a software-pipelined schedule that keeps PE
busy continuously: bigmm i-chunks back-to-back, the highway tail and
the next layer's y matmuls interleaved into the bigmm instruction
stream, and the next layer's first bigmm group filling the last
tail's ACT->PE ping-pong. Tail matmuls use f32r moving operands
(1 cyc/row vs 4 for f32). The per-layer y uses an augmented
contraction row (x row 100 = 1, W row 100 = c[i]) so the relation
bias needs no extra matmul.
"""

import numpy as np
import ml_dtypes

import concourse.bacc as bacc
import concourse.bass as bass
import concourse.mybir as mybir
import concourse.tile as tile
from concourse import bass_utils
from concourse.bass import MemorySpace

B, R, E, D, HID, L, NB = 4, 10, 1500, 100, 100, 5, 3
EP = 1536           # j dim padded to 12*128
C2 = 6              # 256-row contraction chunks (DoubleRow)
E2 = 1504           # i dim padded to 16-aligned
YQ = 32             # y_all per-chunk col stride
CH = 12             # y chunks (128 j's each)
RNB = R * NB        # 30
NS = [(0, 512), (512, 1024), (1024, E2)]   # i-dim psum chunks
f32 = mybir.dt.float32
f32r = mybir.dt.float32r
fp8 = mybir.dt.float8e4
FP8_NP = ml_dtypes.float8_e4m3fn
AF = mybir.ActivationFunctionType
AX = mybir.AxisListType
ALU = mybir.AluOpType
DR = mybir.MatmulPerfMode.DoubleRow

ADJ_ORDER = [(r, c) for r in range(R) for c in range(C2)]

_NC_CACHE = {}


def _build_nc():
    nc = bacc.Bacc("TRN2", target_bir_lowering=False, debug=False)

    adjT = nc.dram_tensor("adjT", [R, C2, 128, 2, E2], fp8, kind="ExternalInput").ap()
    xT0 = nc.dram_tensor("xT0", [D, E], f32, kind="ExternalInput").ap()
    maskrep = nc.dram_tensor("maskrep", [HID, E2], f32, kind="ExternalInput").ap()
    relT = nc.dram_tensor("relT", [D, R], f32, kind="ExternalInput").ap()
    wbxD = nc.dram_tensor("wbx", [L, D, RNB], f32, kind="ExternalInput").ap()
    wbrD = nc.dram_tensor("wbr", [L, D, RNB], f32, kind="ExternalInput").ap()
    wwD = nc.dram_tensor("ww", [L, NB, HID], f32r, kind="ExternalInput").ap()
    whD = nc.dram_tensor("wh", [L, HID, HID], f32r, kind="ExternalInput").ap()
    biasD = nc.dram_tensor("biasL", [L, HID], f32, kind="ExternalInput").ap()
    bhD = nc.dram_tensor("bhL", [L, HID], f32, kind="ExternalInput").ap()
    graphD = nc.dram_tensor("graph", [HID, 1], f32, kind="ExternalOutput").ap()

    with tile.TileContext(nc) as tc:
        with (
            tc.tile_pool(name="singles", bufs=1) as singles,
            tc.tile_pool(name="ypool", bufs=2) as ypool,
            tc.tile_pool(name="workp", bufs=2) as workp,
            tc.tile_pool(name="pzp", bufs=1, space=MemorySpace.PSUM) as pzp,
            tc.tile_pool(name="pyp", bufs=2, space=MemorySpace.PSUM) as pyp,
            tc.tile_pool(name="php", bufs=1, space=MemorySpace.PSUM) as php,
            tc.tile_pool(name="pgp", bufs=1, space=MemorySpace.PSUM) as pgp,
        ):
            # ------- preamble: batched small loads (one DMA each) -------
            relT_sb = singles.tile([D, R], f32, tag="relT", name="relT_sb")
            nc.sync.dma_start(out=relT_sb[:, :], in_=relT)
            wbr_all = singles.tile([D, L * RNB], f32, tag="wbr", name="wbr_all")
            nc.sync.dma_start(
                out=wbr_all[:, :].rearrange("d (l q) -> d l q", l=L),
                in_=wbrD.rearrange("l d q -> d l q")
            )
            # rows 0:100 = Wbx[i] at cols 30i; row 100 = c[i] (written below)
            wbxa = singles.tile([D + 1, L * RNB], f32, tag="wbxa", name="wbxa")
            nc.sync.dma_start(
                out=wbxa[0:D, :].rearrange("d (l q) -> d l q", l=L),
                in_=wbxD.rearrange("l d q -> d l q")
            )
            xTa = singles.tile([D + 1, EP], f32, tag="xTa", name="xTa")
            nc.vector.memset(xTa[96 : D + 1, :], 1.0)
            nc.vector.memset(xTa[0:D, E:EP], 0.0)
            nc.sync.dma_start(out=xTa[0:D, 0:E], in_=xT0)

            # c[i, (r,q)] = rel_r @ Wbr[i, r] -> row 100 of wbxa (via ACT-queue
            # SBUF->SBUF DMA: ACT copy can't write partition base 100, and
            # the SP queue must stay free for the adj granule stream)
            psc = pyp.tile([128, 360], f32, tag="py", name="psc")
            for i in range(L):
                for r in range(R):
                    q = RNB * i + 3 * r
                    nc.tensor.matmul(
                        psc[0:1, q : q + 3],
                        relT_sb[:, r : r + 1],
                        wbr_all[:, q : q + 3],
                        start=True, stop=True,
                    )
            c_sb = workp.tile([1, L * RNB], f32, tag="c_sb", name="c_sb")
            nc.scalar.copy(out=c_sb[:, :], in_=psc[0:1, 0 : L * RNB])
            nc.scalar.dma_start(out=wbxa[D : D + 1, :], in_=c_sb[:, :])

            # ------- adj granule DMAs (the long pole; SP queue) -------
            res_tiles = []
            for r in range(R):
                rt = singles.tile([128, C2 * 2 * E2], fp8, tag=f"res{r}",
                                  name=f"res{r}")
                res_tiles.append(rt)
            for (r, c) in ADJ_ORDER:
                nc.sync.dma_start(
                    out=res_tiles[r][:, c * 2 * E2 : (c + 1) * 2 * E2],
                    in_=adjT[r, c].rearrange("p t i -> p (t i)"),
                )
            res_views = [
                res_tiles[r][:, :].rearrange("p (c t i) -> p c t i", c=C2, t=2)
                for r in range(R)
            ]

            # tail weights: issued after the granules (needed only once
            # layer-0's Z is complete, i.e. right at DMA end)
            ww_all = singles.tile([NB, L * HID], f32r, tag="ww", name="ww_all")
            nc.sync.dma_start(out=ww_all[:, :].rearrange("n (l h) -> n l h", l=L),
                              in_=wwD.rearrange("l n h -> n l h"))
            wh_all = singles.tile([HID, L * HID], f32r, tag="wh", name="wh_all")
            nc.sync.dma_start(out=wh_all[:, :].rearrange("p (l h) -> p l h", l=L),
                              in_=whD.rearrange("l p h -> p l h"))
            bias_all = singles.tile([HID, L], f32, tag="bias", name="bias_all")
            nc.sync.dma_start(out=bias_all[:, :], in_=biasD.rearrange("l h -> h l"))
            bh_all = singles.tile([HID, L], f32, tag="bh", name="bh_all")
            nc.sync.dma_start(out=bh_all[:, :], in_=bhD.rearrange("l h -> h l"))
            mask_sb = singles.tile([HID, E2], f32, tag="mask", name="mask_sb")
            nc.sync.dma_start(out=mask_sb[:, :], in_=maskrep)

            h_sb = singles.tile([HID, E2], f32r, tag="h", name="h_sb")

            # ------- per-layer emission helpers -------
            ydict = {}    # i -> (psy tile, y_all tile, y_view)
            pzt = {}      # (i, n) -> Z psum tile
            zcnt = {}     # (i, n) -> accumulation counter
            zsb = {}      # (i, n) -> z sbuf tile
            phd = {}      # (i, n) -> psh tile
            pgd = {}      # (i, n) -> psg tile

            def emit_y(i, ks):
                if i not in ydict:
                    psy = pyp.tile([128, 360], f32, tag="py", name=f"py{i}")
                    y_all = ypool.tile([128, CH * YQ], fp8, tag="y_all",
                                       name=f"y_all{i}")
                    yv = y_all[:, :].rearrange("p (k q) -> p k q", q=YQ)
                    ydict[i] = (psy, y_all, yv)
                psy, y_all, _ = ydict[i]
                for k in ks:
                    nc.tensor.matmul(
                        psy[:, 30 * k : 30 * k + 30],
                        xTa[:, 128 * k : 128 * (k + 1)],
                        wbxa[:, RNB * i : RNB * (i + 1)],
                        start=True, stop=True,
                    )
                    nc.scalar.copy(
                        out=y_all[:, YQ * k : YQ * k + RNB],
                        in_=psy[:, 30 * k : 30 * k + 30],
                    )

            def bigmm(i, n, cs):
                if (i, n) not in pzt:
                    pzt[(i, n)] = pzp.tile([NB, 512], f32, tag=f"pz{n}",
                                           name=f"pz{i}_{n}")
                lo, hi = NS[n]
                nw = hi - lo
                out = pzt[(i, n)][:, 0:nw]
                yv = ydict[i][2]
                for c in cs:
                    for r in range(R):
                        cnt = zcnt.get((i, n), 0)
                        nc.tensor.matmul(
                            out,
                            yv[:, 2 * c : 2 * c + 2, 3 * r : 3 * r + 3],
                            res_views[r][:, c, :, lo:hi],
                            start=(cnt == 0),
                            stop=(cnt == R * C2 - 1),
                            perf_mode=DR,
                        )
                        zcnt[(i, n)] = cnt + 1

            def emit_zcopy(i, n):
                lo, hi = NS[n]
                nw = hi - lo
                zt = workp.tile([NB, 512], f32r, tag="z", name=f"z{i}_{n}")
                zsb[(i, n)] = zt
                nc.scalar.copy(out=zt[:, 0:nw], in_=pzt[(i, n)][:, 0:nw])

            def emit_psh(i, n):
                lo, hi = NS[n]
                nw = hi - lo
                ph = php.tile([HID, 512], f32, tag="ph", name=f"ph{i}_{n}")
                phd[(i, n)] = ph
                nc.tensor.matmul(
                    ph[:, 0:nw],
                    ww_all[:, HID * i : HID * (i + 1)],
                    zsb[(i, n)][:, 0:nw],
                    start=True, stop=True,
                )

            def emit_relu(i, n):
                lo, hi = NS[n]
                nc.scalar.activation(
                    h_sb[:, lo:hi], phd[(i, n)][:, 0 : hi - lo], AF.Relu,
                    bias=bias_all[:, i : i + 1],
                )

            def emit_psg(i, n):
                lo, hi = NS[n]
                nw = hi - lo
                pg = pgp.tile([HID, 512], f32, tag="pg", name=f"pg{i}_{n}")
                pgd[(i, n)] = pg
                nc.tensor.matmul(
                    pg[:, 0:nw],
                    wh_all[:, HID * i : HID * (i + 1)],
                    h_sb[:, lo:hi],
                    start=True, stop=True,
                )

            def emit_sig(i, n):
                lo, hi = NS[n]
                pg = pgd[(i, n)]
                nc.scalar.activation(
                    pg[:, 0 : hi - lo], pg[:, 0 : hi - lo], AF.Sigmoid,
                    bias=bh_all[:, i : i + 1],
                )

            def emit_xupd(i, n):
                lo, hi = NS[n]
                nw = hi - lo
                pg = pgd[(i, n)]
                nc.vector.tensor_sub(h_sb[:, lo:hi], h_sb[:, lo:hi], xTa[0:D, lo:hi])
                nc.vector.tensor_mul(h_sb[:, lo:hi], h_sb[:, lo:hi], pg[:, 0:nw])
                nc.vector.tensor_add(xTa[0:D, lo:hi], xTa[0:D, lo:hi], h_sb[:, lo:hi])

            # ------- layer 0 (DMA-arrival order, granule-major) -------
            emit_y(0, range(CH))
            for n in range(3):
                pzt[(0, n)] = pzp.tile([NB, 512], f32, tag=f"pz{n}", name=f"pz0_{n}")
            yv0 = ydict[0][2]
            for g, (r, c) in enumerate(ADJ_ORDER):
                for n in range(3):
                    lo, hi = NS[n]
                    nc.tensor.matmul(
                        pzt[(0, n)][:, 0 : hi - lo],
                        yv0[:, 2 * c : 2 * c + 2, 3 * r : 3 * r + 3],
                        res_views[r][:, c, :, lo:hi],
                        start=(g == 0),
                        stop=(g == len(ADJ_ORDER) - 1),
                        perf_mode=DR,
                    )
            for n in range(3):
                emit_zcopy(0, n)
            for n in range(3):
                emit_psh(0, n)
                emit_relu(0, n)
            for n in range(3):
                emit_psg(0, n)
                emit_sig(0, n)
            emit_xupd(0, 0)
            emit_y(1, [0, 1, 2, 3])
            emit_xupd(0, 1)
            emit_y(1, [4, 5, 6, 7])
            bigmm(1, 0, [0, 1])
            emit_xupd(0, 2)

            # ------- layers 1..4 (software-pipelined) -------
            # entry state per layer i: bigmm(i,0,[0,1]) and y(i) k0-7 already
            # emitted by the predecessor; y(i) k8-11 still pending.
            for i in range(1, L):
                bigmm(i, 0, [2, 3])
                emit_y(i, [8, 9, 10, 11])
                bigmm(i, 0, [4, 5])
                emit_zcopy(i, 0)
                bigmm(i, 1, [0, 1, 2, 3, 4, 5])
                emit_psh(i, 0)
                emit_relu(i, 0)
                emit_zcopy(i, 1)
                bigmm(i, 2, [0, 1])
                emit_psg(i, 0)
                emit_sig(i, 0)
                emit_xupd(i, 0)
                bigmm(i, 2, [2, 3, 4, 5])
                emit_psh(i, 1)
                emit_relu(i, 1)
                emit_zcopy(i, 2)
                if i < L - 1:
                    emit_y(i + 1, [0, 1, 2, 3])
                emit_psg(i, 1)
                emit_sig(i, 1)
                emit_xupd(i, 1)
                emit_psh(i, 2)
                emit_relu(i, 2)
                if i < L - 1:
                    # fill the relu->psg ping-pong with next layer's work
                    bigmm(i + 1, 0, [0, 1])
                    emit_y(i + 1, [4, 5, 6, 7])
                emit_psg(i, 2)
                emit_sig(i, 2)
                emit_xupd(i, 2)
                if i == 1:
                    # denominator for the masked mean, on the idle DVE
                    den = workp.tile([HID, 1], f32, tag="den", name="den", bufs=1)
                    nc.vector.reduce_sum(den[:, :], mask_sb[:, :], axis=AX.X)
                    nc.vector.tensor_scalar_max(den[:, :], den[:, :], 1.0)
                    nc.vector.reciprocal(den[:, :], den[:, :])

            # ------- epilogue: masked mean -------
            xm = workp.tile([HID, 512], f32, tag="xm", name="xm", bufs=1)
            gacc = []
            for n in range(3):
                lo, hi = NS[n]
                ga = workp.tile([HID, 1], f32, tag=f"ga{n}", name=f"ga{n}", bufs=1)
                gacc.append(ga)
                nc.vector.scalar_tensor_tensor(
                    out=xm[:, 0 : hi - lo],
                    in0=xTa[0:D, lo:hi],
                    scalar=1.0,
                    in1=mask_sb[:, lo:hi],
                    op0=ALU.mult,
                    op1=ALU.mult,
                    accum_out=ga[:, :],
                )
            nc.vector.tensor_add(gacc[0][:, :], gacc[0][:, :], gacc[1][:, :])
            nc.vector.tensor_add(gacc[0][:, :], gacc[0][:, :], gacc[2][:, :])
            nc.vector.tensor_mul(gacc[0][:, :], gacc[0][:, :], den[:, :])
            nc.sync.dma_start(out=graphD, in_=gacc[0][:, :])

    nc.compile()
    return nc


def get_nc():
    if "nc" not in _NC_CACHE:
        _NC_CACHE["nc"] = _build_nc()
    return _NC_CACHE["nc"]


def make_in_maps(adj, mask_ids, ent_emb, rel_emb, Wb, Ww, bias, Wh, bh):
    adj = np.asarray(adj, dtype=np.float32)
    pad = np.zeros((B, R, EP, E2), dtype=FP8_NP)
    pad[:, :, :E, :E] = adj.transpose(0, 1, 3, 2).astype(FP8_NP)
    # [b, r, c, p, t, i] = adj[b, r, i, j = c*256 + t*128 + p]
    adjT = np.ascontiguousarray(
        pad.reshape(B, R, C2, 2, 128, E2).transpose(0, 1, 2, 4, 3, 5)
    )
    entT = np.ascontiguousarray(np.asarray(ent_emb, np.float32).T)
    relTh = np.ascontiguousarray(np.asarray(rel_emb, np.float32).T)
    Wb5 = np.asarray(Wb, np.float32).reshape(L, R, 2, D, NB)
    wbx = np.ascontiguousarray(Wb5[:, :, 0].transpose(0, 2, 1, 3).reshape(L, D, RNB))
    wbr = np.ascontiguousarray(Wb5[:, :, 1].transpose(0, 2, 1, 3).reshape(L, D, RNB))
    maskf = np.asarray(mask_ids).astype(np.float32)
    common = dict(
        xT0=entT, relT=relTh, wbx=wbx, wbr=wbr,
        ww=np.ascontiguousarray(np.asarray(Ww, np.float32)),
        wh=np.ascontiguousarray(np.asarray(Wh, np.float32)),
        biasL=np.ascontiguousarray(np.asarray(bias, np.float32)),
        bhL=np.ascontiguousarray(np.asarray(bh, np.float32)),
    )
    in_maps = []
    for c in range(8):
        b = c // 2
        m = dict(common)
        m["adjT"] = np.ascontiguousarray(adjT[b])
        mrep = np.zeros((HID, E2), np.float32)
        mrep[:, :E] = np.broadcast_to(maskf[b][None, :], (HID, E))
        m["maskrep"] = mrep
        in_maps.append(m)
    return in_maps


def run(inputs, trace=False):
    nc = get_nc()
    in_maps = make_in_maps(**{k: np.asarray(v) for k, v in inputs.items()})
    res = bass_utils.run_bass_kernel_spmd(
        nc, in_maps, core_ids=list(range(8)), trace=trace
    )
    out = np.stack(
        [np.asarray(res.results[2 * b]["graph"]).reshape(HID) for b in range(B)]
    ).astype(np.float32)
    return out, res


def kernel(**inputs):
    out, _ = run(inputs, trace=False)
    return out


# revision 9
# speedup vs baseline: 1.3675x; 1.0470x over previous
"""Trainium2 Bass kernel for the KGEncoder RGCN (nn_KGEncoder_14027363188782).

Math (per batch element b, L=5 layers):
    x0 = ent_emb                                             (E, D)
    per layer i:
      y_r   = x @ Wb_x[i,r] + 1 * c[i,r]^T    (E, NB)  where c[i,r] = rel_r @ Wb_rel[i,r]
      Z     = sum_r adj_r @ y_r               (E, NB)
      h     = relu(Z @ Ww[i] + bias[i])
      g     = sigmoid(h @ Wh[i] + bh[i])
      x     = x + g * (h - x)
    out_b = sum_e x[e] * m[e] / max(sum_e m[e], 1)

Sharding: core c handles b = c // 2 (pair-replicated, no collectives).
adj shipped pre-transposed, fp8 (exact for 0/1), DoubleRow layout
[r, c, p, t, i] with j = c*256 + t*128 + p.

Schedule: granular (r,c) adj DMAs; layer 0 accumulates in DMA-arrival
order (granule-major over the 3 psum i-chunks) so it finishes right at
DMA end; layers 1-4 run a software-pipelined schedule that keeps PE
busy continuously: bigmm i-chunks back-to-back, the highway tail and
the next layer's y matmuls interleaved into the bigmm instruction
stream, and the next layer's first bigmm group filling the last
tail's ACT->PE ping-pong. Tail matmuls use f32r moving operands
(1 cyc/row vs 4 for f32). The per-layer y uses an augmented
contraction row (x row 100 = 1, W row 100 = c[i]) so the relation
bias needs no extra matmul.
"""

import numpy as np
import ml_dtypes

import concourse.bacc as bacc
import concourse.bass as bass
import concourse.mybir as mybir
import concourse.tile as tile
from concourse import bass_utils
from concourse.bass import MemorySpace

B, R, E, D, HID, L, NB = 4, 10, 1500, 100, 100, 5, 3
EP = 1536           # j dim padded to 12*128
C2 = 6              # 256-row contraction chunks (DoubleRow)
E2 = 1504           # i dim padded to 16-aligned
YQ = 32             # y_all per-chunk col stride
CH = 12             # y chunks (128 j's each)
RNB = R * NB        # 30
NS = [(0, 512), (512, 1024), (1024, E2)]   # i-dim psum chunks
f32 = mybir.dt.float32
f32r = mybir.dt.float32r
fp8 = mybir.dt.float8e4
FP8_NP = ml_dtypes.float8_e4m3fn
AF = mybir.ActivationFunctionType
AX = mybir.AxisListType
ALU = mybir.AluOpType
DR = mybir.MatmulPerfMode.DoubleRow

ADJ_ORDER = [(r, c) for r in range(R) for c in range(C2)]

_NC_CACHE = {}


def _build_nc():
    nc = bacc.Bacc("TRN2", target_bir_lowering=False, debug=False)

    adjT = nc.dram_tensor("adjT", [R, C2, 128, 2, E2], fp8, kind="ExternalInput").ap()
    xT0 = nc.dram_tensor("xT0", [D, E], f32, kind="ExternalInput").ap()
    maskrep = nc.dram_tensor("maskrep", [HID, E2], f32, kind="ExternalInput").ap()
    relT = nc.dram_tensor("relT", [D, R], f32, kind="ExternalInput").ap()
    wbxD = nc.dram_tensor("wbx", [L, D, RNB], f32, kind="ExternalInput").ap()
    wbrD = nc.dram_tensor("wbr", [L, D, RNB], f32, kind="ExternalInput").ap()
    wwD = nc.dram_tensor("ww", [L, NB, HID], f32r, kind="ExternalInput").ap()
    whD = nc.dram_tensor("wh", [L, HID, HID], f32r, kind="ExternalInput").ap()
    biasD = nc.dram_tensor("biasL", [L, HID], f32, kind="ExternalInput").ap()
    bhD = nc.dram_tensor("bhL", [L, HID], f32, kind="ExternalInput").ap()
    graphD = nc.dram_tensor("graph", [HID, 1], f32, kind="ExternalOutput").ap()

    with tile.TileContext(nc) as tc:
        with (
            tc.tile_pool(name="singles", bufs=1) as singles,
            tc.tile_pool(name="ypool", bufs=2) as ypool,
            tc.tile_pool(name="workp", bufs=2) as workp,
            tc.tile_pool(name="pzp", bufs=1, space=MemorySpace.PSUM) as pzp,
            tc.tile_pool(name="pyp", bufs=2, space=MemorySpace.PSUM) as pyp,
            tc.tile_pool(name="php", bufs=1, space=MemorySpace.PSUM) as php,
            tc.tile_pool(name="pgp", bufs=1, space=MemorySpace.PSUM) as pgp,
        ):
            # ------- preamble: batched small loads (one DMA each) -------
            relT_sb = singles.tile([D, R], f32, tag="relT", name="relT_sb")
            nc.sync.dma_start(out=relT_sb[:, :], in_=relT)
            wbr_all = singles.tile([D, L * RNB], f32, tag="wbr", name="wbr_all")
            nc.sync.dma_start(
                out=wbr_all[:, :].rearrange("d (l q) -> d l q", l=L),
                in_=wbrD.rearrange("l d q -> d l q")
            )
            # rows 0:100 = Wbx[i] at cols 30i; row 100 = c[i] (written below)
            wbxa = singles.tile([D + 1, L * RNB], f32, tag="wbxa", name="wbxa")
            nc.sync.dma_start(
                out=wbxa[0:D, :].rearrange("d (l q) -> d l q", l=L),
                in_=wbxD.rearrange("l d q -> d l q")
            )
            xTa = singles.tile([D + 1, EP], f32, tag="xTa", name="xTa")
            nc.vector.memset(xTa[96 : D + 1, :], 1.0)
            nc.vector.memset(xTa[0:D, E:EP], 0.0)
            nc.sync.dma_start(out=xTa[0:D, 0:E], in_=xT0)

            # c[i, (r,q)] = rel_r @ Wbr[i, r] -> row 100 of wbxa (via ACT-queue
            # SBUF->SBUF DMA: ACT copy can't write partition base 100, and
            # the SP queue must stay free for the adj granule stream)
            psc = pyp.tile([128, 360], f32, tag="py", name="psc")
            for i in range(L):
                for r in range(R):
                    q = RNB * i + 3 * r
                    nc.tensor.matmul(
                        psc[0:1, q : q + 3],
                        relT_sb[:, r : r + 1],
                        wbr_all[:, q : q + 3],
                        start=True, stop=True,
                    )
            c_sb = workp.tile([1, L * RNB], f32, tag="c_sb", name="c_sb")
            nc.scalar.copy(out=c_sb[:, :], in_=psc[0:1, 0 : L * RNB])
            nc.scalar.dma_start(out=wbxa[D : D + 1, :], in_=c_sb[:, :])

            # ------- adj granule DMAs (the long pole; SP queue) -------
            res_tiles = []
            for r in range(R):
                rt = singles.tile([128, C2 * 2 * E2], fp8, tag=f"res{r}",
                                  name=f"res{r}")
                res_tiles.append(rt)
            for (r, c) in ADJ_ORDER:
                nc.sync.dma_start(
                    out=res_tiles[r][:, c * 2 * E2 : (c + 1) * 2 * E2],
                    in_=adjT[r, c].rearrange("p t i -> p (t i)"),
                )
            res_views = [
                res_tiles[r][:, :].rearrange("p (c t i) -> p c t i", c=C2, t=2)
                for r in range(R)
            ]

            # tail weights: issued after the granules (needed only once
            # layer-0's Z is complete, i.e. right at DMA end)
            ww_all = singles.tile([NB, L * HID], f32r, tag="ww", name="ww_all")
            nc.sync.dma_start(out=ww_all[:, :].rearrange("n (l h) -> n l h", l=L),
                              in_=wwD.rearrange("l n h -> n l h"))
            wh_all = singles.tile([HID, L * HID], f32r, tag="wh", name="wh_all")
            nc.sync.dma_start(out=wh_all[:, :].rearrange("p (l h) -> p l h", l=L),
                              in_=whD.rearrange("l p h -> p l h"))
            bias_all = singles.tile([HID, L], f32, tag="bias", name="bias_all")
            nc.sync.dma_start(out=bias_all[:, :], in_=biasD.rearrange("l h -> h l"))
            bh_all = singles.tile([HID, L], f32, tag="bh", name="bh_all")
            nc.sync.dma_start(out=bh_all[:, :], in_=bhD.rearrange("l h -> h l"))
            mask_sb = singles.tile([HID, E2], f32, tag="mask", name="mask_sb")
            nc.sync.dma_start(out=mask_sb[:, :], in_=maskrep)

            h_sb = singles.tile([HID, E2], f32r, tag="h", name="h_sb")

            # ------- per-layer emission helpers -------
            ydict = {}    # i -> (psy tile, y_all tile, y_view)
            pzt = {}      # (i, n) -> Z psum tile
            zcnt = {}     # (i, n) -> accumulation counter
            zsb = {}      # (i, n) -> z sbuf tile
            phd = {}      # (i, n) -> psh tile
            pgd = {}      # (i, n) -> psg tile

            def emit_y(i, ks):
                if i not in ydict:
                    psy = pyp.tile([128, 360], f32, tag="py", name=f"py{i}")
                    y_all = ypool.tile([128, CH * YQ], fp8, tag="y_all",
                                       name=f"y_all{i}")
                    yv = y_all[:, :].rearrange("p (k q) -> p k q", q=YQ)
                    ydict[i] = (psy, y_all, yv)
                psy, y_all, _ = ydict[i]
                for k in ks:
                    nc.tensor.matmul(
                        psy[:, 30 * k : 30 * k + 30],
                        xTa[:, 128 * k : 128 * (k + 1)],
                        wbxa[:, RNB * i : RNB * (i + 1)],
                        start=True, stop=True,
                    )
                k0, nk = ks[0], len(ks)
                nc.scalar.copy(
                    out=y_all[:, YQ * k0 : YQ * (k0 + nk)].rearrange(
                        "p (k q) -> p k q", q=YQ)[:, :, 0:RNB],
                    in_=psy[:, 30 * k0 : 30 * (k0 + nk)].rearrange(
                        "p (k q) -> p k q", q=RNB),
                )

            def bigmm(i, n, cs):
                if (i, n) not in pzt:
                    pzt[(i, n)] = pzp.tile([NB, 512], f32, tag=f"pz{n}",
                                           name=f"pz{i}_{n}")
                lo, hi = NS[n]
                nw = hi - lo
                out = pzt[(i, n)][:, 0:nw]
                yv = ydict[i][2]
                for c in cs:
                    for r in range(R):
                        cnt = zcnt.get((i, n), 0)
                        nc.tensor.matmul(
                            out,
                            yv[:, 2 * c : 2 * c + 2, 3 * r : 3 * r + 3],
                            res_views[r][:, c, :, lo:hi],
                            start=(cnt == 0),
                            stop=(cnt == R * C2 - 1),
                            perf_mode=DR,
                        )
                        zcnt[(i, n)] = cnt + 1

            def emit_zcopy(i, n):
                lo, hi = NS[n]
                nw = hi - lo
                zt = workp.tile([NB, 512], f32r, tag="z", name=f"z{i}_{n}")
                zsb[(i, n)] = zt
                nc.scalar.copy(out=zt[:, 0:nw], in_=pzt[(i, n)][:, 0:nw])

            def emit_psh(i, n):
                lo, hi = NS[n]
                nw = hi - lo
                ph = php.tile([HID, 512], f32, tag="ph", name=f"ph{i}_{n}")
                phd[(i, n)] = ph
                nc.tensor.matmul(
                    ph[:, 0:nw],
                    ww_all[:, HID * i : HID * (i + 1)],
                    zsb[(i, n)][:, 0:nw],
                    start=True, stop=True,
                )

            def emit_relu(i, n):
                lo, hi = NS[n]
                nc.scalar.activation(
                    h_sb[:, lo:hi], phd[(i, n)][:, 0 : hi - lo], AF.Relu,
                    bias=bias_all[:, i : i + 1],
                )

            def emit_psg(i, n):
                lo, hi = NS[n]
                nw = hi - lo
                pg = pgp.tile([HID, 512], f32, tag="pg", name=f"pg{i}_{n}")
                pgd[(i, n)] = pg
                nc.tensor.matmul(
                    pg[:, 0:nw],
                    wh_all[:, HID * i : HID * (i + 1)],
                    h_sb[:, lo:hi],
                    start=True, stop=True,
                )

            def emit_sig(i, n):
                lo, hi = NS[n]
                pg = pgd[(i, n)]
                nc.scalar.activation(
                    pg[:, 0 : hi - lo], pg[:, 0 : hi - lo], AF.Sigmoid,
                    bias=bh_all[:, i : i + 1],
                )

            def emit_xupd(i, n):
                lo, hi = NS[n]
                pg = pgd[(i, n)]
                for a in range(lo, hi, 256):
                    b = min(a + 256, hi)
                    nc.vector.tensor_sub(
                        h_sb[:, a:b], h_sb[:, a:b], xTa[0:D, a:b])
                    nc.vector.tensor_mul(
                        h_sb[:, a:b], h_sb[:, a:b], pg[:, a - lo : b - lo])
                    nc.vector.tensor_add(
                        xTa[0:D, a:b], xTa[0:D, a:b], h_sb[:, a:b])

            # ------- layer 0 (DMA-arrival order, granule-major) -------
            emit_y(0, [0, 1, 2, 3])
            emit_y(0, [4, 5, 6, 7])
            emit_y(0, [8, 9, 10, 11])
            for n in range(3):
                pzt[(0, n)] = pzp.tile([NB, 512], f32, tag=f"pz{n}", name=f"pz0_{n}")
            yv0 = ydict[0][2]
            for g, (r, c) in enumerate(ADJ_ORDER):
                for n in range(3):
                    lo, hi = NS[n]
                    nc.tensor.matmul(
                        pzt[(0, n)][:, 0 : hi - lo],
                        yv0[:, 2 * c : 2 * c + 2, 3 * r : 3 * r + 3],
                        res_views[r][:, c, :, lo:hi],
                        start=(g == 0),
                        stop=(g == len(ADJ_ORDER) - 1),
                        perf_mode=DR,
                    )
            for n in range(3):
                emit_zcopy(0, n)
            for n in range(3):
                emit_psh(0, n)
                emit_relu(0, n)
            for n in range(3):
                emit_psg(0, n)
                emit_sig(0, n)
            emit_xupd(0, 0)
            emit_y(1, [0, 1, 2, 3])
            emit_xupd(0, 1)
            emit_y(1, [4, 5, 6, 7])
            bigmm(1, 0, [0, 1])
            emit_xupd(0, 2)

            # ------- layers 1..4 (software-pipelined) -------
            # entry state per layer i: bigmm(i,0,[0,1]) and y(i) k0-7 already
            # emitted by the predecessor; y(i) k8-11 still pending.
            for i in range(1, L):
                bigmm(i, 0, [2, 3])
                emit_y(i, [8, 9, 10, 11])
                bigmm(i, 0, [4, 5])
                emit_zcopy(i, 0)
                bigmm(i, 1, [0, 1, 2, 3, 4, 5])
                emit_psh(i, 0)
                emit_relu(i, 0)
                emit_zcopy(i, 1)
                bigmm(i, 2, [0, 1])
                emit_psg(i, 0)
                emit_sig(i, 0)
                emit_xupd(i, 0)
                bigmm(i, 2, [2, 3, 4, 5])
                emit_psh(i, 1)
                emit_relu(i, 1)
                emit_zcopy(i, 2)
                if i < L - 1:
                    emit_y(i + 1, [0, 1, 2, 3])
                emit_psg(i, 1)
                emit_psh(i, 2)
                emit_relu(i, 2)
                emit_sig(i, 1)
                emit_xupd(i, 1)
                if i < L - 1:
                    # fill the relu->psg ping-pong with next layer's work
                    bigmm(i + 1, 0, [0, 1])
                    emit_y(i + 1, [4, 5, 6, 7])
                emit_psg(i, 2)
                emit_sig(i, 2)
                emit_xupd(i, 2)
                if i == 1:
                    # denominator for the masked mean, on the idle DVE
                    den = workp.tile([HID, 1], f32, tag="den", name="den", bufs=1)
                    nc.vector.reduce_sum(den[:, :], mask_sb[:, :], axis=AX.X)
                    nc.vector.tensor_scalar_max(den[:, :], den[:, :], 1.0)
                    nc.vector.reciprocal(den[:, :], den[:, :])

            # ------- epilogue: masked mean -------
            xm = workp.tile([HID, 512], f32, tag="xm", name="xm", bufs=1)
            gacc = []
            for n in range(3):
                lo, hi = NS[n]
                ga = workp.tile([HID, 1], f32, tag=f"ga{n}", name=f"ga{n}", bufs=1)
                gacc.append(ga)
                nc.vector.scalar_tensor_tensor(
                    out=xm[:, 0 : hi - lo],
                    in0=xTa[0:D, lo:hi],
                    scalar=1.0,
                    in1=mask_sb[:, lo:hi],
                    op0=ALU.mult,
                    op1=ALU.mult,
                    accum_out=ga[:, :],
                )
            nc.vector.tensor_add(gacc[0][:, :], gacc[0][:, :], gacc[1][:, :])
            nc.vector.tensor_add(gacc[0][:, :], gacc[0][:, :], gacc[2][:, :])
            nc.vector.tensor_mul(gacc[0][:, :], gacc[0][:, :], den[:, :])
            nc.sync.dma_start(out=graphD, in_=gacc[0][:, :])

    nc.compile()
    return nc


def get_nc():
    if "nc" not in _NC_CACHE:
        _NC_CACHE["nc"] = _build_nc()
    return _NC_CACHE["nc"]


def make_in_maps(adj, mask_ids, ent_emb, rel_emb, Wb, Ww, bias, Wh, bh):
    adj = np.asarray(adj, dtype=np.float32)
    pad = np.zeros((B, R, EP, E2), dtype=FP8_NP)
    pad[:, :, :E, :E] = adj.transpose(0, 1, 3, 2).astype(FP8_NP)
    # [b, r, c, p, t, i] = adj[b, r, i, j = c*256 + t*128 + p]
    adjT = np.ascontiguousarray(
        pad.reshape(B, R, C2, 2, 128, E2).transpose(0, 1, 2, 4, 3, 5)
    )
    entT = np.ascontiguousarray(np.asarray(ent_emb, np.float32).T)
    relTh = np.ascontiguousarray(np.asarray(rel_emb, np.float32).T)
    Wb5 = np.asarray(Wb, np.float32).reshape(L, R, 2, D, NB)
    wbx = np.ascontiguousarray(Wb5[:, :, 0].transpose(0, 2, 1, 3).reshape(L, D, RNB))
    wbr = np.ascontiguousarray(Wb5[:, :, 1].transpose(0, 2, 1, 3).reshape(L, D, RNB))
    maskf = np.asarray(mask_ids).astype(np.float32)
    common = dict(
        xT0=entT, relT=relTh, wbx=wbx, wbr=wbr,
        ww=np.ascontiguousarray(np.asarray(Ww, np.float32)),
        wh=np.ascontiguousarray(np.asarray(Wh, np.float32)),
        biasL=np.ascontiguousarray(np.asarray(bias, np.float32)),
        bhL=np.ascontiguousarray(np.asarray(bh, np.float32)),
    )
    in_maps = []
    for c in range(8):
        b = c // 2
        m = dict(common)
        m["adjT"] = np.ascontiguousarray(adjT[b])
        mrep = np.zeros((HID, E2), np.float32)
        mrep[:, :E] = np.broadcast_to(maskf[b][None, :], (HID, E))
        m["maskrep"] = mrep
        in_maps.append(m)
    return in_maps


def run(inputs, trace=False):
    nc = get_nc()
    in_maps = make_in_maps(**{k: np.asarray(v) for k, v in inputs.items()})
    res = bass_utils.run_bass_kernel_spmd(
        nc, in_maps, core_ids=list(range(8)), trace=trace
    )
    out = np.stack(
        [np.asarray(res.results[2 * b]["graph"]).reshape(HID) for b in range(B)]
    ).astype(np.float32)
    return out, res


def kernel(**inputs):
    out, _ = run(inputs, trace=False)
    return out


# revision 10
# speedup vs baseline: 1.3873x; 1.0144x over previous
"""Trainium2 Bass kernel for the KGEncoder RGCN (nn_KGEncoder_14027363188782).

Math (per batch element b, L=5 layers):
    x0 = ent_emb                                             (E, D)
    per layer i:
      y_r   = x @ Wb_x[i,r] + 1 * c[i,r]^T    (E, NB)  where c[i,r] = rel_r @ Wb_rel[i,r]
      Z     = sum_r adj_r @ y_r               (E, NB)
      h     = relu(Z @ Ww[i] + bias[i])
      g     = sigmoid(h @ Wh[i] + bh[i])
      x     = x + g * (h - x)
    out_b = sum_e x[e] * m[e] / max(sum_e m[e], 1)

Sharding: core c handles b = c // 2 (pair-replicated, no collectives).
adj shipped pre-transposed, fp8 (exact for 0/1), DoubleRow layout
[r, c, p, t, i] with j = c*256 + t*128 + p.

Schedule: granular (r,c) adj DMAs; layer 0 accumulates in DMA-arrival
order (granule-major over the 3 psum i-chunks) so it finishes right at
DMA end; layers 1-4 run a software-pipelined schedule that keeps PE
busy continuously: bigmm i-chunks back-to-back, the highway tail and
the next layer's y matmuls interleaved into the bigmm instruction
stream, and the next layer's first bigmm group filling the last
tail's ACT->PE ping-pong. Tail matmuls use f32r moving operands
(1 cyc/row vs 4 for f32). The per-layer y uses an augmented
contraction row (x row 100 = 1, W row 100 = c[i]) so the relation
bias needs no extra matmul.
"""

import numpy as np
import ml_dtypes

import concourse.bacc as bacc
import concourse.bass as bass
import concourse.mybir as mybir
import concourse.tile as tile
from concourse import bass_utils
from concourse.bass import MemorySpace

B, R, E, D, HID, L, NB = 4, 10, 1500, 100, 100, 5, 3
EP = 1536           # j dim padded to 12*128
C2 = 6              # 256-row contraction chunks (DoubleRow)
E2 = 1504           # i dim padded to 16-aligned
YQ = 32             # y_all per-chunk col stride
CH = 12             # y chunks (128 j's each)
RNB = R * NB        # 30
NS = [(0, 512), (512, 1024), (1024, E2)]   # i-dim psum chunks
f32 = mybir.dt.float32
f32r = mybir.dt.float32r
fp8 = mybir.dt.float8e4
FP8_NP = ml_dtypes.float8_e4m3fn
AF = mybir.ActivationFunctionType
AX = mybir.AxisListType
ALU = mybir.AluOpType
DR = mybir.MatmulPerfMode.DoubleRow

ADJ_ORDER = [(r, c) for r in range(R) for c in range(C2)]

_NC_CACHE = {}


def _build_nc():
    nc = bacc.Bacc("TRN2", target_bir_lowering=False, debug=False)

    adjT = nc.dram_tensor("adjT", [R, C2, 128, 2, E2], fp8, kind="ExternalInput").ap()
    xT0 = nc.dram_tensor("xT0", [D, E], f32, kind="ExternalInput").ap()
    maskrep = nc.dram_tensor("maskrep", [HID, E2], f32, kind="ExternalInput").ap()
    relT = nc.dram_tensor("relT", [D, R], f32, kind="ExternalInput").ap()
    wbxD = nc.dram_tensor("wbx", [L, D, RNB], f32, kind="ExternalInput").ap()
    wbrD = nc.dram_tensor("wbr", [L, D, RNB], f32, kind="ExternalInput").ap()
    wwD = nc.dram_tensor("ww", [L, NB, HID], f32r, kind="ExternalInput").ap()
    whD = nc.dram_tensor("wh", [L, HID, HID], f32r, kind="ExternalInput").ap()
    biasD = nc.dram_tensor("biasL", [L, HID], f32, kind="ExternalInput").ap()
    bhD = nc.dram_tensor("bhL", [L, HID], f32, kind="ExternalInput").ap()
    graphD = nc.dram_tensor("graph", [HID, 1], f32, kind="ExternalOutput").ap()

    with tile.TileContext(nc) as tc:
        with (
            tc.tile_pool(name="singles", bufs=1) as singles,
            tc.tile_pool(name="ypool", bufs=2) as ypool,
            tc.tile_pool(name="workp", bufs=2) as workp,
            tc.tile_pool(name="pzp", bufs=1, space=MemorySpace.PSUM) as pzp,
            tc.tile_pool(name="pyp", bufs=2, space=MemorySpace.PSUM) as pyp,
            tc.tile_pool(name="php", bufs=1, space=MemorySpace.PSUM) as php,
            tc.tile_pool(name="pgp", bufs=1, space=MemorySpace.PSUM) as pgp,
        ):
            # ------- preamble: batched small loads (one DMA each) -------
            relT_sb = singles.tile([D, R], f32, tag="relT", name="relT_sb")
            nc.sync.dma_start(out=relT_sb[:, :], in_=relT)
            wbr_all = singles.tile([D, L * RNB], f32, tag="wbr", name="wbr_all")
            nc.sync.dma_start(
                out=wbr_all[:, :].rearrange("d (l q) -> d l q", l=L),
                in_=wbrD.rearrange("l d q -> d l q")
            )
            # rows 0:100 = Wbx[i] at cols 30i; row 100 = c[i] (written below)
            wbxa = singles.tile([D + 1, L * RNB], f32, tag="wbxa", name="wbxa")
            nc.sync.dma_start(
                out=wbxa[0:D, :].rearrange("d (l q) -> d l q", l=L),
                in_=wbxD.rearrange("l d q -> d l q")
            )
            xTa = singles.tile([D + 1, EP], f32, tag="xTa", name="xTa")
            nc.vector.memset(xTa[96 : D + 1, :], 1.0)
            nc.vector.memset(xTa[0:D, E:EP], 0.0)
            nc.sync.dma_start(out=xTa[0:D, 0:E], in_=xT0)

            # c[i, (r,q)] = rel_r @ Wbr[i, r] -> row 100 of wbxa (via ACT-queue
            # SBUF->SBUF DMA: ACT copy can't write partition base 100, and
            # the SP queue must stay free for the adj granule stream)
            psc = pyp.tile([128, 360], f32, tag="py", name="psc")
            for i in range(L):
                for r in range(R):
                    q = RNB * i + 3 * r
                    nc.tensor.matmul(
                        psc[0:1, q : q + 3],
                        relT_sb[:, r : r + 1],
                        wbr_all[:, q : q + 3],
                        start=True, stop=True,
                    )
            c_sb = workp.tile([1, L * RNB], f32, tag="c_sb", name="c_sb")
            nc.scalar.copy(out=c_sb[:, :], in_=psc[0:1, 0 : L * RNB])
            nc.scalar.dma_start(out=wbxa[D : D + 1, :], in_=c_sb[:, :])

            # ------- adj granule DMAs (the long pole; SP queue) -------
            res_tiles = []
            for r in range(R):
                rt = singles.tile([128, C2 * 2 * E2], fp8, tag=f"res{r}",
                                  name=f"res{r}")
                res_tiles.append(rt)
            for (r, c) in ADJ_ORDER:
                nc.sync.dma_start(
                    out=res_tiles[r][:, c * 2 * E2 : (c + 1) * 2 * E2],
                    in_=adjT[r, c].rearrange("p t i -> p (t i)"),
                )
            res_views = [
                res_tiles[r][:, :].rearrange("p (c t i) -> p c t i", c=C2, t=2)
                for r in range(R)
            ]

            # tail weights: issued after the granules (needed only once
            # layer-0's Z is complete, i.e. right at DMA end)
            ww_all = singles.tile([NB, L * HID], f32r, tag="ww", name="ww_all")
            nc.sync.dma_start(out=ww_all[:, :].rearrange("n (l h) -> n l h", l=L),
                              in_=wwD.rearrange("l n h -> n l h"))
            wh_all = singles.tile([HID, L * HID], f32r, tag="wh", name="wh_all")
            nc.sync.dma_start(out=wh_all[:, :].rearrange("p (l h) -> p l h", l=L),
                              in_=whD.rearrange("l p h -> p l h"))
            bias_all = singles.tile([HID, L], f32, tag="bias", name="bias_all")
            nc.sync.dma_start(out=bias_all[:, :], in_=biasD.rearrange("l h -> h l"))
            bh_all = singles.tile([HID, L], f32, tag="bh", name="bh_all")
            nc.sync.dma_start(out=bh_all[:, :], in_=bhD.rearrange("l h -> h l"))
            mask_sb = singles.tile([HID, E2], f32, tag="mask", name="mask_sb")
            nc.sync.dma_start(out=mask_sb[:, :], in_=maskrep)

            h_sb = singles.tile([HID, E2], f32r, tag="h", name="h_sb")

            # ------- per-layer emission helpers -------
            ydict = {}    # i -> (psy tile, y_all tile, y_view)
            pzt = {}      # (i, n) -> Z psum tile
            zcnt = {}     # (i, n) -> accumulation counter
            zsb = {}      # (i, n) -> z sbuf tile
            phd = {}      # (i, n) -> psh tile
            pgd = {}      # (i, n) -> psg tile

            def emit_y(i, ks):
                if i not in ydict:
                    psy = pyp.tile([128, 360], f32, tag="py", name=f"py{i}")
                    y_all = ypool.tile([128, CH * YQ], fp8, tag="y_all",
                                       name=f"y_all{i}")
                    yv = y_all[:, :].rearrange("p (k q) -> p k q", q=YQ)
                    ydict[i] = (psy, y_all, yv)
                psy, y_all, _ = ydict[i]
                for k in ks:
                    nc.tensor.matmul(
                        psy[:, 30 * k : 30 * k + 30],
                        xTa[:, 128 * k : 128 * (k + 1)],
                        wbxa[:, RNB * i : RNB * (i + 1)],
                        start=True, stop=True,
                    )
                k0, nk = ks[0], len(ks)
                nc.scalar.copy(
                    out=y_all[:, YQ * k0 : YQ * (k0 + nk)].rearrange(
                        "p (k q) -> p k q", q=YQ)[:, :, 0:RNB],
                    in_=psy[:, 30 * k0 : 30 * (k0 + nk)].rearrange(
                        "p (k q) -> p k q", q=RNB),
                )

            def bigmm(i, n, cs):
                if (i, n) not in pzt:
                    pzt[(i, n)] = pzp.tile([NB, 512], f32, tag=f"pz{n}",
                                           name=f"pz{i}_{n}")
                lo, hi = NS[n]
                nw = hi - lo
                out = pzt[(i, n)][:, 0:nw]
                yv = ydict[i][2]
                for c in cs:
                    for r in range(R):
                        cnt = zcnt.get((i, n), 0)
                        nc.tensor.matmul(
                            out,
                            yv[:, 2 * c : 2 * c + 2, 3 * r : 3 * r + 3],
                            res_views[r][:, c, :, lo:hi],
                            start=(cnt == 0),
                            stop=(cnt == R * C2 - 1),
                            perf_mode=DR,
                        )
                        zcnt[(i, n)] = cnt + 1

            def emit_zcopy(i, n):
                lo, hi = NS[n]
                nw = hi - lo
                zt = workp.tile([NB, 512], f32r, tag="z", name=f"z{i}_{n}")
                zsb[(i, n)] = zt
                nc.scalar.copy(out=zt[:, 0:nw], in_=pzt[(i, n)][:, 0:nw])

            def emit_psh(i, n):
                lo, hi = NS[n]
                nw = hi - lo
                ph = php.tile([HID, 512], f32, tag="ph", name=f"ph{i}_{n}")
                phd[(i, n)] = ph
                nc.tensor.matmul(
                    ph[:, 0:nw],
                    ww_all[:, HID * i : HID * (i + 1)],
                    zsb[(i, n)][:, 0:nw],
                    start=True, stop=True,
                )

            def emit_relu(i, n):
                lo, hi = NS[n]
                nc.scalar.activation(
                    h_sb[:, lo:hi], phd[(i, n)][:, 0 : hi - lo], AF.Relu,
                    bias=bias_all[:, i : i + 1],
                )

            def emit_psg(i, n):
                lo, hi = NS[n]
                nw = hi - lo
                pg = pgp.tile([HID, 512], f32, tag="pg", name=f"pg{i}_{n}")
                pgd[(i, n)] = pg
                nc.tensor.matmul(
                    pg[:, 0:nw],
                    wh_all[:, HID * i : HID * (i + 1)],
                    h_sb[:, lo:hi],
                    start=True, stop=True,
                )

            def emit_sig(i, n):
                lo, hi = NS[n]
                pg = pgd[(i, n)]
                nc.scalar.activation(
                    pg[:, 0 : hi - lo], pg[:, 0 : hi - lo], AF.Sigmoid,
                    bias=bh_all[:, i : i + 1],
                )

            def emit_xupd(i, n):
                lo, hi = NS[n]
                pg = pgd[(i, n)]
                for a in range(lo, hi, 256):
                    b = min(a + 256, hi)
                    nc.vector.tensor_sub(
                        h_sb[:, a:b], h_sb[:, a:b], xTa[0:D, a:b])
                    nc.vector.tensor_mul(
                        h_sb[:, a:b], h_sb[:, a:b], pg[:, a - lo : b - lo])
                    nc.vector.tensor_add(
                        xTa[0:D, a:b], xTa[0:D, a:b], h_sb[:, a:b])

            # ------- layer 0 (DMA-arrival order, granule-major) -------
            emit_y(0, [0, 1, 2, 3])
            emit_y(0, [4, 5, 6, 7])
            emit_y(0, [8, 9, 10, 11])
            for n in range(3):
                pzt[(0, n)] = pzp.tile([NB, 512], f32, tag=f"pz{n}", name=f"pz0_{n}")
            yv0 = ydict[0][2]
            for g, (r, c) in enumerate(ADJ_ORDER):
                for n in range(3):
                    lo, hi = NS[n]
                    nc.tensor.matmul(
                        pzt[(0, n)][:, 0 : hi - lo],
                        yv0[:, 2 * c : 2 * c + 2, 3 * r : 3 * r + 3],
                        res_views[r][:, c, :, lo:hi],
                        start=(g == 0),
                        stop=(g == len(ADJ_ORDER) - 1),
                        perf_mode=DR,
                    )
            emit_zcopy(0, 0)
            emit_psh(0, 0)
            emit_relu(0, 0)
            emit_zcopy(0, 1)
            emit_psh(0, 1)
            emit_relu(0, 1)
            emit_psg(0, 0)
            emit_sig(0, 0)
            emit_xupd(0, 0)
            emit_y(1, [0, 1, 2, 3])
            emit_zcopy(0, 2)
            emit_psh(0, 2)
            emit_relu(0, 2)
            emit_psg(0, 1)
            emit_sig(0, 1)
            emit_xupd(0, 1)
            emit_y(1, [4, 5, 6, 7])
            emit_psg(0, 2)
            emit_sig(0, 2)
            emit_xupd(0, 2)
            bigmm(1, 0, [0, 1])
            bigmm(1, 0, [2, 3])

            # ------- layers 1..4 (software-pipelined) -------
            # entry state per layer i: bigmm(i,0,[0,1]) and y(i) k0-7 already
            # emitted by the predecessor; y(i) k8-11 still pending.
            # invariant entering layer i: bigmm(i,0,[0..3]) and y(i) k0-7
            # already emitted; y(i) k8-11 pending. Each psh/relu pair issues
            # a full bigmm group before its psg consumer so the serial ACT
            # queue (zcopy/relu/sig/y-copy, ~0.6us each) never blocks PE.
            for i in range(1, L):
                emit_y(i, [8, 9, 10, 11])
                bigmm(i, 0, [4, 5])
                emit_zcopy(i, 0)
                bigmm(i, 1, [0, 1])
                emit_psh(i, 0)
                emit_relu(i, 0)
                bigmm(i, 1, [2, 3, 4, 5])
                emit_psg(i, 0)
                emit_sig(i, 0)
                emit_zcopy(i, 1)
                emit_xupd(i, 0)
                bigmm(i, 2, [0, 1])
                emit_psh(i, 1)
                emit_relu(i, 1)
                if i < L - 1:
                    emit_y(i + 1, [0, 1, 2, 3])
                bigmm(i, 2, [2, 3, 4, 5])
                emit_psg(i, 1)
                emit_sig(i, 1)
                emit_zcopy(i, 2)
                emit_xupd(i, 1)
                if i < L - 1:
                    bigmm(i + 1, 0, [0, 1])
                emit_psh(i, 2)
                emit_relu(i, 2)
                if i < L - 1:
                    emit_y(i + 1, [4, 5, 6, 7])
                    bigmm(i + 1, 0, [2, 3])
                emit_psg(i, 2)
                emit_sig(i, 2)
                emit_xupd(i, 2)
                if i == 1:
                    # denominator for the masked mean, on the idle DVE
                    den = workp.tile([HID, 1], f32, tag="den", name="den", bufs=1)
                    nc.vector.reduce_sum(den[:, :], mask_sb[:, :], axis=AX.X)
                    nc.vector.tensor_scalar_max(den[:, :], den[:, :], 1.0)
                    nc.vector.reciprocal(den[:, :], den[:, :])

            # ------- epilogue: masked mean -------
            xm = workp.tile([HID, 512], f32, tag="xm", name="xm", bufs=1)
            gacc = []
            for n in range(3):
                lo, hi = NS[n]
                ga = workp.tile([HID, 1], f32, tag=f"ga{n}", name=f"ga{n}", bufs=1)
                gacc.append(ga)
                nc.vector.scalar_tensor_tensor(
                    out=xm[:, 0 : hi - lo],
                    in0=xTa[0:D, lo:hi],
                    scalar=1.0,
                    in1=mask_sb[:, lo:hi],
                    op0=ALU.mult,
                    op1=ALU.mult,
                    accum_out=ga[:, :],
                )
            nc.vector.tensor_add(gacc[0][:, :], gacc[0][:, :], gacc[1][:, :])
            nc.vector.tensor_add(gacc[0][:, :], gacc[0][:, :], gacc[2][:, :])
            nc.vector.tensor_mul(gacc[0][:, :], gacc[0][:, :], den[:, :])
            nc.sync.dma_start(out=graphD, in_=gacc[0][:, :])

    nc.compile()
    return nc


def get_nc():
    if "nc" not in _NC_CACHE:
        _NC_CACHE["nc"] = _build_nc()
    return _NC_CACHE["nc"]


def make_in_maps(adj, mask_ids, ent_emb, rel_emb, Wb, Ww, bias, Wh, bh):
    adj = np.asarray(adj, dtype=np.float32)
    pad = np.zeros((B, R, EP, E2), dtype=FP8_NP)
    pad[:, :, :E, :E] = adj.transpose(0, 1, 3, 2).astype(FP8_NP)
    # [b, r, c, p, t, i] = adj[b, r, i, j = c*256 + t*128 + p]
    adjT = np.ascontiguousarray(
        pad.reshape(B, R, C2, 2, 128, E2).transpose(0, 1, 2, 4, 3, 5)
    )
    entT = np.ascontiguousarray(np.asarray(ent_emb, np.float32).T)
    relTh = np.ascontiguousarray(np.asarray(rel_emb, np.float32).T)
    Wb5 = np.asarray(Wb, np.float32).reshape(L, R, 2, D, NB)
    wbx = np.ascontiguousarray(Wb5[:, :, 0].transpose(0, 2, 1, 3).reshape(L, D, RNB))
    wbr = np.ascontiguousarray(Wb5[:, :, 1].transpose(0, 2, 1, 3).reshape(L, D, RNB))
    maskf = np.asarray(mask_ids).astype(np.float32)
    common = dict(
        xT0=entT, relT=relTh, wbx=wbx, wbr=wbr,
        ww=np.ascontiguousarray(np.asarray(Ww, np.float32)),
        wh=np.ascontiguousarray(np.asarray(Wh, np.float32)),
        biasL=np.ascontiguousarray(np.asarray(bias, np.float32)),
        bhL=np.ascontiguousarray(np.asarray(bh, np.float32)),
    )
    in_maps = []
    for c in range(8):
        b = c // 2
        m = dict(common)
        m["adjT"] = np.ascontiguousarray(adjT[b])
        mrep = np.zeros((HID, E2), np.float32)
        mrep[:, :E] = np.broadcast_to(maskf[b][None, :], (HID, E))
        m["maskrep"] = mrep
        in_maps.append(m)
    return in_maps


def run(inputs, trace=False):
    nc = get_nc()
    in_maps = make_in_maps(**{k: np.asarray(v) for k, v in inputs.items()})
    res = bass_utils.run_bass_kernel_spmd(
        nc, in_maps, core_ids=list(range(8)), trace=trace
    )
    out = np.stack(
        [np.asarray(res.results[2 * b]["graph"]).reshape(HID) for b in range(B)]
    ).astype(np.float32)
    return out, res


def kernel(**inputs):
    out, _ = run(inputs, trace=False)
    return out


# revision 14
# speedup vs baseline: 1.3881x; 1.0006x over previous
"""Trainium2 Bass kernel for the KGEncoder RGCN (nn_KGEncoder_14027363188782).

Math (per batch element b, L=5 layers):
    x0 = ent_emb                                             (E, D)
    per layer i:
      y_r   = x @ Wb_x[i,r] + 1 * c[i,r]^T    (E, NB)  where c[i,r] = rel_r @ Wb_rel[i,r]
      Z     = sum_r adj_r @ y_r               (E, NB)
      h     = relu(Z @ Ww[i] + bias[i])
      g     = sigmoid(h @ Wh[i] + bh[i])
      x     = x + g * (h - x)
    out_b = sum_e x[e] * m[e] / max(sum_e m[e], 1)

Sharding: core c handles b = c // 2 (pair-replicated, no collectives).
adj shipped pre-transposed, fp8 (exact for 0/1), DoubleRow layout
[r, c, p, t, i] with j = c*256 + t*128 + p.

Schedule: granular (r,c) adj DMAs; layer 0 accumulates in DMA-arrival
order (granule-major over the 3 psum i-chunks) so it finishes right at
DMA end; layers 1-4 run a software-pipelined schedule that keeps PE
busy continuously: bigmm i-chunks back-to-back, the highway tail and
the next layer's y matmuls interleaved into the bigmm instruction
stream, and the next layer's first bigmm group filling the last
tail's ACT->PE ping-pong. Tail matmuls use f32r moving operands
(1 cyc/row vs 4 for f32). The per-layer y uses an augmented
contraction row (x row 100 = 1, W row 100 = c[i]) so the relation
bias needs no extra matmul.
"""

import numpy as np
import ml_dtypes

import concourse.bacc as bacc
import concourse.bass as bass
import concourse.mybir as mybir
import concourse.tile as tile
from concourse import bass_utils
from concourse.bass import MemorySpace

B, R, E, D, HID, L, NB = 4, 10, 1500, 100, 100, 5, 3
EP = 1536           # j dim padded to 12*128
C2 = 6              # 256-row contraction chunks (DoubleRow)
E2 = 1504           # i dim padded to 16-aligned
YQ = 32             # y_all per-chunk col stride
CH = 12             # y chunks (128 j's each)
RNB = R * NB        # 30
NS = [(0, 512), (512, 1024), (1024, E2)]   # i-dim psum chunks
f32 = mybir.dt.float32
f32r = mybir.dt.float32r
fp8 = mybir.dt.float8e4
FP8_NP = ml_dtypes.float8_e4m3fn
AF = mybir.ActivationFunctionType
AX = mybir.AxisListType
ALU = mybir.AluOpType
DR = mybir.MatmulPerfMode.DoubleRow

ADJ_ORDER = [(r, c) for r in range(R) for c in range(C2)]

_NC_CACHE = {}


def _build_nc():
    nc = bacc.Bacc("TRN2", target_bir_lowering=False, debug=False)

    adjT = nc.dram_tensor("adjT", [R, C2, 128, 2, E2], fp8, kind="ExternalInput").ap()
    xT0 = nc.dram_tensor("xT0", [D, E], f32, kind="ExternalInput").ap()
    maskrep = nc.dram_tensor("maskrep", [HID, E2], f32, kind="ExternalInput").ap()
    relT = nc.dram_tensor("relT", [D, R], f32, kind="ExternalInput").ap()
    wbxD = nc.dram_tensor("wbx", [L, D, RNB], f32, kind="ExternalInput").ap()
    wbrD = nc.dram_tensor("wbr", [L, D, RNB], f32, kind="ExternalInput").ap()
    wwD = nc.dram_tensor("ww", [L, NB, HID], f32r, kind="ExternalInput").ap()
    whD = nc.dram_tensor("wh", [L, HID, HID], f32r, kind="ExternalInput").ap()
    biasD = nc.dram_tensor("biasL", [L, HID], f32, kind="ExternalInput").ap()
    bhD = nc.dram_tensor("bhL", [L, HID], f32, kind="ExternalInput").ap()
    graphD = nc.dram_tensor("graph", [HID, 1], f32, kind="ExternalOutput").ap()

    with tile.TileContext(nc) as tc:
        with (
            tc.tile_pool(name="singles", bufs=1) as singles,
            tc.tile_pool(name="ypool", bufs=2) as ypool,
            tc.tile_pool(name="workp", bufs=2) as workp,
            tc.tile_pool(name="pzp", bufs=1, space=MemorySpace.PSUM) as pzp,
            tc.tile_pool(name="pyp", bufs=2, space=MemorySpace.PSUM) as pyp,
            tc.tile_pool(name="php", bufs=1, space=MemorySpace.PSUM) as php,
            tc.tile_pool(name="pgp", bufs=1, space=MemorySpace.PSUM) as pgp,
        ):
            # ------- preamble: batched small loads (one DMA each) -------
            relT_sb = singles.tile([D, R], f32, tag="relT", name="relT_sb")
            nc.scalar.dma_start(out=relT_sb[:, :], in_=relT)
            wbr_all = singles.tile([D, L * RNB], f32, tag="wbr", name="wbr_all")
            nc.scalar.dma_start(
                out=wbr_all[:, :].rearrange("d (l q) -> d l q", l=L),
                in_=wbrD.rearrange("l d q -> d l q")
            )
            # rows 0:100 = Wbx[i] at cols 30i; row 100 = c[i] (written below)
            wbxa = singles.tile([D + 1, L * RNB], f32, tag="wbxa", name="wbxa")
            nc.scalar.dma_start(
                out=wbxa[0:D, :].rearrange("d (l q) -> d l q", l=L),
                in_=wbxD.rearrange("l d q -> d l q")
            )
            xTa = singles.tile([D + 1, EP], f32, tag="xTa", name="xTa")
            nc.vector.memset(xTa[96 : D + 1, :], 1.0)
            nc.vector.memset(xTa[0:D, E:EP], 0.0)
            nc.scalar.dma_start(out=xTa[0:D, 0:E], in_=xT0)

            # ------- adj granule DMAs (the long pole; SP queue) -------
            res_tiles = []
            for r in range(R):
                rt = singles.tile([128, C2 * 2 * E2], fp8, tag=f"res{r}",
                                  name=f"res{r}")
                res_tiles.append(rt)
            for (r, c) in ADJ_ORDER:
                nc.sync.dma_start(
                    out=res_tiles[r][:, c * 2 * E2 : (c + 1) * 2 * E2],
                    in_=adjT[r, c].rearrange("p t i -> p (t i)"),
                )
            res_views = [
                res_tiles[r][:, :].rearrange("p (c t i) -> p c t i", c=C2, t=2)
                for r in range(R)
            ]

            # c[i, (r,q)] = rel_r @ Wbr[i, r] -> row 100 of wbxa (via ACT-queue
            # SBUF->SBUF DMA: ACT copy can't write partition base 100, and
            # the SP queue must stay free for the adj granule stream)
            psc = pyp.tile([128, 360], f32, tag="py", name="psc")
            for i in range(L):
                for r in range(R):
                    q = RNB * i + 3 * r
                    nc.tensor.matmul(
                        psc[0:1, q : q + 3],
                        relT_sb[:, r : r + 1],
                        wbr_all[:, q : q + 3],
                        start=True, stop=True,
                    )
            c_sb = workp.tile([1, L * RNB], f32, tag="c_sb", name="c_sb")
            nc.scalar.copy(out=c_sb[:, :], in_=psc[0:1, 0 : L * RNB])
            nc.scalar.dma_start(out=wbxa[D : D + 1, :], in_=c_sb[:, :])

            # tail weights: issued after the granules (needed only once
            # layer-0's Z is complete, i.e. right at DMA end)
            ww_all = singles.tile([NB, L * HID], f32r, tag="ww", name="ww_all")
            nc.sync.dma_start(out=ww_all[:, :].rearrange("n (l h) -> n l h", l=L),
                              in_=wwD.rearrange("l n h -> n l h"))
            wh_all = singles.tile([HID, L * HID], f32r, tag="wh", name="wh_all")
            nc.sync.dma_start(out=wh_all[:, :].rearrange("p (l h) -> p l h", l=L),
                              in_=whD.rearrange("l p h -> p l h"))
            bias_all = singles.tile([HID, L], f32, tag="bias", name="bias_all")
            nc.sync.dma_start(out=bias_all[:, :], in_=biasD.rearrange("l h -> h l"))
            bh_all = singles.tile([HID, L], f32, tag="bh", name="bh_all")
            nc.sync.dma_start(out=bh_all[:, :], in_=bhD.rearrange("l h -> h l"))
            mask_sb = singles.tile([HID, E2], f32, tag="mask", name="mask_sb")
            nc.sync.dma_start(out=mask_sb[:, :], in_=maskrep)

            h_sb = singles.tile([HID, E2], f32r, tag="h", name="h_sb")

            # ------- per-layer emission helpers -------
            ydict = {}    # i -> (psy tile, y_all tile, y_view)
            pzt = {}      # (i, n) -> Z psum tile
            zcnt = {}     # (i, n) -> accumulation counter
            zsb = {}      # (i, n) -> z sbuf tile
            phd = {}      # (i, n) -> psh tile
            pgd = {}      # (i, n) -> psg tile

            def emit_y(i, ks):
                if i not in ydict:
                    psy = pyp.tile([128, 360], f32, tag="py", name=f"py{i}")
                    y_all = ypool.tile([128, CH * YQ], fp8, tag="y_all",
                                       name=f"y_all{i}")
                    yv = y_all[:, :].rearrange("p (k q) -> p k q", q=YQ)
                    ydict[i] = (psy, y_all, yv)
                psy, y_all, _ = ydict[i]
                for k in ks:
                    nc.tensor.matmul(
                        psy[:, 30 * k : 30 * k + 30],
                        xTa[:, 128 * k : 128 * (k + 1)],
                        wbxa[:, RNB * i : RNB * (i + 1)],
                        start=True, stop=True,
                    )
                k0, nk = ks[0], len(ks)
                nc.scalar.copy(
                    out=y_all[:, YQ * k0 : YQ * (k0 + nk)].rearrange(
                        "p (k q) -> p k q", q=YQ)[:, :, 0:RNB],
                    in_=psy[:, 30 * k0 : 30 * (k0 + nk)].rearrange(
                        "p (k q) -> p k q", q=RNB),
                )

            def bigmm(i, n, cs):
                if (i, n) not in pzt:
                    pzt[(i, n)] = pzp.tile([NB, 512], f32, tag=f"pz{n}",
                                           name=f"pz{i}_{n}")
                lo, hi = NS[n]
                nw = hi - lo
                out = pzt[(i, n)][:, 0:nw]
                yv = ydict[i][2]
                for c in cs:
                    for r in range(R):
                        cnt = zcnt.get((i, n), 0)
                        nc.tensor.matmul(
                            out,
                            yv[:, 2 * c : 2 * c + 2, 3 * r : 3 * r + 3],
                            res_views[r][:, c, :, lo:hi],
                            start=(cnt == 0),
                            stop=(cnt == R * C2 - 1),
                            perf_mode=DR,
                        )
                        zcnt[(i, n)] = cnt + 1

            def emit_zcopy(i, n):
                lo, hi = NS[n]
                nw = hi - lo
                zt = workp.tile([NB, 512], f32r, tag="z", name=f"z{i}_{n}", bufs=1)
                zsb[(i, n)] = zt
                nc.scalar.copy(out=zt[:, 0:nw], in_=pzt[(i, n)][:, 0:nw])

            def emit_psh(i, n):
                lo, hi = NS[n]
                nw = hi - lo
                ph = php.tile([HID, 512], f32, tag="ph", name=f"ph{i}_{n}")
                phd[(i, n)] = ph
                nc.tensor.matmul(
                    ph[:, 0:nw],
                    ww_all[:, HID * i : HID * (i + 1)],
                    zsb[(i, n)][:, 0:nw],
                    start=True, stop=True,
                )

            def emit_relu(i, n):
                lo, hi = NS[n]
                nc.scalar.activation(
                    h_sb[:, lo:hi], phd[(i, n)][:, 0 : hi - lo], AF.Relu,
                    bias=bias_all[:, i : i + 1],
                )

            def emit_psg(i, n):
                lo, hi = NS[n]
                nw = hi - lo
                pg = pgp.tile([HID, 512], f32, tag="pg", name=f"pg{i}_{n}")
                pgd[(i, n)] = pg
                nc.tensor.matmul(
                    pg[:, 0:nw],
                    wh_all[:, HID * i : HID * (i + 1)],
                    h_sb[:, lo:hi],
                    start=True, stop=True,
                )

            def emit_sig(i, n):
                lo, hi = NS[n]
                pg = pgd[(i, n)]
                nc.scalar.activation(
                    pg[:, 0 : hi - lo], pg[:, 0 : hi - lo], AF.Sigmoid,
                    bias=bh_all[:, i : i + 1],
                )

            def emit_xupd(i, n):
                lo, hi = NS[n]
                pg = pgd[(i, n)]
                for a in range(lo, hi, 256):
                    b = min(a + 256, hi)
                    nc.vector.tensor_sub(
                        h_sb[:, a:b], h_sb[:, a:b], xTa[0:D, a:b])
                    nc.vector.tensor_mul(
                        h_sb[:, a:b], h_sb[:, a:b], pg[:, a - lo : b - lo])
                    nc.vector.tensor_add(
                        xTa[0:D, a:b], xTa[0:D, a:b], h_sb[:, a:b])

            ga = [workp.tile([HID, 1], f32, tag=f"ga{k}", name=f"ga{k}",
                             bufs=1) for k in range(4)]
            xmp = workp.tile([HID, 512], f32, tag="xmp", name="xmp", bufs=1)

            def emit_mask_part(n):
                l2, h2 = NS[n]
                nc.vector.scalar_tensor_tensor(
                    out=xmp[:, 0 : h2 - l2], in0=xTa[0:D, l2:h2], scalar=1.0,
                    in1=mask_sb[:, l2:h2], op0=ALU.mult, op1=ALU.mult,
                    accum_out=ga[n][:, :])

            # ------- layer 0 (DMA-arrival order, granule-major) -------
            emit_y(0, [0, 1, 2, 3])
            emit_y(0, [4, 5, 6, 7])
            emit_y(0, [8, 9, 10, 11])
            for n in range(3):
                pzt[(0, n)] = pzp.tile([NB, 512], f32, tag=f"pz{n}", name=f"pz0_{n}")
            yv0 = ydict[0][2]
            for g, (r, c) in enumerate(ADJ_ORDER):
                for n in range(3):
                    lo, hi = NS[n]
                    nc.tensor.matmul(
                        pzt[(0, n)][:, 0 : hi - lo],
                        yv0[:, 2 * c : 2 * c + 2, 3 * r : 3 * r + 3],
                        res_views[r][:, c, :, lo:hi],
                        start=(g == 0),
                        stop=(g == len(ADJ_ORDER) - 1),
                        perf_mode=DR,
                    )
            emit_zcopy(0, 0)
            emit_psh(0, 0)
            emit_relu(0, 0)
            emit_zcopy(0, 1)
            emit_psh(0, 1)
            emit_relu(0, 1)
            emit_psg(0, 0)
            emit_sig(0, 0)
            emit_xupd(0, 0)
            emit_y(1, [0, 1, 2, 3])
            emit_zcopy(0, 2)
            emit_psh(0, 2)
            emit_relu(0, 2)
            emit_psg(0, 1)
            emit_sig(0, 1)
            emit_xupd(0, 1)
            emit_y(1, [4, 5, 6, 7])
            emit_psg(0, 2)
            emit_sig(0, 2)
            emit_xupd(0, 2)
            bigmm(1, 0, [0, 1])
            bigmm(1, 0, [2, 3])

            # ------- layers 1..4 (software-pipelined) -------
            # entry state per layer i: bigmm(i,0,[0,1]) and y(i) k0-7 already
            # emitted by the predecessor; y(i) k8-11 still pending.
            # invariant entering layer i: bigmm(i,0,[0..3]) and y(i) k0-7
            # already emitted; y(i) k8-11 pending. Each psh/relu pair issues
            # a full bigmm group before its psg consumer so the serial ACT
            # queue (zcopy/relu/sig/y-copy, ~0.6us each) never blocks PE.
            for i in range(1, L):
                emit_y(i, [8, 9, 10, 11])
                bigmm(i, 0, [4, 5])
                emit_zcopy(i, 0)
                bigmm(i, 1, [0, 1])
                emit_psh(i, 0)
                emit_relu(i, 0)
                bigmm(i, 1, [2, 3, 4, 5])
                emit_psg(i, 0)
                emit_sig(i, 0)
                emit_zcopy(i, 1)
                emit_xupd(i, 0)
                if i == L - 1:
                    emit_mask_part(0)
                bigmm(i, 2, [0, 1])
                emit_psh(i, 1)
                emit_relu(i, 1)
                if i < L - 1:
                    emit_y(i + 1, [0, 1, 2, 3])
                bigmm(i, 2, [2, 3, 4, 5])
                emit_psg(i, 1)
                emit_sig(i, 1)
                emit_zcopy(i, 2)
                emit_xupd(i, 1)
                if i == L - 1:
                    emit_mask_part(1)
                if i < L - 1:
                    bigmm(i + 1, 0, [0, 1])
                    emit_psh(i, 2)
                    emit_relu(i, 2)
                    emit_y(i + 1, [4, 5, 6, 7])
                    bigmm(i + 1, 0, [2, 3])
                    emit_psg(i, 2)
                    emit_sig(i, 2)
                    emit_xupd(i, 2)
                if i == 1:
                    # denominator for the masked mean, on the idle DVE
                    den = workp.tile([HID, 1], f32, tag="den", name="den", bufs=1)
                    nc.vector.reduce_sum(den[:, :], mask_sb[:, :], axis=AX.X)
                    nc.vector.tensor_scalar_max(den[:, :], den[:, :], 1.0)
                    nc.vector.reciprocal(den[:, :], den[:, :])

            # ------- layer-4 final chunk (fine-grained halves) + epilogue ---
            # Pool computes the n=0/1 mask partials in parallel with the
            # final chunk's tail chain; the last chunk runs in two 240-col
            # halves so PE/ACT/DVE pipeline with minimal exposed latency.
            lo, hi = NS[2]
            mid = lo + 240
            ph4 = php.tile([HID, 512], f32, tag="ph", name="ph4_2")
            pg4 = pgp.tile([HID, 512], f32, tag="pg", name="pg4_2")
            zt4 = zsb[(L - 1, 2)]
            xm = workp.tile([HID, 240], f32, tag="xm", name="xm", bufs=1)
            i = L - 1
            for hvi, (a, b) in enumerate([(lo, mid), (mid, hi)]):
                al, bl = a - lo, b - lo
                nc.tensor.matmul(
                    ph4[:, al:bl], ww_all[:, HID * i : HID * (i + 1)],
                    zt4[:, al:bl], start=True, stop=True)
                nc.scalar.activation(
                    h_sb[:, a:b], ph4[:, al:bl], AF.Relu,
                    bias=bias_all[:, i : i + 1])
                nc.tensor.matmul(
                    pg4[:, al:bl], wh_all[:, HID * i : HID * (i + 1)],
                    h_sb[:, a:b], start=True, stop=True)
                nc.scalar.activation(
                    pg4[:, al:bl], pg4[:, al:bl], AF.Sigmoid,
                    bias=bh_all[:, i : i + 1])
                nc.vector.tensor_sub(h_sb[:, a:b], h_sb[:, a:b], xTa[0:D, a:b])
                nc.vector.tensor_mul(h_sb[:, a:b], h_sb[:, a:b], pg4[:, al:bl])
                nc.vector.tensor_add(xTa[0:D, a:b], xTa[0:D, a:b], h_sb[:, a:b])
                nc.vector.scalar_tensor_tensor(
                    out=xm[:, 0 : b - a], in0=xTa[0:D, a:b], scalar=1.0,
                    in1=mask_sb[:, a:b], op0=ALU.mult, op1=ALU.mult,
                    accum_out=ga[2 + hvi][:, :])
            nc.vector.tensor_add(ga[0][:, :], ga[0][:, :], ga[1][:, :])
            nc.vector.tensor_add(ga[2][:, :], ga[2][:, :], ga[3][:, :])
            nc.vector.tensor_add(ga[0][:, :], ga[0][:, :], ga[2][:, :])
            nc.vector.tensor_mul(ga[0][:, :], ga[0][:, :], den[:, :])
            nc.sync.dma_start(out=graphD, in_=ga[0][:, :])

    nc.compile()
    return nc


def get_nc():
    if "nc" not in _NC_CACHE:
        _NC_CACHE["nc"] = _build_nc()
    return _NC_CACHE["nc"]


def make_in_maps(adj, mask_ids, ent_emb, rel_emb, Wb, Ww, bias, Wh, bh):
    adj = np.asarray(adj, dtype=np.float32)
    pad = np.zeros((B, R, EP, E2), dtype=FP8_NP)
    pad[:, :, :E, :E] = adj.transpose(0, 1, 3, 2).astype(FP8_NP)
    # [b, r, c, p, t, i] = adj[b, r, i, j = c*256 + t*128 + p]
    adjT = np.ascontiguousarray(
        pad.reshape(B, R, C2, 2, 128, E2).transpose(0, 1, 2, 4, 3, 5)
    )
    entT = np.ascontiguousarray(np.asarray(ent_emb, np.float32).T)
    relTh = np.ascontiguousarray(np.asarray(rel_emb, np.float32).T)
    Wb5 = np.asarray(Wb, np.float32).reshape(L, R, 2, D, NB)
    wbx = np.ascontiguousarray(Wb5[:, :, 0].transpose(0, 2, 1, 3).reshape(L, D, RNB))
    wbr = np.ascontiguousarray(Wb5[:, :, 1].transpose(0, 2, 1, 3).reshape(L, D, RNB))
    maskf = np.asarray(mask_ids).astype(np.float32)
    common = dict(
        xT0=entT, relT=relTh, wbx=wbx, wbr=wbr,
        ww=np.ascontiguousarray(np.asarray(Ww, np.float32)),
        wh=np.ascontiguousarray(np.asarray(Wh, np.float32)),
        biasL=np.ascontiguousarray(np.asarray(bias, np.float32)),
        bhL=np.ascontiguousarray(np.asarray(bh, np.float32)),
    )
    in_maps = []
    for c in range(8):
        b = c // 2
        m = dict(common)
        m["adjT"] = np.ascontiguousarray(adjT[b])
        mrep = np.zeros((HID, E2), np.float32)
        mrep[:, :E] = np.broadcast_to(maskf[b][None, :], (HID, E))
        m["maskrep"] = mrep
        in_maps.append(m)
    return in_maps


def run(inputs, trace=False):
    nc = get_nc()
    in_maps = make_in_maps(**{k: np.asarray(v) for k, v in inputs.items()})
    res = bass_utils.run_bass_kernel_spmd(
        nc, in_maps, core_ids=list(range(8)), trace=trace
    )
    out = np.stack(
        [np.asarray(res.results[2 * b]["graph"]).reshape(HID) for b in range(B)]
    ).astype(np.float32)
    return out, res


def kernel(**inputs):
    out, _ = run(inputs, trace=False)
    return out


# revision 17
# speedup vs baseline: 1.4161x; 1.0202x over previous
"""Trainium2 Bass kernel for the KGEncoder RGCN (nn_KGEncoder_14027363188782).

Math (per batch element b, L=5 layers):
    x0 = ent_emb                                             (E, D)
    per layer i:
      y_r   = x @ Wb_x[i,r] + 1 * c[i,r]^T    (E, NB)  where c[i,r] = rel_r @ Wb_rel[i,r]
      Z     = sum_r adj_r @ y_r               (E, NB)
      h     = relu(Z @ Ww[i] + bias[i])
      g     = sigmoid(h @ Wh[i] + bh[i])
      x     = x + g * (h - x)
    out_b = sum_e x[e] * m[e] / max(sum_e m[e], 1)

Sharding: core c handles b = c // 2 (pair-replicated, no collectives).
adj shipped pre-transposed, fp8 (exact for 0/1), DoubleRow layout
[r, c, p, t, i] with j = c*256 + t*128 + p.

Schedule: granular (r,c) adj DMAs; layer 0 accumulates in DMA-arrival
order (granule-major over the 3 psum i-chunks) so it finishes right at
DMA end; layers 1-4 run a software-pipelined schedule that keeps PE
busy continuously: bigmm i-chunks back-to-back, the highway tail and
the next layer's y matmuls interleaved into the bigmm instruction
stream, and the next layer's first bigmm group filling the last
tail's ACT->PE ping-pong. Tail matmuls use f32r moving operands
(1 cyc/row vs 4 for f32). The per-layer y uses an augmented
contraction row (x row 100 = 1, W row 100 = c[i]) so the relation
bias needs no extra matmul.
"""

import numpy as np
import ml_dtypes

import concourse.bacc as bacc
import concourse.bass as bass
import concourse.mybir as mybir
import concourse.tile as tile
from concourse import bass_utils
from concourse.bass import MemorySpace

B, R, E, D, HID, L, NB = 4, 10, 1500, 100, 100, 5, 3
EP = 1536           # j dim padded to 12*128
C2 = 6              # 256-row contraction chunks (DoubleRow)
E2 = 1504           # i dim padded to 16-aligned
YQ = 32             # y_all per-chunk col stride
CH = 12             # y chunks (128 j's each)
RNB = R * NB        # 30
NS = [(0, 512), (512, 1024), (1024, E2)]   # i-dim psum chunks
f32 = mybir.dt.float32
f32r = mybir.dt.float32r
fp8 = mybir.dt.float8e4
FP8_NP = ml_dtypes.float8_e4m3fn
AF = mybir.ActivationFunctionType
AX = mybir.AxisListType
ALU = mybir.AluOpType
DR = mybir.MatmulPerfMode.DoubleRow

ADJ_ORDER = [(r, c) for r in range(R) for c in range(C2)]

_NC_CACHE = {}


def _build_nc():
    nc = bacc.Bacc("TRN2", target_bir_lowering=False, debug=False)

    adjT = nc.dram_tensor("adjT", [R, C2, 128, 2, E2], fp8, kind="ExternalInput").ap()
    xT0 = nc.dram_tensor("xT0", [D, E], f32, kind="ExternalInput").ap()
    maskrep = nc.dram_tensor("maskrep", [HID, E2], f32, kind="ExternalInput").ap()
    mask1 = nc.dram_tensor("mask1", [1, E2], f32, kind="ExternalInput").ap()
    relT = nc.dram_tensor("relT", [D, R], f32, kind="ExternalInput").ap()
    wbxD = nc.dram_tensor("wbx", [L, D, RNB], f32, kind="ExternalInput").ap()
    wbrD = nc.dram_tensor("wbr", [L, D, RNB], f32, kind="ExternalInput").ap()
    wwD = nc.dram_tensor("ww", [L, NB, HID], f32r, kind="ExternalInput").ap()
    whD = nc.dram_tensor("wh", [L, HID, HID], f32r, kind="ExternalInput").ap()
    biasD = nc.dram_tensor("biasL", [L, HID], f32, kind="ExternalInput").ap()
    bhD = nc.dram_tensor("bhL", [L, HID], f32, kind="ExternalInput").ap()
    graphD = nc.dram_tensor("graph", [HID, 1], f32, kind="ExternalOutput").ap()

    with tile.TileContext(nc) as tc:
        with (
            tc.tile_pool(name="singles", bufs=1) as singles,
            tc.tile_pool(name="ypool", bufs=2) as ypool,
            tc.tile_pool(name="workp", bufs=2) as workp,
            tc.tile_pool(name="pzp", bufs=1, space=MemorySpace.PSUM) as pzp,
            tc.tile_pool(name="pyp", bufs=2, space=MemorySpace.PSUM) as pyp,
            tc.tile_pool(name="php", bufs=1, space=MemorySpace.PSUM) as php,
            tc.tile_pool(name="pgp", bufs=1, space=MemorySpace.PSUM) as pgp,
        ):
            # ------- preamble: batched small loads (one DMA each) -------
            relT_sb = singles.tile([D, R], f32, tag="relT", name="relT_sb")
            nc.scalar.dma_start(out=relT_sb[:, :], in_=relT)
            wbr_all = singles.tile([D, L * RNB], f32, tag="wbr", name="wbr_all")
            nc.scalar.dma_start(
                out=wbr_all[:, :].rearrange("d (l q) -> d l q", l=L),
                in_=wbrD.rearrange("l d q -> d l q")
            )
            # rows 0:100 = Wbx[i] at cols 30i; row 100 = c[i] (written below)
            wbxa = singles.tile([D + 1, L * RNB], f32, tag="wbxa", name="wbxa")
            nc.scalar.dma_start(
                out=wbxa[0:D, :].rearrange("d (l q) -> d l q", l=L),
                in_=wbxD.rearrange("l d q -> d l q")
            )
            xTa = singles.tile([D + 1, EP], f32, tag="xTa", name="xTa")
            nc.vector.memset(xTa[96 : D + 1, :], 1.0)
            nc.vector.memset(xTa[0:D, E:EP], 0.0)
            nc.scalar.dma_start(out=xTa[0:D, 0:E], in_=xT0)

            # ------- adj granule DMAs (the long pole; SP queue) -------
            res_tiles = []
            for r in range(R):
                rt = singles.tile([128, C2 * 2 * E2], fp8, tag=f"res{r}",
                                  name=f"res{r}")
                res_tiles.append(rt)
            for (r, c) in ADJ_ORDER:
                nc.sync.dma_start(
                    out=res_tiles[r][:, c * 2 * E2 : (c + 1) * 2 * E2],
                    in_=adjT[r, c].rearrange("p t i -> p (t i)"),
                )
            res_views = [
                res_tiles[r][:, :].rearrange("p (c t i) -> p c t i", c=C2, t=2)
                for r in range(R)
            ]

            # c[i, (r,q)] = rel_r @ Wbr[i, r] -> row 100 of wbxa (via ACT-queue
            # SBUF->SBUF DMA: ACT copy can't write partition base 100, and
            # the SP queue must stay free for the adj granule stream)
            psc = pyp.tile([128, 360], f32, tag="py", name="psc")
            for i in range(L):
                for r in range(R):
                    q = RNB * i + 3 * r
                    nc.tensor.matmul(
                        psc[0:1, q : q + 3],
                        relT_sb[:, r : r + 1],
                        wbr_all[:, q : q + 3],
                        start=True, stop=True,
                    )
            c_sb = workp.tile([1, L * RNB], f32, tag="c_sb", name="c_sb")
            nc.scalar.copy(out=c_sb[:, :], in_=psc[0:1, 0 : L * RNB])
            nc.scalar.dma_start(out=wbxa[D : D + 1, :], in_=c_sb[:, :])

            # preload the Sigmoid ACT table (else a 1.3us table load lands in
            # the layer-0 tail) and precompute the masked-mean denominator
            # from a [1, E2] mask row, broadcast to 100 partitions via PE --
            # all during the adj DMA window, off every critical chain
            scr = workp.tile([1, 1], f32, tag="scr", name="scr", bufs=1)
            nc.scalar.activation(scr[:, :], xTa[96:97, 0:1], AF.Sigmoid)
            # mask row lands in mask_sb row 0; the full maskrep DMA (much
            # later) overwrites it after the reduce has consumed it
            mask_sb = singles.tile([HID, E2], f32, tag="mask", name="mask_sb")
            nc.scalar.dma_start(out=mask_sb[0:1, :], in_=mask1)
            den1 = workp.tile([1, 1], f32, tag="den1", name="den1", bufs=1)
            nc.vector.reduce_sum(den1[:, :], mask_sb[0:1, :], axis=AX.X)
            nc.vector.tensor_scalar_max(den1[:, :], den1[:, :], 1.0)
            nc.vector.reciprocal(den1[:, :], den1[:, :])
            ones_h = workp.tile([1, HID], f32, tag="ones_h", name="ones_h", bufs=1)
            nc.vector.memset(ones_h[:, :], 1.0)
            nc.tensor.matmul(psc[0:HID, 200:201], ones_h[:, :], den1[:, :],
                             start=True, stop=True)
            den = workp.tile([HID, 1], f32, tag="den", name="den", bufs=1)
            nc.scalar.copy(out=den[:, :], in_=psc[0:HID, 200:201])

            # tail weights: issued after the granules (needed only once
            # layer-0's Z is complete, i.e. right at DMA end)
            ww_all = singles.tile([NB, L * HID], f32r, tag="ww", name="ww_all")
            nc.sync.dma_start(out=ww_all[:, :].rearrange("n (l h) -> n l h", l=L),
                              in_=wwD.rearrange("l n h -> n l h"))
            wh_all = singles.tile([HID, L * HID], f32r, tag="wh", name="wh_all")
            nc.sync.dma_start(out=wh_all[:, :].rearrange("p (l h) -> p l h", l=L),
                              in_=whD.rearrange("l p h -> p l h"))
            bias_all = singles.tile([HID, L], f32, tag="bias", name="bias_all")
            nc.sync.dma_start(out=bias_all[:, :], in_=biasD.rearrange("l h -> h l"))
            bh_all = singles.tile([HID, L], f32, tag="bh", name="bh_all")
            nc.sync.dma_start(out=bh_all[:, :], in_=bhD.rearrange("l h -> h l"))
            nc.sync.dma_start(out=mask_sb[:, :], in_=maskrep)

            h_sb = singles.tile([HID, E2], f32r, tag="h", name="h_sb")

            # ------- per-layer emission helpers -------
            ydict = {}    # i -> (psy tile, y_all tile, y_view)
            pzt = {}      # (i, n) -> Z psum tile
            zcnt = {}     # (i, n) -> accumulation counter
            zsb = {}      # (i, n) -> z sbuf tile
            phd = {}      # (i, n) -> psh tile
            pgd = {}      # (i, n) -> psg tile

            def emit_y(i, ks):
                if i not in ydict:
                    psy = pyp.tile([128, 360], f32, tag="py", name=f"py{i}")
                    y_all = ypool.tile([128, CH * YQ], fp8, tag="y_all",
                                       name=f"y_all{i}")
                    yv = y_all[:, :].rearrange("p (k q) -> p k q", q=YQ)
                    ydict[i] = (psy, y_all, yv)
                psy, y_all, _ = ydict[i]
                for k in ks:
                    nc.tensor.matmul(
                        psy[:, 30 * k : 30 * k + 30],
                        xTa[:, 128 * k : 128 * (k + 1)],
                        wbxa[:, RNB * i : RNB * (i + 1)],
                        start=True, stop=True,
                    )
                k0, nk = ks[0], len(ks)
                nc.scalar.copy(
                    out=y_all[:, YQ * k0 : YQ * (k0 + nk)].rearrange(
                        "p (k q) -> p k q", q=YQ)[:, :, 0:RNB],
                    in_=psy[:, 30 * k0 : 30 * (k0 + nk)].rearrange(
                        "p (k q) -> p k q", q=RNB),
                )

            def bigmm(i, n, cs):
                if (i, n) not in pzt:
                    pzt[(i, n)] = pzp.tile([NB, 512], f32, tag=f"pz{n}",
                                           name=f"pz{i}_{n}")
                lo, hi = NS[n]
                nw = hi - lo
                out = pzt[(i, n)][:, 0:nw]
                yv = ydict[i][2]
                for c in cs:
                    for r in range(R):
                        cnt = zcnt.get((i, n), 0)
                        nc.tensor.matmul(
                            out,
                            yv[:, 2 * c : 2 * c + 2, 3 * r : 3 * r + 3],
                            res_views[r][:, c, :, lo:hi],
                            start=(cnt == 0),
                            stop=(cnt == R * C2 - 1),
                            perf_mode=DR,
                        )
                        zcnt[(i, n)] = cnt + 1

            def emit_zcopy(i, n):
                lo, hi = NS[n]
                nw = hi - lo
                zt = workp.tile([NB, 512], f32r, tag="z", name=f"z{i}_{n}", bufs=1)
                zsb[(i, n)] = zt
                nc.scalar.copy(out=zt[:, 0:nw], in_=pzt[(i, n)][:, 0:nw])

            def emit_psh(i, n):
                lo, hi = NS[n]
                nw = hi - lo
                ph = php.tile([HID, 512], f32, tag="ph", name=f"ph{i}_{n}")
                phd[(i, n)] = ph
                nc.tensor.matmul(
                    ph[:, 0:nw],
                    ww_all[:, HID * i : HID * (i + 1)],
                    zsb[(i, n)][:, 0:nw],
                    start=True, stop=True,
                )

            def emit_relu(i, n):
                lo, hi = NS[n]
                nc.scalar.activation(
                    h_sb[:, lo:hi], phd[(i, n)][:, 0 : hi - lo], AF.Relu,
                    bias=bias_all[:, i : i + 1],
                )

            def emit_psg(i, n):
                lo, hi = NS[n]
                nw = hi - lo
                pg = pgp.tile([HID, 512], f32, tag="pg", name=f"pg{i}_{n}")
                pgd[(i, n)] = pg
                nc.tensor.matmul(
                    pg[:, 0:nw],
                    wh_all[:, HID * i : HID * (i + 1)],
                    h_sb[:, lo:hi],
                    start=True, stop=True,
                )

            def emit_sig(i, n):
                lo, hi = NS[n]
                pg = pgd[(i, n)]
                nc.scalar.activation(
                    pg[:, 0 : hi - lo], pg[:, 0 : hi - lo], AF.Sigmoid,
                    bias=bh_all[:, i : i + 1],
                )

            def emit_xupd(i, n):
                lo, hi = NS[n]
                pg = pgd[(i, n)]
                for a in range(lo, hi, 256):
                    b = min(a + 256, hi)
                    nc.vector.tensor_sub(
                        h_sb[:, a:b], h_sb[:, a:b], xTa[0:D, a:b])
                    nc.vector.tensor_mul(
                        h_sb[:, a:b], h_sb[:, a:b], pg[:, a - lo : b - lo])
                    nc.vector.tensor_add(
                        xTa[0:D, a:b], xTa[0:D, a:b], h_sb[:, a:b])

            ga = [workp.tile([HID, 1], f32, tag=f"ga{k}", name=f"ga{k}",
                             bufs=1) for k in range(4)]
            xmp = workp.tile([HID, 512], f32, tag="xmp", name="xmp", bufs=1)

            def emit_mask_part(n):
                l2, h2 = NS[n]
                nc.vector.scalar_tensor_tensor(
                    out=xmp[:, 0 : h2 - l2], in0=xTa[0:D, l2:h2], scalar=1.0,
                    in1=mask_sb[:, l2:h2], op0=ALU.mult, op1=ALU.mult,
                    accum_out=ga[n][:, :])

            # ------- layer 0 (DMA-arrival order, granule-major) -------
            emit_y(0, [0, 1, 2, 3])
            emit_y(0, [4, 5, 6, 7])
            emit_y(0, [8, 9, 10, 11])
            for n in range(3):
                pzt[(0, n)] = pzp.tile([NB, 512], f32, tag=f"pz{n}", name=f"pz0_{n}")
            yv0 = ydict[0][2]
            for g, (r, c) in enumerate(ADJ_ORDER):
                for n in range(3):
                    lo, hi = NS[n]
                    nc.tensor.matmul(
                        pzt[(0, n)][:, 0 : hi - lo],
                        yv0[:, 2 * c : 2 * c + 2, 3 * r : 3 * r + 3],
                        res_views[r][:, c, :, lo:hi],
                        start=(g == 0),
                        stop=(g == len(ADJ_ORDER) - 1),
                        perf_mode=DR,
                    )
            emit_zcopy(0, 0)
            emit_psh(0, 0)
            emit_relu(0, 0)
            emit_zcopy(0, 1)
            emit_psh(0, 1)
            emit_relu(0, 1)
            emit_psg(0, 0)
            emit_sig(0, 0)
            emit_xupd(0, 0)
            emit_y(1, [0, 1, 2, 3])
            emit_zcopy(0, 2)
            emit_psh(0, 2)
            emit_relu(0, 2)
            emit_psg(0, 1)
            emit_sig(0, 1)
            emit_xupd(0, 1)
            emit_y(1, [4, 5, 6, 7])
            emit_psg(0, 2)
            emit_sig(0, 2)
            emit_xupd(0, 2)
            bigmm(1, 0, [0, 1])
            bigmm(1, 0, [2, 3])

            # ------- layers 1..4 (software-pipelined) -------
            # entry state per layer i: bigmm(i,0,[0,1]) and y(i) k0-7 already
            # emitted by the predecessor; y(i) k8-11 still pending.
            # invariant entering layer i: bigmm(i,0,[0..3]) and y(i) k0-7
            # already emitted; y(i) k8-11 pending. Each psh/relu pair issues
            # a full bigmm group before its psg consumer so the serial ACT
            # queue (zcopy/relu/sig/y-copy, ~0.6us each) never blocks PE.
            for i in range(1, L):
                emit_y(i, [8, 9, 10, 11])
                bigmm(i, 0, [4, 5])
                emit_zcopy(i, 0)
                bigmm(i, 1, [0, 1])
                emit_psh(i, 0)
                emit_relu(i, 0)
                bigmm(i, 1, [2, 3, 4, 5])
                emit_psg(i, 0)
                emit_sig(i, 0)
                emit_zcopy(i, 1)
                emit_xupd(i, 0)
                if i == L - 1:
                    emit_mask_part(0)
                bigmm(i, 2, [0, 1])
                emit_psh(i, 1)
                emit_relu(i, 1)
                if i < L - 1:
                    emit_y(i + 1, [0, 1, 2, 3])
                bigmm(i, 2, [2, 3, 4, 5])
                emit_psg(i, 1)
                emit_sig(i, 1)
                emit_zcopy(i, 2)
                emit_xupd(i, 1)
                if i == L - 1:
                    emit_mask_part(1)
                if i < L - 1:
                    bigmm(i + 1, 0, [0, 1])
                    emit_psh(i, 2)
                    emit_relu(i, 2)
                    emit_y(i + 1, [4, 5, 6, 7])
                    bigmm(i + 1, 0, [2, 3])
                    emit_psg(i, 2)
                    emit_sig(i, 2)
                    emit_xupd(i, 2)

            # ------- layer-4 final chunk (fine-grained halves) + epilogue ---
            # Pool computes the n=0/1 mask partials in parallel with the
            # final chunk's tail chain; the last chunk runs in two 240-col
            # halves so PE/ACT/DVE pipeline with minimal exposed latency.
            lo, hi = NS[2]
            ph4 = php.tile([HID, 512], f32, tag="ph", name="ph4_2")
            pg4 = pgp.tile([HID, 512], f32, tag="pg", name="pg4_2")
            zt4 = zsb[(L - 1, 2)]
            xm = workp.tile([HID, 240], f32, tag="xm", name="xm", bufs=1)
            i = L - 1
            nc.tensor.matmul(
                ph4[:, 0 : hi - lo], ww_all[:, HID * i : HID * (i + 1)],
                zt4[:, 0 : hi - lo], start=True, stop=True)
            nc.scalar.activation(
                h_sb[:, lo:hi], ph4[:, 0 : hi - lo], AF.Relu,
                bias=bias_all[:, i : i + 1])
            nc.tensor.matmul(
                pg4[:, 0 : hi - lo], wh_all[:, HID * i : HID * (i + 1)],
                h_sb[:, lo:hi], start=True, stop=True)
            nc.scalar.activation(
                pg4[:, 0 : hi - lo], pg4[:, 0 : hi - lo], AF.Sigmoid,
                bias=bh_all[:, i : i + 1])
            for hvi, (a, b) in enumerate([(lo, lo + 240), (lo + 240, hi)]):
                al, bl = a - lo, b - lo
                nc.vector.tensor_sub(h_sb[:, a:b], h_sb[:, a:b], xTa[0:D, a:b])
                nc.vector.tensor_mul(h_sb[:, a:b], h_sb[:, a:b], pg4[:, al:bl])
                nc.vector.tensor_add(xTa[0:D, a:b], xTa[0:D, a:b], h_sb[:, a:b])
                nc.vector.scalar_tensor_tensor(
                    out=xm[:, 0 : b - a], in0=xTa[0:D, a:b], scalar=1.0,
                    in1=mask_sb[:, a:b], op0=ALU.mult, op1=ALU.mult,
                    accum_out=ga[2 + hvi][:, :])
            nc.vector.tensor_add(ga[0][:, :], ga[0][:, :], ga[1][:, :])
            nc.vector.tensor_add(ga[2][:, :], ga[2][:, :], ga[3][:, :])
            nc.vector.tensor_add(ga[0][:, :], ga[0][:, :], ga[2][:, :])
            nc.vector.tensor_mul(ga[0][:, :], ga[0][:, :], den[:, :])
            nc.sync.dma_start(out=graphD, in_=ga[0][:, :])

    nc.compile()
    return nc


def get_nc():
    if "nc" not in _NC_CACHE:
        _NC_CACHE["nc"] = _build_nc()
    return _NC_CACHE["nc"]


def make_in_maps(adj, mask_ids, ent_emb, rel_emb, Wb, Ww, bias, Wh, bh):
    adj = np.asarray(adj, dtype=np.float32)
    pad = np.zeros((B, R, EP, E2), dtype=FP8_NP)
    pad[:, :, :E, :E] = adj.transpose(0, 1, 3, 2).astype(FP8_NP)
    # [b, r, c, p, t, i] = adj[b, r, i, j = c*256 + t*128 + p]
    adjT = np.ascontiguousarray(
        pad.reshape(B, R, C2, 2, 128, E2).transpose(0, 1, 2, 4, 3, 5)
    )
    entT = np.ascontiguousarray(np.asarray(ent_emb, np.float32).T)
    relTh = np.ascontiguousarray(np.asarray(rel_emb, np.float32).T)
    Wb5 = np.asarray(Wb, np.float32).reshape(L, R, 2, D, NB)
    wbx = np.ascontiguousarray(Wb5[:, :, 0].transpose(0, 2, 1, 3).reshape(L, D, RNB))
    wbr = np.ascontiguousarray(Wb5[:, :, 1].transpose(0, 2, 1, 3).reshape(L, D, RNB))
    maskf = np.asarray(mask_ids).astype(np.float32)
    common = dict(
        xT0=entT, relT=relTh, wbx=wbx, wbr=wbr,
        ww=np.ascontiguousarray(np.asarray(Ww, np.float32)),
        wh=np.ascontiguousarray(np.asarray(Wh, np.float32)),
        biasL=np.ascontiguousarray(np.asarray(bias, np.float32)),
        bhL=np.ascontiguousarray(np.asarray(bh, np.float32)),
    )
    in_maps = []
    for c in range(8):
        b = c // 2
        m = dict(common)
        m["adjT"] = np.ascontiguousarray(adjT[b])
        mrep = np.zeros((HID, E2), np.float32)
        mrep[:, :E] = np.broadcast_to(maskf[b][None, :], (HID, E))
        m["maskrep"] = mrep
        m1 = np.zeros((1, E2), np.float32)
        m1[0, :E] = maskf[b]
        m["mask1"] = m1
        in_maps.append(m)
    return in_maps


def run(inputs, trace=False):
    nc = get_nc()
    in_maps = make_in_maps(**{k: np.asarray(v) for k, v in inputs.items()})
    res = bass_utils.run_bass_kernel_spmd(
        nc, in_maps, core_ids=list(range(8)), trace=trace
    )
    out = np.stack(
        [np.asarray(res.results[2 * b]["graph"]).reshape(HID) for b in range(B)]
    ).astype(np.float32)
    return out, res


def kernel(**inputs):
    out, _ = run(inputs, trace=False)
    return out


# revision 18
# speedup vs baseline: 1.4303x; 1.0100x over previous
"""Trainium2 Bass kernel for the KGEncoder RGCN (nn_KGEncoder_14027363188782).

Math (per batch element b, L=5 layers):
    x0 = ent_emb                                             (E, D)
    per layer i:
      y_r   = x @ Wb_x[i,r] + 1 * c[i,r]^T    (E, NB)  where c[i,r] = rel_r @ Wb_rel[i,r]
      Z     = sum_r adj_r @ y_r               (E, NB)
      h     = relu(Z @ Ww[i] + bias[i])
      g     = sigmoid(h @ Wh[i] + bh[i])
      x     = x + g * (h - x)
    out_b = sum_e x[e] * m[e] / max(sum_e m[e], 1)

Sharding: core c handles b = c // 2 (pair-replicated, no collectives).
adj shipped pre-transposed, fp8 (exact for 0/1), DoubleRow layout
[r, c, p, t, i] with j = c*256 + t*128 + p.

Schedule: granular (r,c) adj DMAs; layer 0 accumulates in DMA-arrival
order (granule-major over the 3 psum i-chunks) so it finishes right at
DMA end; layers 1-4 run a software-pipelined schedule that keeps PE
busy continuously: bigmm i-chunks back-to-back, the highway tail and
the next layer's y matmuls interleaved into the bigmm instruction
stream, and the next layer's first bigmm group filling the last
tail's ACT->PE ping-pong. Tail matmuls use f32r moving operands
(1 cyc/row vs 4 for f32). The per-layer y uses an augmented
contraction row (x row 100 = 1, W row 100 = c[i]) so the relation
bias needs no extra matmul.
"""

import numpy as np
import ml_dtypes

import concourse.bacc as bacc
import concourse.bass as bass
import concourse.mybir as mybir
import concourse.tile as tile
from concourse import bass_utils
from concourse.bass import MemorySpace

B, R, E, D, HID, L, NB = 4, 10, 1500, 100, 100, 5, 3
EP = 1536           # j dim padded to 12*128
C2 = 6              # 256-row contraction chunks (DoubleRow)
E2 = 1504           # i dim padded to 16-aligned
YQ = 32             # y_all per-chunk col stride
CH = 12             # y chunks (128 j's each)
RNB = R * NB        # 30
NS = [(0, 512), (512, 1024), (1024, E2)]   # i-dim psum chunks
f32 = mybir.dt.float32
f32r = mybir.dt.float32r
fp8 = mybir.dt.float8e4
FP8_NP = ml_dtypes.float8_e4m3fn
AF = mybir.ActivationFunctionType
AX = mybir.AxisListType
ALU = mybir.AluOpType
DR = mybir.MatmulPerfMode.DoubleRow

ADJ_ORDER = [(r, c) for r in range(R) for c in range(C2)]
# y chunk k -> x column range; k=10/11 are the 110-wide halves of j 1280:1500
YCOLS = [(128 * k, 128 * (k + 1)) for k in range(10)] + [(1280, 1390), (1390, 1500)]


def _copy_groups(ks):
    # contiguous runs with uniform partition count (128 for k<10, 110 after)
    out = []
    run = [ks[0]]
    for k in ks[1:]:
        if k == run[-1] + 1 and (k < 10) == (run[0] < 10):
            run.append(k)
        else:
            out.append(run)
            run = [k]
    out.append(run)
    return [(r[0], len(r), 110 if r[0] >= 10 else 128) for r in out]

_NC_CACHE = {}


def _build_nc():
    nc = bacc.Bacc("TRN2", target_bir_lowering=False, debug=False)

    adjT = nc.dram_tensor("adjT", [R, C2, 128, 2, E2], fp8, kind="ExternalInput").ap()
    xT0 = nc.dram_tensor("xT0", [D, E], f32, kind="ExternalInput").ap()
    maskrep = nc.dram_tensor("maskrep", [HID, E2], f32, kind="ExternalInput").ap()
    mask1 = nc.dram_tensor("mask1", [1, E2], f32, kind="ExternalInput").ap()
    relT = nc.dram_tensor("relT", [D, R], f32, kind="ExternalInput").ap()
    wbxD = nc.dram_tensor("wbx", [L, D, RNB], f32, kind="ExternalInput").ap()
    wbrD = nc.dram_tensor("wbr", [L, D, RNB], f32, kind="ExternalInput").ap()
    wwD = nc.dram_tensor("ww", [L, NB, HID], f32r, kind="ExternalInput").ap()
    whD = nc.dram_tensor("wh", [L, HID, HID], f32r, kind="ExternalInput").ap()
    biasD = nc.dram_tensor("biasL", [L, HID], f32, kind="ExternalInput").ap()
    bhD = nc.dram_tensor("bhL", [L, HID], f32, kind="ExternalInput").ap()
    graphD = nc.dram_tensor("graph", [HID, 1], f32, kind="ExternalOutput").ap()

    with tile.TileContext(nc) as tc:
        with (
            tc.tile_pool(name="singles", bufs=1) as singles,
            tc.tile_pool(name="ypool", bufs=2) as ypool,
            tc.tile_pool(name="workp", bufs=2) as workp,
            tc.tile_pool(name="pzp", bufs=1, space=MemorySpace.PSUM) as pzp,
            tc.tile_pool(name="pyp", bufs=2, space=MemorySpace.PSUM) as pyp,
            tc.tile_pool(name="php", bufs=1, space=MemorySpace.PSUM) as php,
            tc.tile_pool(name="pgp", bufs=1, space=MemorySpace.PSUM) as pgp,
        ):
            # ------- preamble: batched small loads (one DMA each) -------
            relT_sb = singles.tile([D, R], f32, tag="relT", name="relT_sb")
            nc.scalar.dma_start(out=relT_sb[:, :], in_=relT)
            wbr_all = singles.tile([D, L * RNB], f32, tag="wbr", name="wbr_all")
            nc.scalar.dma_start(
                out=wbr_all[:, :].rearrange("d (l q) -> d l q", l=L),
                in_=wbrD.rearrange("l d q -> d l q")
            )
            # rows 0:100 = Wbx[i] at cols 30i; row 100 = c[i] (written below)
            wbxa = singles.tile([D + 1, L * RNB], f32, tag="wbxa", name="wbxa")
            nc.scalar.dma_start(
                out=wbxa[0:D, :].rearrange("d (l q) -> d l q", l=L),
                in_=wbxD.rearrange("l d q -> d l q")
            )
            xTa = singles.tile([D + 1, EP], f32, tag="xTa", name="xTa")
            nc.vector.memset(xTa[96 : D + 1, :], 1.0)
            nc.vector.memset(xTa[0:D, E:EP], 0.0)
            nc.scalar.dma_start(out=xTa[0:D, 0:E], in_=xT0)

            # ------- adj granule DMAs (the long pole; SP queue) -------
            res_tiles = []
            for r in range(R):
                rt = singles.tile([128, C2 * 2 * E2], fp8, tag=f"res{r}",
                                  name=f"res{r}")
                res_tiles.append(rt)
            # chunk c=5 covers j 1280..1499 repacked as 110 DoubleRow pairs
            # (j = 1280 + 110*t + p), so only 110 partitions ship
            for (r, c) in ADJ_ORDER:
                pp = 110 if c == C2 - 1 else 128
                nc.sync.dma_start(
                    out=res_tiles[r][0:pp, c * 2 * E2 : (c + 1) * 2 * E2],
                    in_=adjT[r, c, 0:pp].rearrange("p t i -> p (t i)"),
                )
            res_views = [
                res_tiles[r][:, :].rearrange("p (c t i) -> p c t i", c=C2, t=2)
                for r in range(R)
            ]

            # c[i, (r,q)] = rel_r @ Wbr[i, r] -> row 100 of wbxa (via ACT-queue
            # SBUF->SBUF DMA: ACT copy can't write partition base 100, and
            # the SP queue must stay free for the adj granule stream)
            psc = pyp.tile([128, 360], f32, tag="py", name="psc")
            for i in range(L):
                for r in range(R):
                    q = RNB * i + 3 * r
                    nc.tensor.matmul(
                        psc[0:1, q : q + 3],
                        relT_sb[:, r : r + 1],
                        wbr_all[:, q : q + 3],
                        start=True, stop=True,
                    )
            c_sb = workp.tile([1, L * RNB], f32, tag="c_sb", name="c_sb")
            nc.scalar.copy(out=c_sb[:, :], in_=psc[0:1, 0 : L * RNB])
            nc.scalar.dma_start(out=wbxa[D : D + 1, :], in_=c_sb[:, :])

            # preload the Sigmoid ACT table (else a 1.3us table load lands in
            # the layer-0 tail) and precompute the masked-mean denominator
            # from a [1, E2] mask row, broadcast to 100 partitions via PE --
            # all during the adj DMA window, off every critical chain
            scr = workp.tile([1, 1], f32, tag="scr", name="scr", bufs=1)
            nc.scalar.activation(scr[:, :], xTa[96:97, 0:1], AF.Sigmoid)
            # mask row lands in mask_sb row 0; the full maskrep DMA (much
            # later) overwrites it after the reduce has consumed it
            mask_sb = singles.tile([HID, E2], f32, tag="mask", name="mask_sb")
            nc.scalar.dma_start(out=mask_sb[0:1, :], in_=mask1)
            den1 = workp.tile([1, 1], f32, tag="den1", name="den1", bufs=1)
            nc.vector.reduce_sum(den1[:, :], mask_sb[0:1, :], axis=AX.X)
            nc.vector.tensor_scalar_max(den1[:, :], den1[:, :], 1.0)
            nc.vector.reciprocal(den1[:, :], den1[:, :])
            ones_h = workp.tile([1, HID], f32, tag="ones_h", name="ones_h", bufs=1)
            nc.vector.memset(ones_h[:, :], 1.0)
            nc.tensor.matmul(psc[0:HID, 200:201], ones_h[:, :], den1[:, :],
                             start=True, stop=True)
            den = workp.tile([HID, 1], f32, tag="den", name="den", bufs=1)
            nc.scalar.copy(out=den[:, :], in_=psc[0:HID, 200:201])

            # tail weights: issued after the granules (needed only once
            # layer-0's Z is complete, i.e. right at DMA end)
            ww_all = singles.tile([NB, L * HID], f32r, tag="ww", name="ww_all")
            nc.sync.dma_start(out=ww_all[:, :].rearrange("n (l h) -> n l h", l=L),
                              in_=wwD.rearrange("l n h -> n l h"))
            wh_all = singles.tile([HID, L * HID], f32r, tag="wh", name="wh_all")
            nc.sync.dma_start(out=wh_all[:, :].rearrange("p (l h) -> p l h", l=L),
                              in_=whD.rearrange("l p h -> p l h"))
            bias_all = singles.tile([HID, L], f32, tag="bias", name="bias_all")
            nc.sync.dma_start(out=bias_all[:, :], in_=biasD.rearrange("l h -> h l"))
            bh_all = singles.tile([HID, L], f32, tag="bh", name="bh_all")
            nc.sync.dma_start(out=bh_all[:, :], in_=bhD.rearrange("l h -> h l"))
            nc.sync.dma_start(out=mask_sb[:, :], in_=maskrep)

            h_sb = singles.tile([HID, E2], f32r, tag="h", name="h_sb")

            # ------- per-layer emission helpers -------
            ydict = {}    # i -> (psy tile, y_all tile, y_view)
            pzt = {}      # (i, n) -> Z psum tile
            zcnt = {}     # (i, n) -> accumulation counter
            zsb = {}      # (i, n) -> z sbuf tile
            phd = {}      # (i, n) -> psh tile
            pgd = {}      # (i, n) -> psg tile

            def emit_y(i, ks):
                if i not in ydict:
                    psy = pyp.tile([128, 360], f32, tag="py", name=f"py{i}")
                    y_all = ypool.tile([128, CH * YQ], fp8, tag="y_all",
                                       name=f"y_all{i}")
                    yv = y_all[:, :].rearrange("p (k q) -> p k q", q=YQ)
                    ydict[i] = (psy, y_all, yv)
                psy, y_all, _ = ydict[i]
                for k in ks:
                    a, b = YCOLS[k]
                    nc.tensor.matmul(
                        psy[0 : b - a, 30 * k : 30 * k + 30],
                        xTa[:, a:b],
                        wbxa[:, RNB * i : RNB * (i + 1)],
                        start=True, stop=True,
                    )
                for k0, nk, pp in _copy_groups(ks):
                    nc.scalar.copy(
                        out=y_all[0:pp, YQ * k0 : YQ * (k0 + nk)].rearrange(
                            "p (k q) -> p k q", q=YQ)[:, :, 0:RNB],
                        in_=psy[0:pp, 30 * k0 : 30 * (k0 + nk)].rearrange(
                            "p (k q) -> p k q", q=RNB),
                    )

            def bigmm(i, n, cs):
                if (i, n) not in pzt:
                    pzt[(i, n)] = pzp.tile([NB, 512], f32, tag=f"pz{n}",
                                           name=f"pz{i}_{n}")
                lo, hi = NS[n]
                nw = hi - lo
                out = pzt[(i, n)][:, 0:nw]
                yv = ydict[i][2]
                for c in cs:
                    pp = 110 if c == C2 - 1 else 128
                    for r in range(R):
                        cnt = zcnt.get((i, n), 0)
                        nc.tensor.matmul(
                            out,
                            yv[0:pp, 2 * c : 2 * c + 2, 3 * r : 3 * r + 3],
                            res_views[r][0:pp, c, :, lo:hi],
                            start=(cnt == 0),
                            stop=(cnt == R * C2 - 1),
                            perf_mode=DR,
                        )
                        zcnt[(i, n)] = cnt + 1

            def emit_zcopy(i, n):
                lo, hi = NS[n]
                nw = hi - lo
                zt = workp.tile([NB, 512], f32r, tag="z", name=f"z{i}_{n}", bufs=1)
                zsb[(i, n)] = zt
                nc.scalar.copy(out=zt[:, 0:nw], in_=pzt[(i, n)][:, 0:nw])

            def emit_psh(i, n):
                lo, hi = NS[n]
                nw = hi - lo
                ph = php.tile([HID, 512], f32, tag="ph", name=f"ph{i}_{n}")
                phd[(i, n)] = ph
                nc.tensor.matmul(
                    ph[:, 0:nw],
                    ww_all[:, HID * i : HID * (i + 1)],
                    zsb[(i, n)][:, 0:nw],
                    start=True, stop=True,
                )

            def emit_relu(i, n):
                lo, hi = NS[n]
                nc.scalar.activation(
                    h_sb[:, lo:hi], phd[(i, n)][:, 0 : hi - lo], AF.Relu,
                    bias=bias_all[:, i : i + 1],
                )

            def emit_psg(i, n):
                lo, hi = NS[n]
                nw = hi - lo
                pg = pgp.tile([HID, 512], f32, tag="pg", name=f"pg{i}_{n}")
                pgd[(i, n)] = pg
                nc.tensor.matmul(
                    pg[:, 0:nw],
                    wh_all[:, HID * i : HID * (i + 1)],
                    h_sb[:, lo:hi],
                    start=True, stop=True,
                )

            def emit_sig(i, n):
                lo, hi = NS[n]
                pg = pgd[(i, n)]
                nc.scalar.activation(
                    pg[:, 0 : hi - lo], pg[:, 0 : hi - lo], AF.Sigmoid,
                    bias=bh_all[:, i : i + 1],
                )

            def emit_xupd(i, n):
                lo, hi = NS[n]
                pg = pgd[(i, n)]
                for a in range(lo, hi, 256):
                    b = min(a + 256, hi)
                    nc.vector.tensor_sub(
                        h_sb[:, a:b], h_sb[:, a:b], xTa[0:D, a:b])
                    nc.vector.tensor_mul(
                        h_sb[:, a:b], h_sb[:, a:b], pg[:, a - lo : b - lo])
                    nc.vector.tensor_add(
                        xTa[0:D, a:b], xTa[0:D, a:b], h_sb[:, a:b])

            ga = [workp.tile([HID, 1], f32, tag=f"ga{k}", name=f"ga{k}",
                             bufs=1) for k in range(4)]
            xmp = workp.tile([HID, 512], f32, tag="xmp", name="xmp", bufs=1)

            def emit_mask_part(n):
                l2, h2 = NS[n]
                nc.vector.scalar_tensor_tensor(
                    out=xmp[:, 0 : h2 - l2], in0=xTa[0:D, l2:h2], scalar=1.0,
                    in1=mask_sb[:, l2:h2], op0=ALU.mult, op1=ALU.mult,
                    accum_out=ga[n][:, :])

            # ------- layer 0 (DMA-arrival order, granule-major) -------
            emit_y(0, [0, 1, 2, 3])
            emit_y(0, [4, 5, 6, 7])
            emit_y(0, [8, 9, 10, 11])
            for n in range(3):
                pzt[(0, n)] = pzp.tile([NB, 512], f32, tag=f"pz{n}", name=f"pz0_{n}")
            yv0 = ydict[0][2]
            for g, (r, c) in enumerate(ADJ_ORDER):
                pp = 110 if c == C2 - 1 else 128
                for n in range(3):
                    lo, hi = NS[n]
                    nc.tensor.matmul(
                        pzt[(0, n)][:, 0 : hi - lo],
                        yv0[0:pp, 2 * c : 2 * c + 2, 3 * r : 3 * r + 3],
                        res_views[r][0:pp, c, :, lo:hi],
                        start=(g == 0),
                        stop=(g == len(ADJ_ORDER) - 1),
                        perf_mode=DR,
                    )
            emit_zcopy(0, 0)
            emit_psh(0, 0)
            emit_relu(0, 0)
            emit_zcopy(0, 1)
            emit_psh(0, 1)
            emit_relu(0, 1)
            emit_psg(0, 0)
            emit_sig(0, 0)
            emit_xupd(0, 0)
            emit_y(1, [0, 1, 2, 3])
            emit_zcopy(0, 2)
            emit_psh(0, 2)
            emit_relu(0, 2)
            emit_psg(0, 1)
            emit_sig(0, 1)
            emit_xupd(0, 1)
            emit_y(1, [4, 5, 6, 7])
            emit_psg(0, 2)
            emit_sig(0, 2)
            emit_xupd(0, 2)
            bigmm(1, 0, [0, 1])
            bigmm(1, 0, [2, 3])

            # ------- layers 1..4 (software-pipelined) -------
            # entry state per layer i: bigmm(i,0,[0,1]) and y(i) k0-7 already
            # emitted by the predecessor; y(i) k8-11 still pending.
            # invariant entering layer i: bigmm(i,0,[0..3]) and y(i) k0-7
            # already emitted; y(i) k8-11 pending. Each psh/relu pair issues
            # a full bigmm group before its psg consumer so the serial ACT
            # queue (zcopy/relu/sig/y-copy, ~0.6us each) never blocks PE.
            for i in range(1, L):
                emit_y(i, [8, 9, 10, 11])
                bigmm(i, 0, [4, 5])
                emit_zcopy(i, 0)
                bigmm(i, 1, [0, 1])
                emit_psh(i, 0)
                emit_relu(i, 0)
                bigmm(i, 1, [2, 3, 4, 5])
                emit_psg(i, 0)
                emit_sig(i, 0)
                emit_zcopy(i, 1)
                emit_xupd(i, 0)
                if i == L - 1:
                    emit_mask_part(0)
                bigmm(i, 2, [0, 1])
                emit_psh(i, 1)
                emit_relu(i, 1)
                if i < L - 1:
                    emit_y(i + 1, [0, 1, 2, 3])
                bigmm(i, 2, [2, 3, 4, 5])
                emit_psg(i, 1)
                emit_sig(i, 1)
                emit_zcopy(i, 2)
                emit_xupd(i, 1)
                if i == L - 1:
                    emit_mask_part(1)
                if i < L - 1:
                    bigmm(i + 1, 0, [0, 1])
                    emit_psh(i, 2)
                    emit_relu(i, 2)
                    emit_y(i + 1, [4, 5, 6, 7])
                    bigmm(i + 1, 0, [2, 3])
                    emit_psg(i, 2)
                    emit_sig(i, 2)
                    emit_xupd(i, 2)

            # ------- layer-4 final chunk (fine-grained halves) + epilogue ---
            # Pool computes the n=0/1 mask partials in parallel with the
            # final chunk's tail chain; the last chunk runs in two 240-col
            # halves so PE/ACT/DVE pipeline with minimal exposed latency.
            lo, hi = NS[2]
            ph4 = php.tile([HID, 512], f32, tag="ph", name="ph4_2")
            pg4 = pgp.tile([HID, 512], f32, tag="pg", name="pg4_2")
            zt4 = zsb[(L - 1, 2)]
            xm = workp.tile([HID, 240], f32, tag="xm", name="xm", bufs=1)
            i = L - 1
            nc.tensor.matmul(
                ph4[:, 0 : hi - lo], ww_all[:, HID * i : HID * (i + 1)],
                zt4[:, 0 : hi - lo], start=True, stop=True)
            nc.scalar.activation(
                h_sb[:, lo:hi], ph4[:, 0 : hi - lo], AF.Relu,
                bias=bias_all[:, i : i + 1])
            nc.tensor.matmul(
                pg4[:, 0 : hi - lo], wh_all[:, HID * i : HID * (i + 1)],
                h_sb[:, lo:hi], start=True, stop=True)
            nc.scalar.activation(
                pg4[:, 0 : hi - lo], pg4[:, 0 : hi - lo], AF.Sigmoid,
                bias=bh_all[:, i : i + 1])
            for hvi, (a, b) in enumerate([(lo, lo + 240), (lo + 240, hi)]):
                al, bl = a - lo, b - lo
                nc.vector.tensor_sub(h_sb[:, a:b], h_sb[:, a:b], xTa[0:D, a:b])
                nc.vector.tensor_mul(h_sb[:, a:b], h_sb[:, a:b], pg4[:, al:bl])
                nc.vector.tensor_add(xTa[0:D, a:b], xTa[0:D, a:b], h_sb[:, a:b])
                nc.vector.scalar_tensor_tensor(
                    out=xm[:, 0 : b - a], in0=xTa[0:D, a:b], scalar=1.0,
                    in1=mask_sb[:, a:b], op0=ALU.mult, op1=ALU.mult,
                    accum_out=ga[2 + hvi][:, :])
            nc.vector.tensor_add(ga[0][:, :], ga[0][:, :], ga[1][:, :])
            nc.vector.tensor_add(ga[2][:, :], ga[2][:, :], ga[3][:, :])
            nc.vector.tensor_add(ga[0][:, :], ga[0][:, :], ga[2][:, :])
            nc.vector.tensor_mul(ga[0][:, :], ga[0][:, :], den[:, :])
            nc.sync.dma_start(out=graphD, in_=ga[0][:, :])

    nc.compile()
    return nc


def get_nc():
    if "nc" not in _NC_CACHE:
        _NC_CACHE["nc"] = _build_nc()
    return _NC_CACHE["nc"]


def make_in_maps(adj, mask_ids, ent_emb, rel_emb, Wb, Ww, bias, Wh, bh):
    adj = np.asarray(adj, dtype=np.float32)
    pad = np.zeros((B, R, EP, E2), dtype=FP8_NP)
    pad[:, :, :E, :E] = adj.transpose(0, 1, 3, 2).astype(FP8_NP)
    # [b, r, c, p, t, i] = adj[b, r, i, j = c*256 + t*128 + p]
    adjT = pad.reshape(B, R, C2, 2, 128, E2).transpose(0, 1, 2, 4, 3, 5).copy()
    # c=5: j = 1280 + 110*t + p (110 pairs covering the 220 real rows)
    adjT[:, :, 5] = 0
    adjT[:, :, 5, 0:110] = (
        pad[:, :, 1280:1500].reshape(B, R, 2, 110, E2).transpose(0, 1, 3, 2, 4)
    )
    adjT = np.ascontiguousarray(adjT)
    entT = np.ascontiguousarray(np.asarray(ent_emb, np.float32).T)
    relTh = np.ascontiguousarray(np.asarray(rel_emb, np.float32).T)
    Wb5 = np.asarray(Wb, np.float32).reshape(L, R, 2, D, NB)
    wbx = np.ascontiguousarray(Wb5[:, :, 0].transpose(0, 2, 1, 3).reshape(L, D, RNB))
    wbr = np.ascontiguousarray(Wb5[:, :, 1].transpose(0, 2, 1, 3).reshape(L, D, RNB))
    maskf = np.asarray(mask_ids).astype(np.float32)
    common = dict(
        xT0=entT, relT=relTh, wbx=wbx, wbr=wbr,
        ww=np.ascontiguousarray(np.asarray(Ww, np.float32)),
        wh=np.ascontiguousarray(np.asarray(Wh, np.float32)),
        biasL=np.ascontiguousarray(np.asarray(bias, np.float32)),
        bhL=np.ascontiguousarray(np.asarray(bh, np.float32)),
    )
    in_maps = []
    for c in range(8):
        b = c // 2
        m = dict(common)
        m["adjT"] = np.ascontiguousarray(adjT[b])
        mrep = np.zeros((HID, E2), np.float32)
        mrep[:, :E] = np.broadcast_to(maskf[b][None, :], (HID, E))
        m["maskrep"] = mrep
        m1 = np.zeros((1, E2), np.float32)
        m1[0, :E] = maskf[b]
        m["mask1"] = m1
        in_maps.append(m)
    return in_maps


def run(inputs, trace=False):
    nc = get_nc()
    in_maps = make_in_maps(**{k: np.asarray(v) for k, v in inputs.items()})
    res = bass_utils.run_bass_kernel_spmd(
        nc, in_maps, core_ids=list(range(8)), trace=trace
    )
    out = np.stack(
        [np.asarray(res.results[2 * b]["graph"]).reshape(HID) for b in range(B)]
    ).astype(np.float32)
    return out, res


def kernel(**inputs):
    out, _ = run(inputs, trace=False)
    return out


# revision 19
# speedup vs baseline: 1.4393x; 1.0063x over previous
"""Trainium2 Bass kernel for the KGEncoder RGCN (nn_KGEncoder_14027363188782).

Math (per batch element b, L=5 layers):
    x0 = ent_emb                                             (E, D)
    per layer i:
      y_r   = x @ Wb_x[i,r] + 1 * c[i,r]^T    (E, NB)  where c[i,r] = rel_r @ Wb_rel[i,r]
      Z     = sum_r adj_r @ y_r               (E, NB)
      h     = relu(Z @ Ww[i] + bias[i])
      g     = sigmoid(h @ Wh[i] + bh[i])
      x     = x + g * (h - x)
    out_b = sum_e x[e] * m[e] / max(sum_e m[e], 1)

Sharding: core c handles b = c // 2 (pair-replicated, no collectives).
adj shipped pre-transposed, fp8 (exact for 0/1), DoubleRow layout
[r, c, p, t, i] with j = c*256 + t*128 + p.

Schedule: granular (r,c) adj DMAs; layer 0 accumulates in DMA-arrival
order (granule-major over the 3 psum i-chunks) so it finishes right at
DMA end; layers 1-4 run a software-pipelined schedule that keeps PE
busy continuously: bigmm i-chunks back-to-back, the highway tail and
the next layer's y matmuls interleaved into the bigmm instruction
stream, and the next layer's first bigmm group filling the last
tail's ACT->PE ping-pong. Tail matmuls use f32r moving operands
(1 cyc/row vs 4 for f32). The per-layer y uses an augmented
contraction row (x row 100 = 1, W row 100 = c[i]) so the relation
bias needs no extra matmul.
"""

import numpy as np
import ml_dtypes

import concourse.bacc as bacc
import concourse.bass as bass
import concourse.mybir as mybir
import concourse.tile as tile
from concourse import bass_utils
from concourse.bass import MemorySpace

B, R, E, D, HID, L, NB = 4, 10, 1500, 100, 100, 5, 3
EP = 1536           # j dim padded to 12*128
C2 = 6              # 256-row contraction chunks (DoubleRow)
E2 = 1504           # i dim padded to 16-aligned
YQ = 32             # y_all per-chunk col stride
CH = 12             # y chunks (128 j's each)
RNB = R * NB        # 30
NS = [(0, 512), (512, 1024), (1024, E2)]   # i-dim psum chunks
f32 = mybir.dt.float32
f32r = mybir.dt.float32r
fp8 = mybir.dt.float8e4
FP8_NP = ml_dtypes.float8_e4m3fn
AF = mybir.ActivationFunctionType
AX = mybir.AxisListType
ALU = mybir.AluOpType
DR = mybir.MatmulPerfMode.DoubleRow

ADJ_ORDER = [(r, c) for r in range(R) for c in range(C2)]
# y chunk k -> x column range; k=10/11 are the 110-wide halves of j 1280:1500
YCOLS = [(128 * k, 128 * (k + 1)) for k in range(10)] + [(1280, 1390), (1390, 1500)]


def _copy_groups(ks):
    # contiguous runs with uniform partition count (128 for k<10, 110 after)
    out = []
    run = [ks[0]]
    for k in ks[1:]:
        if k == run[-1] + 1 and (k < 10) == (run[0] < 10):
            run.append(k)
        else:
            out.append(run)
            run = [k]
    out.append(run)
    return [(r[0], len(r), 110 if r[0] >= 10 else 128) for r in out]

_NC_CACHE = {}


def _build_nc():
    nc = bacc.Bacc("TRN2", target_bir_lowering=False, debug=False)

    adjT = nc.dram_tensor("adjT", [R, C2, 128, 2, E2], fp8, kind="ExternalInput").ap()
    xT0 = nc.dram_tensor("xT0", [D, E], f32, kind="ExternalInput").ap()
    maskrep = nc.dram_tensor("maskrep", [HID, E2], f32, kind="ExternalInput").ap()
    mask1 = nc.dram_tensor("mask1", [1, E2], f32, kind="ExternalInput").ap()
    relT = nc.dram_tensor("relT", [D, R], f32, kind="ExternalInput").ap()
    wbxD = nc.dram_tensor("wbx", [L, D, RNB], f32, kind="ExternalInput").ap()
    wbrD = nc.dram_tensor("wbr", [L, D, RNB], f32, kind="ExternalInput").ap()
    wwD = nc.dram_tensor("ww", [L, NB, HID], f32r, kind="ExternalInput").ap()
    whD = nc.dram_tensor("wh", [L, HID, HID], f32r, kind="ExternalInput").ap()
    biasD = nc.dram_tensor("biasL", [L, HID], f32, kind="ExternalInput").ap()
    bhD = nc.dram_tensor("bhL", [L, HID], f32, kind="ExternalInput").ap()
    graphD = nc.dram_tensor("graph", [HID, 1], f32, kind="ExternalOutput").ap()

    with tile.TileContext(nc) as tc:
        with (
            tc.tile_pool(name="singles", bufs=1) as singles,
            tc.tile_pool(name="ypool", bufs=2) as ypool,
            tc.tile_pool(name="workp", bufs=2) as workp,
            tc.tile_pool(name="pzp", bufs=1, space=MemorySpace.PSUM) as pzp,
            tc.tile_pool(name="pyp", bufs=2, space=MemorySpace.PSUM) as pyp,
            tc.tile_pool(name="php", bufs=1, space=MemorySpace.PSUM) as php,
            tc.tile_pool(name="pgp", bufs=1, space=MemorySpace.PSUM) as pgp,
        ):
            # ------- preamble -------
            res_tiles = []
            for r in range(R):
                rt = singles.tile([128, C2 * 2 * E2], fp8, tag=f"res{r}",
                                  name=f"res{r}")
                res_tiles.append(rt)
            # first granule DMA precedes everything so the adj stream owns
            # HWDGE from t=0; the ACT-queue smalls queue behind a copy that
            # waits on it
            r0, c0 = ADJ_ORDER[0]
            nc.sync.dma_start(
                out=res_tiles[r0][0:128, c0 * 2 * E2 : (c0 + 1) * 2 * E2],
                in_=adjT[r0, c0, 0:128].rearrange("p t i -> p (t i)"),
            )
            hold = workp.tile([1, 1], fp8, tag="hold", name="hold", bufs=1)
            nc.scalar.copy(out=hold[:, :], in_=res_tiles[r0][0:1, 0:1])

            # batched small loads (one DMA each, ACT queue)
            relT_sb = singles.tile([D, R], f32, tag="relT", name="relT_sb")
            nc.scalar.dma_start(out=relT_sb[:, :], in_=relT)
            wbr_all = singles.tile([D, L * RNB], f32, tag="wbr", name="wbr_all")
            nc.scalar.dma_start(
                out=wbr_all[:, :].rearrange("d (l q) -> d l q", l=L),
                in_=wbrD.rearrange("l d q -> d l q")
            )
            # rows 0:100 = Wbx[i] at cols 30i; row 100 = c[i] (written below)
            wbxa = singles.tile([D + 1, L * RNB], f32, tag="wbxa", name="wbxa")
            nc.scalar.dma_start(
                out=wbxa[0:D, :].rearrange("d (l q) -> d l q", l=L),
                in_=wbxD.rearrange("l d q -> d l q")
            )
            xTa = singles.tile([D + 1, EP], f32, tag="xTa", name="xTa")
            nc.vector.memset(xTa[96 : D + 1, :], 1.0)
            nc.vector.memset(xTa[0:D, E:EP], 0.0)
            nc.scalar.dma_start(out=xTa[0:D, 0:E], in_=xT0)

            # ------- adj granule DMAs (the long pole; SP queue) -------
            # chunk c=5 covers j 1280..1499 repacked as 110 DoubleRow pairs
            # (j = 1280 + 110*t + p), so only 110 partitions ship
            for (r, c) in ADJ_ORDER[1:]:
                pp = 110 if c == C2 - 1 else 128
                nc.sync.dma_start(
                    out=res_tiles[r][0:pp, c * 2 * E2 : (c + 1) * 2 * E2],
                    in_=adjT[r, c, 0:pp].rearrange("p t i -> p (t i)"),
                )
            res_views = [
                res_tiles[r][:, :].rearrange("p (c t i) -> p c t i", c=C2, t=2)
                for r in range(R)
            ]

            # c[i, (r,q)] = rel_r @ Wbr[i, r] -> row 100 of wbxa (via ACT-queue
            # SBUF->SBUF DMA: ACT copy can't write partition base 100, and
            # the SP queue must stay free for the adj granule stream)
            psc = pyp.tile([128, 360], f32, tag="py", name="psc")
            for i in range(L):
                for r in range(R):
                    q = RNB * i + 3 * r
                    nc.tensor.matmul(
                        psc[0:1, q : q + 3],
                        relT_sb[:, r : r + 1],
                        wbr_all[:, q : q + 3],
                        start=True, stop=True,
                    )
            c_sb = workp.tile([1, L * RNB], f32, tag="c_sb", name="c_sb")
            nc.scalar.copy(out=c_sb[:, :], in_=psc[0:1, 0 : L * RNB])
            nc.scalar.dma_start(out=wbxa[D : D + 1, :], in_=c_sb[:, :])

            # preload the Sigmoid ACT table (else a 1.3us table load lands in
            # the layer-0 tail) and precompute the masked-mean denominator
            # from a [1, E2] mask row, broadcast to 100 partitions via PE --
            # all during the adj DMA window, off every critical chain
            scr = workp.tile([1, 1], f32, tag="scr", name="scr", bufs=1)
            nc.scalar.activation(scr[:, :], xTa[96:97, 0:1], AF.Sigmoid)
            # mask row lands in mask_sb row 0; the full maskrep DMA (much
            # later) overwrites it after the reduce has consumed it
            mask_sb = singles.tile([HID, E2], f32, tag="mask", name="mask_sb")
            nc.scalar.dma_start(out=mask_sb[0:1, :], in_=mask1)
            den1 = workp.tile([1, 1], f32, tag="den1", name="den1", bufs=1)
            nc.vector.reduce_sum(den1[:, :], mask_sb[0:1, :], axis=AX.X)
            nc.vector.tensor_scalar_max(den1[:, :], den1[:, :], 1.0)
            nc.vector.reciprocal(den1[:, :], den1[:, :])
            ones_h = workp.tile([1, HID], f32, tag="ones_h", name="ones_h", bufs=1)
            nc.vector.memset(ones_h[:, :], 1.0)
            nc.tensor.matmul(psc[0:HID, 200:201], ones_h[:, :], den1[:, :],
                             start=True, stop=True)
            den = workp.tile([HID, 1], f32, tag="den", name="den", bufs=1)
            nc.scalar.copy(out=den[:, :], in_=psc[0:HID, 200:201])

            # tail weights: issued after the granules (needed only once
            # layer-0's Z is complete, i.e. right at DMA end)
            ww_all = singles.tile([NB, L * HID], f32r, tag="ww", name="ww_all")
            nc.sync.dma_start(out=ww_all[:, :].rearrange("n (l h) -> n l h", l=L),
                              in_=wwD.rearrange("l n h -> n l h"))
            wh_all = singles.tile([HID, L * HID], f32r, tag="wh", name="wh_all")
            nc.sync.dma_start(out=wh_all[:, :].rearrange("p (l h) -> p l h", l=L),
                              in_=whD.rearrange("l p h -> p l h"))
            bias_all = singles.tile([HID, L], f32, tag="bias", name="bias_all")
            nc.sync.dma_start(out=bias_all[:, :], in_=biasD.rearrange("l h -> h l"))
            bh_all = singles.tile([HID, L], f32, tag="bh", name="bh_all")
            nc.sync.dma_start(out=bh_all[:, :], in_=bhD.rearrange("l h -> h l"))
            nc.sync.dma_start(out=mask_sb[:, :], in_=maskrep)

            h_sb = singles.tile([HID, E2], f32r, tag="h", name="h_sb")

            # ------- per-layer emission helpers -------
            ydict = {}    # i -> (psy tile, y_all tile, y_view)
            pzt = {}      # (i, n) -> Z psum tile
            zcnt = {}     # (i, n) -> accumulation counter
            zsb = {}      # (i, n) -> z sbuf tile
            phd = {}      # (i, n) -> psh tile
            pgd = {}      # (i, n) -> psg tile

            def emit_y(i, ks):
                if i not in ydict:
                    psy = pyp.tile([128, 360], f32, tag="py", name=f"py{i}")
                    y_all = ypool.tile([128, CH * YQ], fp8, tag="y_all",
                                       name=f"y_all{i}")
                    yv = y_all[:, :].rearrange("p (k q) -> p k q", q=YQ)
                    ydict[i] = (psy, y_all, yv)
                psy, y_all, _ = ydict[i]
                for k in ks:
                    a, b = YCOLS[k]
                    nc.tensor.matmul(
                        psy[0 : b - a, 30 * k : 30 * k + 30],
                        xTa[:, a:b],
                        wbxa[:, RNB * i : RNB * (i + 1)],
                        start=True, stop=True,
                    )
                for k0, nk, pp in _copy_groups(ks):
                    nc.scalar.copy(
                        out=y_all[0:pp, YQ * k0 : YQ * (k0 + nk)].rearrange(
                            "p (k q) -> p k q", q=YQ)[:, :, 0:RNB],
                        in_=psy[0:pp, 30 * k0 : 30 * (k0 + nk)].rearrange(
                            "p (k q) -> p k q", q=RNB),
                    )

            def bigmm(i, n, cs):
                if (i, n) not in pzt:
                    pzt[(i, n)] = pzp.tile([NB, 512], f32, tag=f"pz{n}",
                                           name=f"pz{i}_{n}")
                lo, hi = NS[n]
                nw = hi - lo
                out = pzt[(i, n)][:, 0:nw]
                yv = ydict[i][2]
                for c in cs:
                    pp = 110 if c == C2 - 1 else 128
                    for r in range(R):
                        cnt = zcnt.get((i, n), 0)
                        nc.tensor.matmul(
                            out,
                            yv[0:pp, 2 * c : 2 * c + 2, 3 * r : 3 * r + 3],
                            res_views[r][0:pp, c, :, lo:hi],
                            start=(cnt == 0),
                            stop=(cnt == R * C2 - 1),
                            perf_mode=DR,
                        )
                        zcnt[(i, n)] = cnt + 1

            def emit_zcopy(i, n):
                lo, hi = NS[n]
                nw = hi - lo
                zt = workp.tile([NB, 512], f32r, tag="z", name=f"z{i}_{n}", bufs=1)
                zsb[(i, n)] = zt
                nc.scalar.copy(out=zt[:, 0:nw], in_=pzt[(i, n)][:, 0:nw])

            def emit_psh(i, n):
                lo, hi = NS[n]
                nw = hi - lo
                ph = php.tile([HID, 512], f32, tag="ph", name=f"ph{i}_{n}")
                phd[(i, n)] = ph
                nc.tensor.matmul(
                    ph[:, 0:nw],
                    ww_all[:, HID * i : HID * (i + 1)],
                    zsb[(i, n)][:, 0:nw],
                    start=True, stop=True,
                )

            def emit_relu(i, n):
                lo, hi = NS[n]
                nc.scalar.activation(
                    h_sb[:, lo:hi], phd[(i, n)][:, 0 : hi - lo], AF.Relu,
                    bias=bias_all[:, i : i + 1],
                )

            def emit_psg(i, n):
                lo, hi = NS[n]
                nw = hi - lo
                pg = pgp.tile([HID, 512], f32, tag="pg", name=f"pg{i}_{n}")
                pgd[(i, n)] = pg
                nc.tensor.matmul(
                    pg[:, 0:nw],
                    wh_all[:, HID * i : HID * (i + 1)],
                    h_sb[:, lo:hi],
                    start=True, stop=True,
                )

            def emit_sig(i, n):
                lo, hi = NS[n]
                pg = pgd[(i, n)]
                nc.scalar.activation(
                    pg[:, 0 : hi - lo], pg[:, 0 : hi - lo], AF.Sigmoid,
                    bias=bh_all[:, i : i + 1],
                )

            def emit_xupd(i, n):
                lo, hi = NS[n]
                pg = pgd[(i, n)]
                for a in range(lo, hi, 256):
                    b = min(a + 256, hi)
                    nc.vector.tensor_sub(
                        h_sb[:, a:b], h_sb[:, a:b], xTa[0:D, a:b])
                    nc.vector.tensor_mul(
                        h_sb[:, a:b], h_sb[:, a:b], pg[:, a - lo : b - lo])
                    nc.vector.tensor_add(
                        xTa[0:D, a:b], xTa[0:D, a:b], h_sb[:, a:b])

            ga = [workp.tile([HID, 1], f32, tag=f"ga{k}", name=f"ga{k}",
                             bufs=1) for k in range(4)]
            xmp = workp.tile([HID, 512], f32, tag="xmp", name="xmp", bufs=1)

            def emit_mask_part(n):
                l2, h2 = NS[n]
                nc.vector.scalar_tensor_tensor(
                    out=xmp[:, 0 : h2 - l2], in0=xTa[0:D, l2:h2], scalar=1.0,
                    in1=mask_sb[:, l2:h2], op0=ALU.mult, op1=ALU.mult,
                    accum_out=ga[n][:, :])

            # ------- layer 0 (DMA-arrival order, granule-major) -------
            emit_y(0, [0, 1, 2, 3])
            emit_y(0, [4, 5, 6, 7])
            emit_y(0, [8, 9, 10, 11])
            for n in range(3):
                pzt[(0, n)] = pzp.tile([NB, 512], f32, tag=f"pz{n}", name=f"pz0_{n}")
            yv0 = ydict[0][2]
            for g, (r, c) in enumerate(ADJ_ORDER):
                pp = 110 if c == C2 - 1 else 128
                for n in range(3):
                    lo, hi = NS[n]
                    nc.tensor.matmul(
                        pzt[(0, n)][:, 0 : hi - lo],
                        yv0[0:pp, 2 * c : 2 * c + 2, 3 * r : 3 * r + 3],
                        res_views[r][0:pp, c, :, lo:hi],
                        start=(g == 0),
                        stop=(g == len(ADJ_ORDER) - 1),
                        perf_mode=DR,
                    )
            emit_zcopy(0, 0)
            emit_psh(0, 0)
            emit_relu(0, 0)
            emit_zcopy(0, 1)
            emit_psh(0, 1)
            emit_relu(0, 1)
            emit_psg(0, 0)
            emit_sig(0, 0)
            emit_xupd(0, 0)
            emit_y(1, [0, 1, 2, 3])
            emit_zcopy(0, 2)
            emit_psh(0, 2)
            emit_relu(0, 2)
            emit_psg(0, 1)
            emit_sig(0, 1)
            emit_xupd(0, 1)
            emit_y(1, [4, 5, 6, 7])
            emit_psg(0, 2)
            emit_sig(0, 2)
            emit_xupd(0, 2)
            bigmm(1, 0, [0, 1])
            bigmm(1, 0, [2, 3])

            # ------- layers 1..4 (software-pipelined) -------
            # entry state per layer i: bigmm(i,0,[0,1]) and y(i) k0-7 already
            # emitted by the predecessor; y(i) k8-11 still pending.
            # invariant entering layer i: bigmm(i,0,[0..3]) and y(i) k0-7
            # already emitted; y(i) k8-11 pending. Each psh/relu pair issues
            # a full bigmm group before its psg consumer so the serial ACT
            # queue (zcopy/relu/sig/y-copy, ~0.6us each) never blocks PE.
            for i in range(1, L):
                emit_y(i, [8, 9, 10, 11])
                bigmm(i, 0, [4, 5])
                emit_zcopy(i, 0)
                bigmm(i, 1, [0, 1])
                emit_psh(i, 0)
                emit_relu(i, 0)
                bigmm(i, 1, [2, 3, 4, 5])
                emit_psg(i, 0)
                emit_sig(i, 0)
                emit_zcopy(i, 1)
                emit_xupd(i, 0)
                if i == L - 1:
                    emit_mask_part(0)
                bigmm(i, 2, [0, 1])
                emit_psh(i, 1)
                emit_relu(i, 1)
                if i < L - 1:
                    emit_y(i + 1, [0, 1, 2, 3])
                    bigmm(i, 2, [2, 3, 4, 5])
                    emit_psg(i, 1)
                    emit_sig(i, 1)
                    emit_zcopy(i, 2)
                else:
                    bigmm(i, 2, [2, 3])
                    emit_psg(i, 1)
                    bigmm(i, 2, [4, 5])
                    emit_zcopy(i, 2)
                    emit_sig(i, 1)
                emit_xupd(i, 1)
                if i == L - 1:
                    emit_mask_part(1)
                if i < L - 1:
                    bigmm(i + 1, 0, [0, 1])
                    emit_psh(i, 2)
                    emit_relu(i, 2)
                    emit_y(i + 1, [4, 5, 6, 7])
                    bigmm(i + 1, 0, [2, 3])
                    emit_psg(i, 2)
                    emit_sig(i, 2)
                    emit_xupd(i, 2)

            # ------- layer-4 final chunk (fine-grained halves) + epilogue ---
            # Pool computes the n=0/1 mask partials in parallel with the
            # final chunk's tail chain; the last chunk runs in two 240-col
            # halves so PE/ACT/DVE pipeline with minimal exposed latency.
            lo, hi = NS[2]
            ph4 = php.tile([HID, 512], f32, tag="ph", name="ph4_2")
            pg4 = pgp.tile([HID, 512], f32, tag="pg", name="pg4_2")
            zt4 = zsb[(L - 1, 2)]
            xm = workp.tile([HID, 240], f32, tag="xm", name="xm", bufs=1)
            i = L - 1
            nc.tensor.matmul(
                ph4[:, 0 : hi - lo], ww_all[:, HID * i : HID * (i + 1)],
                zt4[:, 0 : hi - lo], start=True, stop=True)
            nc.scalar.activation(
                h_sb[:, lo:hi], ph4[:, 0 : hi - lo], AF.Relu,
                bias=bias_all[:, i : i + 1])
            nc.tensor.matmul(
                pg4[:, 0 : hi - lo], wh_all[:, HID * i : HID * (i + 1)],
                h_sb[:, lo:hi], start=True, stop=True)
            nc.scalar.activation(
                pg4[:, 0 : hi - lo], pg4[:, 0 : hi - lo], AF.Sigmoid,
                bias=bh_all[:, i : i + 1])
            for hvi, (a, b) in enumerate([(lo, lo + 240), (lo + 240, hi)]):
                al, bl = a - lo, b - lo
                nc.vector.tensor_sub(h_sb[:, a:b], h_sb[:, a:b], xTa[0:D, a:b])
                nc.vector.tensor_mul(h_sb[:, a:b], h_sb[:, a:b], pg4[:, al:bl])
                nc.vector.tensor_add(xTa[0:D, a:b], xTa[0:D, a:b], h_sb[:, a:b])
                nc.vector.scalar_tensor_tensor(
                    out=xm[:, 0 : b - a], in0=xTa[0:D, a:b], scalar=1.0,
                    in1=mask_sb[:, a:b], op0=ALU.mult, op1=ALU.mult,
                    accum_out=ga[2 + hvi][:, :])
            nc.vector.tensor_add(ga[0][:, :], ga[0][:, :], ga[1][:, :])
            nc.vector.tensor_add(ga[2][:, :], ga[2][:, :], ga[3][:, :])
            nc.vector.tensor_add(ga[0][:, :], ga[0][:, :], ga[2][:, :])
            nc.vector.tensor_mul(ga[0][:, :], ga[0][:, :], den[:, :])
            nc.sync.dma_start(out=graphD, in_=ga[0][:, :])

    nc.compile()
    return nc


def get_nc():
    if "nc" not in _NC_CACHE:
        _NC_CACHE["nc"] = _build_nc()
    return _NC_CACHE["nc"]


def make_in_maps(adj, mask_ids, ent_emb, rel_emb, Wb, Ww, bias, Wh, bh):
    adj = np.asarray(adj, dtype=np.float32)
    pad = np.zeros((B, R, EP, E2), dtype=FP8_NP)
    pad[:, :, :E, :E] = adj.transpose(0, 1, 3, 2).astype(FP8_NP)
    # [b, r, c, p, t, i] = adj[b, r, i, j = c*256 + t*128 + p]
    adjT = pad.reshape(B, R, C2, 2, 128, E2).transpose(0, 1, 2, 4, 3, 5).copy()
    # c=5: j = 1280 + 110*t + p (110 pairs covering the 220 real rows)
    adjT[:, :, 5] = 0
    adjT[:, :, 5, 0:110] = (
        pad[:, :, 1280:1500].reshape(B, R, 2, 110, E2).transpose(0, 1, 3, 2, 4)
    )
    adjT = np.ascontiguousarray(adjT)
    entT = np.ascontiguousarray(np.asarray(ent_emb, np.float32).T)
    relTh = np.ascontiguousarray(np.asarray(rel_emb, np.float32).T)
    Wb5 = np.asarray(Wb, np.float32).reshape(L, R, 2, D, NB)
    wbx = np.ascontiguousarray(Wb5[:, :, 0].transpose(0, 2, 1, 3).reshape(L, D, RNB))
    wbr = np.ascontiguousarray(Wb5[:, :, 1].transpose(0, 2, 1, 3).reshape(L, D, RNB))
    maskf = np.asarray(mask_ids).astype(np.float32)
    common = dict(
        xT0=entT, relT=relTh, wbx=wbx, wbr=wbr,
        ww=np.ascontiguousarray(np.asarray(Ww, np.float32)),
        wh=np.ascontiguousarray(np.asarray(Wh, np.float32)),
        biasL=np.ascontiguousarray(np.asarray(bias, np.float32)),
        bhL=np.ascontiguousarray(np.asarray(bh, np.float32)),
    )
    in_maps = []
    for c in range(8):
        b = c // 2
        m = dict(common)
        m["adjT"] = np.ascontiguousarray(adjT[b])
        mrep = np.zeros((HID, E2), np.float32)
        mrep[:, :E] = np.broadcast_to(maskf[b][None, :], (HID, E))
        m["maskrep"] = mrep
        m1 = np.zeros((1, E2), np.float32)
        m1[0, :E] = maskf[b]
        m["mask1"] = m1
        in_maps.append(m)
    return in_maps


def run(inputs, trace=False):
    nc = get_nc()
    in_maps = make_in_maps(**{k: np.asarray(v) for k, v in inputs.items()})
    res = bass_utils.run_bass_kernel_spmd(
        nc, in_maps, core_ids=list(range(8)), trace=trace
    )
    out = np.stack(
        [np.asarray(res.results[2 * b]["graph"]).reshape(HID) for b in range(B)]
    ).astype(np.float32)
    return out, res


def kernel(**inputs):
    out, _ = run(inputs, trace=False)
    return out


# revision 21
# speedup vs baseline: 1.4397x; 1.0003x over previous
"""Trainium2 Bass kernel for the KGEncoder RGCN (nn_KGEncoder_14027363188782).

Math (per batch element b, L=5 layers):
    x0 = ent_emb                                             (E, D)
    per layer i:
      y_r   = x @ Wb_x[i,r] + 1 * c[i,r]^T    (E, NB)  where c[i,r] = rel_r @ Wb_rel[i,r]
      Z     = sum_r adj_r @ y_r               (E, NB)
      h     = relu(Z @ Ww[i] + bias[i])
      g     = sigmoid(h @ Wh[i] + bh[i])
      x     = x + g * (h - x)
    out_b = sum_e x[e] * m[e] / max(sum_e m[e], 1)

Sharding: core c handles b = c // 2 (pair-replicated, no collectives).
adj shipped pre-transposed, fp8 (exact for 0/1), DoubleRow layout
[r, c, p, t, i] with j = c*256 + t*128 + p.

Schedule: granular (r,c) adj DMAs; layer 0 accumulates in DMA-arrival
order (granule-major over the 3 psum i-chunks) so it finishes right at
DMA end; layers 1-4 run a software-pipelined schedule that keeps PE
busy continuously: bigmm i-chunks back-to-back, the highway tail and
the next layer's y matmuls interleaved into the bigmm instruction
stream, and the next layer's first bigmm group filling the last
tail's ACT->PE ping-pong. Tail matmuls use f32r moving operands
(1 cyc/row vs 4 for f32). The per-layer y uses an augmented
contraction row (x row 100 = 1, W row 100 = c[i]) so the relation
bias needs no extra matmul.
"""

import numpy as np
import ml_dtypes

import concourse.bacc as bacc
import concourse.bass as bass
import concourse.mybir as mybir
import concourse.tile as tile
from concourse import bass_utils
from concourse.bass import MemorySpace

B, R, E, D, HID, L, NB = 4, 10, 1500, 100, 100, 5, 3
EP = 1536           # j dim padded to 12*128
C2 = 6              # 256-row contraction chunks (DoubleRow)
E2 = 1504           # i dim padded to 16-aligned
YQ = 32             # y_all per-chunk col stride
CH = 12             # y chunks (128 j's each)
RNB = R * NB        # 30
NS = [(0, 512), (512, 1024), (1024, E2)]   # i-dim psum chunks
f32 = mybir.dt.float32
f32r = mybir.dt.float32r
fp8 = mybir.dt.float8e4
FP8_NP = ml_dtypes.float8_e4m3fn
AF = mybir.ActivationFunctionType
AX = mybir.AxisListType
ALU = mybir.AluOpType
DR = mybir.MatmulPerfMode.DoubleRow

ADJ_ORDER = [(r, c) for r in range(R) for c in range(C2)]
# y chunk k -> x column range; k=10/11 are the 110-wide halves of j 1280:1500
YCOLS = [(128 * k, 128 * (k + 1)) for k in range(10)] + [(1280, 1390), (1390, 1500)]


def _copy_groups(ks):
    # contiguous runs with uniform partition count (128 for k<10, 110 after)
    out = []
    run = [ks[0]]
    for k in ks[1:]:
        if k == run[-1] + 1 and (k < 10) == (run[0] < 10):
            run.append(k)
        else:
            out.append(run)
            run = [k]
    out.append(run)
    return [(r[0], len(r), 110 if r[0] >= 10 else 128) for r in out]

_NC_CACHE = {}


def _build_nc():
    nc = bacc.Bacc("TRN2", target_bir_lowering=False, debug=False)

    adjT = nc.dram_tensor("adjT", [R, C2, 128, 2, E2], fp8, kind="ExternalInput").ap()
    xT0 = nc.dram_tensor("xT0", [D, E], f32, kind="ExternalInput").ap()
    maskrep = nc.dram_tensor("maskrep", [HID, E2], f32, kind="ExternalInput").ap()
    mask1 = nc.dram_tensor("mask1", [1, E2], f32, kind="ExternalInput").ap()
    relT = nc.dram_tensor("relT", [D, R], f32, kind="ExternalInput").ap()
    wbxD = nc.dram_tensor("wbx", [L, D, RNB], f32, kind="ExternalInput").ap()
    wbrD = nc.dram_tensor("wbr", [L, D, RNB], f32, kind="ExternalInput").ap()
    wwD = nc.dram_tensor("ww", [L, NB, HID], f32r, kind="ExternalInput").ap()
    whD = nc.dram_tensor("wh", [L, HID, HID], f32r, kind="ExternalInput").ap()
    biasD = nc.dram_tensor("biasL", [L, HID], f32, kind="ExternalInput").ap()
    bhD = nc.dram_tensor("bhL", [L, HID], f32, kind="ExternalInput").ap()
    graphD = nc.dram_tensor("graph", [HID, 1], f32, kind="ExternalOutput").ap()

    with tile.TileContext(nc) as tc:
        with (
            tc.tile_pool(name="singles", bufs=1) as singles,
            tc.tile_pool(name="ypool", bufs=2) as ypool,
            tc.tile_pool(name="workp", bufs=2) as workp,
            tc.tile_pool(name="pzp", bufs=1, space=MemorySpace.PSUM) as pzp,
            tc.tile_pool(name="pyp", bufs=2, space=MemorySpace.PSUM) as pyp,
            tc.tile_pool(name="php", bufs=1, space=MemorySpace.PSUM) as php,
            tc.tile_pool(name="pgp", bufs=1, space=MemorySpace.PSUM) as pgp,
        ):
            # ------- preamble -------
            res_tiles = []
            for r in range(R):
                rt = singles.tile([128, C2 * 2 * E2], fp8, tag=f"res{r}",
                                  name=f"res{r}")
                res_tiles.append(rt)
            # first granule DMA precedes everything so the adj stream owns
            # HWDGE from t=0; the ACT-queue smalls queue behind a copy that
            # waits on it
            r0, c0 = ADJ_ORDER[0]
            nc.sync.dma_start(
                out=res_tiles[r0][0:128, c0 * 2 * E2 : (c0 + 1) * 2 * E2],
                in_=adjT[r0, c0, 0:128].rearrange("p t i -> p (t i)"),
            )
            hold = workp.tile([1, 1], fp8, tag="hold", name="hold", bufs=1)
            nc.scalar.copy(out=hold[:, :], in_=res_tiles[r0][0:1, 0:1])

            # batched small loads (one DMA each, ACT queue)
            relT_sb = singles.tile([D, R], f32, tag="relT", name="relT_sb")
            nc.scalar.dma_start(out=relT_sb[:, :], in_=relT)
            wbr_all = singles.tile([D, L * RNB], f32, tag="wbr", name="wbr_all")
            nc.scalar.dma_start(
                out=wbr_all[:, :].rearrange("d (l q) -> d l q", l=L),
                in_=wbrD.rearrange("l d q -> d l q")
            )
            # rows 0:100 = Wbx[i] at cols 30i; row 100 = c[i] (written below)
            wbxa = singles.tile([D + 1, L * RNB], f32, tag="wbxa", name="wbxa")
            nc.scalar.dma_start(
                out=wbxa[0:D, :].rearrange("d (l q) -> d l q", l=L),
                in_=wbxD.rearrange("l d q -> d l q")
            )
            xTa = singles.tile([D + 1, EP], f32, tag="xTa", name="xTa")
            nc.vector.memset(xTa[96 : D + 1, :], 1.0)
            nc.vector.memset(xTa[0:D, E:EP], 0.0)
            nc.scalar.dma_start(out=xTa[0:D, 0:E], in_=xT0)

            # ------- adj granule DMAs (the long pole; SP queue) -------
            # chunk c=5 covers j 1280..1499 repacked as 110 DoubleRow pairs
            # (j = 1280 + 110*t + p), so only 110 partitions ship
            for (r, c) in ADJ_ORDER[1:]:
                pp = 110 if c == C2 - 1 else 128
                nc.sync.dma_start(
                    out=res_tiles[r][0:pp, c * 2 * E2 : (c + 1) * 2 * E2],
                    in_=adjT[r, c, 0:pp].rearrange("p t i -> p (t i)"),
                )
            res_views = [
                res_tiles[r][:, :].rearrange("p (c t i) -> p c t i", c=C2, t=2)
                for r in range(R)
            ]

            # c[i, (r,q)] = rel_r @ Wbr[i, r] -> row 100 of wbxa (via ACT-queue
            # SBUF->SBUF DMA: ACT copy can't write partition base 100, and
            # the SP queue must stay free for the adj granule stream)
            psc = pyp.tile([128, 360], f32, tag="py", name="psc")
            for i in range(L):
                for r in range(R):
                    q = RNB * i + 3 * r
                    nc.tensor.matmul(
                        psc[0:1, q : q + 3],
                        relT_sb[:, r : r + 1],
                        wbr_all[:, q : q + 3],
                        start=True, stop=True,
                    )
            c_sb = workp.tile([1, L * RNB], f32, tag="c_sb", name="c_sb")
            nc.scalar.copy(out=c_sb[:, :], in_=psc[0:1, 0 : L * RNB])
            nc.scalar.dma_start(out=wbxa[D : D + 1, :], in_=c_sb[:, :])

            # preload the Sigmoid ACT table (else a 1.3us table load lands in
            # the layer-0 tail) and precompute the masked-mean denominator
            # from a [1, E2] mask row, broadcast to 100 partitions via PE --
            # all during the adj DMA window, off every critical chain
            scr = workp.tile([1, 1], f32, tag="scr", name="scr", bufs=1)
            nc.scalar.activation(scr[:, :], xTa[96:97, 0:1], AF.Sigmoid)
            # mask row lands in mask_sb row 0; the full maskrep DMA (much
            # later) overwrites it after the reduce has consumed it
            mask_sb = singles.tile([HID, E2], f32, tag="mask", name="mask_sb")
            nc.scalar.dma_start(out=mask_sb[0:1, :], in_=mask1)
            den1 = workp.tile([1, 1], f32, tag="den1", name="den1", bufs=1)
            nc.vector.reduce_sum(den1[:, :], mask_sb[0:1, :], axis=AX.X)
            nc.vector.tensor_scalar_max(den1[:, :], den1[:, :], 1.0)
            nc.vector.reciprocal(den1[:, :], den1[:, :])
            ones_h = workp.tile([1, HID], f32, tag="ones_h", name="ones_h", bufs=1)
            nc.vector.memset(ones_h[:, :], 1.0)
            nc.tensor.matmul(psc[0:HID, 200:201], ones_h[:, :], den1[:, :],
                             start=True, stop=True)
            den = workp.tile([HID, 1], f32, tag="den", name="den", bufs=1)
            nc.scalar.copy(out=den[:, :], in_=psc[0:HID, 200:201])

            # tail weights: issued after the granules (needed only once
            # layer-0's Z is complete, i.e. right at DMA end)
            ww_all = singles.tile([NB, L * HID], f32r, tag="ww", name="ww_all")
            nc.sync.dma_start(out=ww_all[:, :].rearrange("n (l h) -> n l h", l=L),
                              in_=wwD.rearrange("l n h -> n l h"))
            wh_all = singles.tile([HID, L * HID], f32r, tag="wh", name="wh_all")
            nc.sync.dma_start(out=wh_all[:, :].rearrange("p (l h) -> p l h", l=L),
                              in_=whD.rearrange("l p h -> p l h"))
            bias_all = singles.tile([HID, L], f32, tag="bias", name="bias_all")
            nc.sync.dma_start(out=bias_all[:, :], in_=biasD.rearrange("l h -> h l"))
            bh_all = singles.tile([HID, L], f32, tag="bh", name="bh_all")
            nc.sync.dma_start(out=bh_all[:, :], in_=bhD.rearrange("l h -> h l"))
            nc.sync.dma_start(out=mask_sb[:, :], in_=maskrep)

            h_sb = singles.tile([HID, E2], f32r, tag="h", name="h_sb")

            # ------- per-layer emission helpers -------
            ydict = {}    # i -> (psy tile, y_all tile, y_view)
            pzt = {}      # (i, n) -> Z psum tile
            zcnt = {}     # (i, n) -> accumulation counter
            zsb = {}      # (i, n) -> z sbuf tile
            phd = {}      # (i, n) -> psh tile
            pgd = {}      # (i, n) -> psg tile

            def emit_y(i, ks):
                if i not in ydict:
                    psy = pyp.tile([128, 360], f32, tag="py", name=f"py{i}")
                    y_all = ypool.tile([128, CH * YQ], fp8, tag="y_all",
                                       name=f"y_all{i}")
                    yv = y_all[:, :].rearrange("p (k q) -> p k q", q=YQ)
                    ydict[i] = (psy, y_all, yv)
                psy, y_all, _ = ydict[i]
                for k in ks:
                    a, b = YCOLS[k]
                    nc.tensor.matmul(
                        psy[0 : b - a, 30 * k : 30 * k + 30],
                        xTa[:, a:b],
                        wbxa[:, RNB * i : RNB * (i + 1)],
                        start=True, stop=True,
                    )
                for k0, nk, pp in _copy_groups(ks):
                    nc.scalar.copy(
                        out=y_all[0:pp, YQ * k0 : YQ * (k0 + nk)].rearrange(
                            "p (k q) -> p k q", q=YQ)[:, :, 0:RNB],
                        in_=psy[0:pp, 30 * k0 : 30 * (k0 + nk)].rearrange(
                            "p (k q) -> p k q", q=RNB),
                    )

            def bigmm(i, n, cs):
                if (i, n) not in pzt:
                    pzt[(i, n)] = pzp.tile([NB, 512], f32, tag=f"pz{n}",
                                           name=f"pz{i}_{n}")
                lo, hi = NS[n]
                nw = hi - lo
                out = pzt[(i, n)][:, 0:nw]
                yv = ydict[i][2]
                for c in cs:
                    pp = 110 if c == C2 - 1 else 128
                    for r in range(R):
                        cnt = zcnt.get((i, n), 0)
                        nc.tensor.matmul(
                            out,
                            yv[0:pp, 2 * c : 2 * c + 2, 3 * r : 3 * r + 3],
                            res_views[r][0:pp, c, :, lo:hi],
                            start=(cnt == 0),
                            stop=(cnt == R * C2 - 1),
                            perf_mode=DR,
                        )
                        zcnt[(i, n)] = cnt + 1

            def emit_zcopy(i, n):
                lo, hi = NS[n]
                nw = hi - lo
                zt = workp.tile([NB, 512], f32r, tag="z", name=f"z{i}_{n}", bufs=1)
                zsb[(i, n)] = zt
                nc.scalar.copy(out=zt[:, 0:nw], in_=pzt[(i, n)][:, 0:nw])

            def emit_psh(i, n):
                lo, hi = NS[n]
                nw = hi - lo
                ph = php.tile([HID, 512], f32, tag="ph", name=f"ph{i}_{n}")
                phd[(i, n)] = ph
                nc.tensor.matmul(
                    ph[:, 0:nw],
                    ww_all[:, HID * i : HID * (i + 1)],
                    zsb[(i, n)][:, 0:nw],
                    start=True, stop=True,
                )

            def emit_relu(i, n):
                lo, hi = NS[n]
                nc.scalar.activation(
                    h_sb[:, lo:hi], phd[(i, n)][:, 0 : hi - lo], AF.Relu,
                    bias=bias_all[:, i : i + 1],
                )

            def emit_psg(i, n):
                lo, hi = NS[n]
                nw = hi - lo
                pg = pgp.tile([HID, 512], f32, tag="pg", name=f"pg{i}_{n}")
                pgd[(i, n)] = pg
                nc.tensor.matmul(
                    pg[:, 0:nw],
                    wh_all[:, HID * i : HID * (i + 1)],
                    h_sb[:, lo:hi],
                    start=True, stop=True,
                )

            def emit_sig(i, n):
                lo, hi = NS[n]
                pg = pgd[(i, n)]
                nc.scalar.activation(
                    pg[:, 0 : hi - lo], pg[:, 0 : hi - lo], AF.Sigmoid,
                    bias=bh_all[:, i : i + 1],
                )

            def emit_xupd(i, n):
                lo, hi = NS[n]
                pg = pgd[(i, n)]
                for a in range(lo, hi, 256):
                    b = min(a + 256, hi)
                    nc.vector.tensor_sub(
                        h_sb[:, a:b], h_sb[:, a:b], xTa[0:D, a:b])
                    nc.vector.tensor_mul(
                        h_sb[:, a:b], h_sb[:, a:b], pg[:, a - lo : b - lo])
                    nc.vector.tensor_add(
                        xTa[0:D, a:b], xTa[0:D, a:b], h_sb[:, a:b])

            ga = [workp.tile([HID, 1], f32, tag=f"ga{k}", name=f"ga{k}",
                             bufs=1) for k in range(4)]
            xmp = workp.tile([HID, 512], f32, tag="xmp", name="xmp", bufs=1)

            def emit_mask_part(n):
                l2, h2 = NS[n]
                nc.vector.scalar_tensor_tensor(
                    out=xmp[:, 0 : h2 - l2], in0=xTa[0:D, l2:h2], scalar=1.0,
                    in1=mask_sb[:, l2:h2], op0=ALU.mult, op1=ALU.mult,
                    accum_out=ga[n][:, :])

            jt = pyp.tile([128, 360], f32, tag="py", name="jt")

            def warm(n):
                for _ in range(n):
                    nc.tensor.matmul(
                        jt[0:3, 0:360],
                        ydict[0][2][0:128, 0:2, 0:3],
                        res_views[0][0:128, 0, :, 0:360],
                        start=True, stop=True,
                        perf_mode=DR,
                    )

            # ------- layer 0 (DMA-arrival order, granule-major) -------
            emit_y(0, [0, 1, 2, 3])
            emit_y(0, [4, 5, 6, 7])
            emit_y(0, [8, 9, 10, 11])
            for n in range(3):
                pzt[(0, n)] = pzp.tile([NB, 512], f32, tag=f"pz{n}", name=f"pz0_{n}")
            yv0 = ydict[0][2]
            for g, (r, c) in enumerate(ADJ_ORDER):
                pp = 110 if c == C2 - 1 else 128
                for n in range(3):
                    lo, hi = NS[n]
                    nc.tensor.matmul(
                        pzt[(0, n)][:, 0 : hi - lo],
                        yv0[0:pp, 2 * c : 2 * c + 2, 3 * r : 3 * r + 3],
                        res_views[r][0:pp, c, :, lo:hi],
                        start=(g == 0),
                        stop=(g == len(ADJ_ORDER) - 1),
                        perf_mode=DR,
                    )
                if g >= len(ADJ_ORDER) - 4:
                    warm(5)
            emit_zcopy(0, 0)
            emit_zcopy(0, 1)
            warm(10)
            emit_psh(0, 0)
            emit_relu(0, 0)
            warm(5)
            emit_psh(0, 1)
            emit_relu(0, 1)
            emit_psg(0, 0)
            emit_sig(0, 0)
            emit_xupd(0, 0)
            warm(9)
            emit_y(1, [0, 1])
            bigmm(1, 0, [0])
            emit_y(1, [2, 3])
            bigmm(1, 0, [1])
            emit_zcopy(0, 2)
            emit_psh(0, 2)
            emit_relu(0, 2)
            emit_psg(0, 1)
            emit_sig(0, 1)
            emit_xupd(0, 1)
            emit_y(1, [4, 5])
            bigmm(1, 0, [2])
            emit_y(1, [6, 7])
            bigmm(1, 0, [3])
            emit_psg(0, 2)
            emit_sig(0, 2)
            emit_xupd(0, 2)

            # ------- layers 1..4 (software-pipelined) -------
            # entry state per layer i: bigmm(i,0,[0,1]) and y(i) k0-7 already
            # emitted by the predecessor; y(i) k8-11 still pending.
            # invariant entering layer i: bigmm(i,0,[0..3]) and y(i) k0-7
            # already emitted; y(i) k8-11 pending. Each psh/relu pair issues
            # a full bigmm group before its psg consumer so the serial ACT
            # queue (zcopy/relu/sig/y-copy, ~0.6us each) never blocks PE.
            for i in range(1, L):
                emit_y(i, [8, 9, 10, 11])
                bigmm(i, 0, [4, 5])
                emit_zcopy(i, 0)
                bigmm(i, 1, [0, 1])
                emit_psh(i, 0)
                emit_relu(i, 0)
                bigmm(i, 1, [2, 3, 4, 5])
                emit_psg(i, 0)
                emit_sig(i, 0)
                emit_zcopy(i, 1)
                emit_xupd(i, 0)
                if i == L - 1:
                    emit_mask_part(0)
                bigmm(i, 2, [0, 1])
                emit_psh(i, 1)
                emit_relu(i, 1)
                if i < L - 1:
                    emit_y(i + 1, [0, 1, 2, 3])
                    bigmm(i, 2, [2, 3, 4, 5])
                    emit_psg(i, 1)
                    emit_sig(i, 1)
                    emit_zcopy(i, 2)
                else:
                    bigmm(i, 2, [2, 3])
                    emit_psg(i, 1)
                    bigmm(i, 2, [4, 5])
                    emit_zcopy(i, 2)
                    emit_sig(i, 1)
                emit_xupd(i, 1)
                if i == L - 1:
                    emit_mask_part(1)
                if i < L - 1:
                    bigmm(i + 1, 0, [0, 1])
                    emit_psh(i, 2)
                    emit_relu(i, 2)
                    emit_y(i + 1, [4, 5, 6, 7])
                    bigmm(i + 1, 0, [2, 3])
                    emit_psg(i, 2)
                    emit_sig(i, 2)
                    emit_xupd(i, 2)

            # ------- layer-4 final chunk (fine-grained halves) + epilogue ---
            # Pool computes the n=0/1 mask partials in parallel with the
            # final chunk's tail chain; the last chunk runs in two 240-col
            # halves so PE/ACT/DVE pipeline with minimal exposed latency.
            lo, hi = NS[2]
            ph4 = php.tile([HID, 512], f32, tag="ph", name="ph4_2")
            pg4 = pgp.tile([HID, 512], f32, tag="pg", name="pg4_2")
            zt4 = zsb[(L - 1, 2)]
            xm = workp.tile([HID, 240], f32, tag="xm", name="xm", bufs=1)
            i = L - 1
            nc.tensor.matmul(
                ph4[:, 0 : hi - lo], ww_all[:, HID * i : HID * (i + 1)],
                zt4[:, 0 : hi - lo], start=True, stop=True)
            nc.scalar.activation(
                h_sb[:, lo:hi], ph4[:, 0 : hi - lo], AF.Relu,
                bias=bias_all[:, i : i + 1])
            nc.tensor.matmul(
                pg4[:, 0 : hi - lo], wh_all[:, HID * i : HID * (i + 1)],
                h_sb[:, lo:hi], start=True, stop=True)
            for hvi, (a, b) in enumerate([(lo, lo + 240), (lo + 240, hi)]):
                al, bl = a - lo, b - lo
                nc.scalar.activation(
                    pg4[:, al:bl], pg4[:, al:bl], AF.Sigmoid,
                    bias=bh_all[:, i : i + 1])
                nc.vector.tensor_sub(h_sb[:, a:b], h_sb[:, a:b], xTa[0:D, a:b])
                nc.vector.tensor_mul(h_sb[:, a:b], h_sb[:, a:b], pg4[:, al:bl])
                nc.vector.tensor_add(xTa[0:D, a:b], xTa[0:D, a:b], h_sb[:, a:b])
                nc.vector.scalar_tensor_tensor(
                    out=xm[:, 0 : b - a], in0=xTa[0:D, a:b], scalar=1.0,
                    in1=mask_sb[:, a:b], op0=ALU.mult, op1=ALU.mult,
                    accum_out=ga[2 + hvi][:, :])
            nc.vector.tensor_add(ga[0][:, :], ga[0][:, :], ga[1][:, :])
            nc.vector.tensor_add(ga[2][:, :], ga[2][:, :], ga[3][:, :])
            nc.vector.tensor_add(ga[0][:, :], ga[0][:, :], ga[2][:, :])
            nc.vector.tensor_mul(ga[0][:, :], ga[0][:, :], den[:, :])
            nc.sync.dma_start(out=graphD, in_=ga[0][:, :])

    nc.compile()
    return nc


def get_nc():
    if "nc" not in _NC_CACHE:
        _NC_CACHE["nc"] = _build_nc()
    return _NC_CACHE["nc"]


def make_in_maps(adj, mask_ids, ent_emb, rel_emb, Wb, Ww, bias, Wh, bh):
    adj = np.asarray(adj, dtype=np.float32)
    pad = np.zeros((B, R, EP, E2), dtype=FP8_NP)
    pad[:, :, :E, :E] = adj.transpose(0, 1, 3, 2).astype(FP8_NP)
    # [b, r, c, p, t, i] = adj[b, r, i, j = c*256 + t*128 + p]
    adjT = pad.reshape(B, R, C2, 2, 128, E2).transpose(0, 1, 2, 4, 3, 5).copy()
    # c=5: j = 1280 + 110*t + p (110 pairs covering the 220 real rows)
    adjT[:, :, 5] = 0
    adjT[:, :, 5, 0:110] = (
        pad[:, :, 1280:1500].reshape(B, R, 2, 110, E2).transpose(0, 1, 3, 2, 4)
    )
    adjT = np.ascontiguousarray(adjT)
    entT = np.ascontiguousarray(np.asarray(ent_emb, np.float32).T)
    relTh = np.ascontiguousarray(np.asarray(rel_emb, np.float32).T)
    Wb5 = np.asarray(Wb, np.float32).reshape(L, R, 2, D, NB)
    wbx = np.ascontiguousarray(Wb5[:, :, 0].transpose(0, 2, 1, 3).reshape(L, D, RNB))
    wbr = np.ascontiguousarray(Wb5[:, :, 1].transpose(0, 2, 1, 3).reshape(L, D, RNB))
    maskf = np.asarray(mask_ids).astype(np.float32)
    common = dict(
        xT0=entT, relT=relTh, wbx=wbx, wbr=wbr,
        ww=np.ascontiguousarray(np.asarray(Ww, np.float32)),
        wh=np.ascontiguousarray(np.asarray(Wh, np.float32)),
        biasL=np.ascontiguousarray(np.asarray(bias, np.float32)),
        bhL=np.ascontiguousarray(np.asarray(bh, np.float32)),
    )
    in_maps = []
    for c in range(8):
        b = c // 2
        m = dict(common)
        m["adjT"] = np.ascontiguousarray(adjT[b])
        mrep = np.zeros((HID, E2), np.float32)
        mrep[:, :E] = np.broadcast_to(maskf[b][None, :], (HID, E))
        m["maskrep"] = mrep
        m1 = np.zeros((1, E2), np.float32)
        m1[0, :E] = maskf[b]
        m["mask1"] = m1
        in_maps.append(m)
    return in_maps


def run(inputs, trace=False):
    nc = get_nc()
    in_maps = make_in_maps(**{k: np.asarray(v) for k, v in inputs.items()})
    res = bass_utils.run_bass_kernel_spmd(
        nc, in_maps, core_ids=list(range(8)), trace=trace
    )
    out = np.stack(
        [np.asarray(res.results[2 * b]["graph"]).reshape(HID) for b in range(B)]
    ).astype(np.float32)
    return out, res


def kernel(**inputs):
    out, _ = run(inputs, trace=False)
    return out
